# revision 1
# baseline (speedup 1.0000x reference)
"""Trainium2 Bass kernel for nn_EquivariantNeuralField.

Per-pixel top-4-nearest-latent cross-attention neural field.
Sharding: 8 cores; core i handles batch i//4, pixel rows (i%4)*4096..+4096.

Per-core pipeline (32 chunks of 128 pixels):
  A. exact-fp32 squared distances zx[n,l] (DVE), top-4 via max8+max_index
  B. one-hot selection matrices (bf16), PE-transposed -> gather matmuls
     (tables in bf16 hi/lo two-pass for ~2^-17 accuracy)
  C. sin-embedding features with range reduction (round-cast), MLPs on PE
     in fp32r, fused attention (softmax pixel-major), output MLP.
"""
import numpy as np

B, N, L, K = 2, 16384, 256, 4
DIN, DOUT, DLAT, H, A, NH = 2, 3, 64, 128, 32, 4
NCORE = 8
NPC = N * B // NCORE          # pixels per core = 4096
CHUNK = 128
PI = float(np.pi)

_cache = {}


def _build(nchunk, debug=False):
    import concourse.bacc as bacc
    import concourse.mybir as mybir
    from concourse.tile import TileContext

    F32 = mybir.dt.float32
    F32R = mybir.dt.float32r
    BF16 = mybir.dt.bfloat16
    I32 = mybir.dt.int32
    U32 = mybir.dt.uint32
    AF = mybir.ActivationFunctionType
    OP = mybir.AluOpType

    nc = bacc.Bacc()

    # ---------------- DRAM tensors ----------------
    xd = nc.dram_tensor("x", [NPC, DIN], F32, kind="ExternalInput")
    pd = nc.dram_tensor("p", [L, DIN], F32, kind="ExternalInput")
    cd = nc.dram_tensor("c", [L, DLAT], F32, kind="ExternalInput")
    gd = nc.dram_tensor("g", [L, 1], F32, kind="ExternalInput")
    W_stem = nc.dram_tensor("W_stem", [DLAT, H], F32, kind="ExternalInput")
    b_stem = nc.dram_tensor("b_stem", [H], F32, kind="ExternalInput")
    Wq_sin = nc.dram_tensor("Wq_sin", [DIN, H // 2], F32, kind="ExternalInput")
    Wq1 = nc.dram_tensor("Wq1", [H + DIN, H], F32, kind="ExternalInput")
    bq1 = nc.dram_tensor("bq1", [H], F32, kind="ExternalInput")
    Wq2 = nc.dram_tensor("Wq2", [H, NH * A], F32, kind="ExternalInput")
    bq2 = nc.dram_tensor("bq2", [NH * A], F32, kind="ExternalInput")
    Wv_sin = nc.dram_tensor("Wv_sin", [DIN, H // 2], F32, kind="ExternalInput")
    Wv1 = nc.dram_tensor("Wv1", [H + DIN, H], F32, kind="ExternalInput")
    bv1 = nc.dram_tensor("bv1", [H], F32, kind="ExternalInput")
    Wv2 = nc.dram_tensor("Wv2", [H, 2 * H], F32, kind="ExternalInput")
    bv2 = nc.dram_tensor("bv2", [2 * H], F32, kind="ExternalInput")
    Wk = nc.dram_tensor("Wk", [H, NH * A], F32, kind="ExternalInput")
    bk = nc.dram_tensor("bk", [NH * A], F32, kind="ExternalInput")
    Wv = nc.dram_tensor("Wv", [H, NH * H], F32, kind="ExternalInput")
    bv = nc.dram_tensor("bv", [NH * H], F32, kind="ExternalInput")
    Wo1 = nc.dram_tensor("Wo1", [NH * H, NH * H], F32, kind="ExternalInput")
    bo1 = nc.dram_tensor("bo1", [NH * H], F32, kind="ExternalInput")
    Wo2 = nc.dram_tensor("Wo2", [NH * H, DOUT], F32, kind="ExternalInput")
    bo2 = nc.dram_tensor("bo2", [DOUT], F32, kind="ExternalInput")
    outd = nc.dram_tensor("out", [NPC, DOUT], F32, kind="ExternalOutput")
    dbg = {}
    if debug:
        for nm, shp in [("nzx", [128, 256]), ("m8", [128, 8]), ("idxf", [128, 8]),
                        ("c_kT", [128, 512]), ("k_kT", [128, 512]), ("smT", [3, 512]),
                        ("relp", [2, 512]), ("fe", [128, 512]), ("S", [128, 512]),
                        ("Ct", [128, 512]), ("sincc", [2, 512]), ("h1q", [128, 512]),
                        ("qk", [128, 512]), ("u", [128, 512]), ("lgpm", [128, 16]),
                        ("att_pm", [128, 16]), ("uw", [128, 2048]), ("y_bf", [128, 512]), ("att_flat", [1, 2048]), ("attB0", [128, 512]), ("att_sh", [16, 128]),
                        ("y1", [128, 512]), ("ohT0", [128, 512])]:
            dbg[nm] = nc.dram_tensor("dbg_" + nm, shp, F32, kind="ExternalOutput")

    with TileContext(nc) as tc:
        with tc.tile_pool(name="const", bufs=1) as cpool, \
             tc.tile_pool(name="work", bufs=2) as wpool, \
             tc.tile_pool(name="psA", bufs=3, space="PSUM") as psA, \
             tc.tile_pool(name="psB", bufs=2, space="PSUM") as psB, \
             tc.tile_pool(name="psS", bufs=3, space="PSUM") as psS, \
             tc.tile_pool(name="drp", bufs=2, space="DRAM") as drpool:

            # ============ one-time constants ============
            # iota row 0..255 (f32) and identities
            iota_i = cpool.tile([128, 256], I32)
            nc.gpsimd.iota(iota_i[:], [[1, 256]], base=0, channel_multiplier=0)
            iota_f = cpool.tile([128, 256], F32)
            nc.vector.tensor_copy(iota_f[:], iota_i[:])
            idn_i = cpool.tile([128, 128], I32)
            nc.gpsimd.iota(idn_i[:], [[1, 128]], base=0, channel_multiplier=-1)
            idn_f0 = cpool.tile([128, 128], F32)
            nc.vector.tensor_copy(idn_f0[:], idn_i[:])
            ident = cpool.tile([128, 128], F32)
            nc.vector.tensor_scalar(ident[:], idn_f0[:], 0.0, None, OP.is_equal)
            ident_bf = cpool.tile([128, 128], BF16)
            nc.vector.tensor_copy(ident_bf[:], ident[:])
            ones16_bf = cpool.tile([16, 128], BF16)
            nc.gpsimd.memset(ones16_bf[:], 1.0)
            # blockones [128, NH] f32r : bo[c, h] = (c//A == h)
            blockones_f = cpool.tile([128, NH], F32)
            nc.gpsimd.memset(blockones_f[:], 0.0)
            for h in range(NH):
                nc.gpsimd.memset(blockones_f[h * A:(h + 1) * A, h:h + 1], 1.0)
            blockones = cpool.tile([128, NH], F32R)
            nc.vector.tensor_copy(blockones[:], blockones_f[:])
            halfpi = cpool.tile([128, 1], F32)
            nc.gpsimd.memset(halfpi[:], PI / 2.0)

            # ============ weights ============
            def load_cast(dram_ap, shape, dt, tag):
                if dt == F32:
                    t0 = cpool.tile(shape, F32, tag=tag + "_f32", name=tag)
                    nc.sync.dma_start(t0[:], dram_ap)
                    return t0
                t0 = wpool.tile([128, 512], F32, tag="stage", name="stage_" + tag)
                nc.sync.dma_start(t0[0:shape[0], 0:shape[1]], dram_ap)
                t1 = cpool.tile(shape, dt, tag=tag)
                nc.vector.tensor_copy(t1[:], t0[0:shape[0], 0:shape[1]])
                return t1

            def load_bias(dram, n, tag):
                if n <= 128:
                    t = cpool.tile([n, 1], F32, tag=tag)
                    nc.sync.dma_start(t[:], dram[:].rearrange("(n o) -> n o", o=1))
                    return t
                k = n // 128
                t = cpool.tile([128, k], F32, tag=tag)
                nc.sync.dma_start(t[:], dram[:].rearrange("(j p) -> p j", p=128))
                return t

            Wstem_t = load_cast(W_stem[:], [DLAT, H], F32, "wstem")
            Wqsin_t = load_cast(Wq_sin[:], [DIN, H // 2], F32, "wqsin")
            Wvsin_t = load_cast(Wv_sin[:], [DIN, H // 2], F32, "wvsin")
            Wq1_cc = load_cast(Wq1[0:DIN, :], [DIN, H], F32R, "wq1cc")
            Wq1_sin = load_cast(Wq1[DIN:DIN + 64, :], [64, H], F32R, "wq1sin")
            Wq1_cos = load_cast(Wq1[DIN + 64:DIN + 128, :], [64, H], F32R, "wq1cos")
            Wv1_cc = load_cast(Wv1[0:DIN, :], [DIN, H], F32R, "wv1cc")
            Wv1_sf = cpool.tile([128, H], F32, tag="wv1sf")
            nc.sync.dma_start(Wv1_sf[64:128, :], Wv1[DIN:DIN + 64, :])
            Wv1_sin_t = cpool.tile([128, H], F32R, tag="wv1sin")
            nc.vector.tensor_copy(Wv1_sin_t[64:128, :], Wv1_sf[64:128, :])
            Wv1_cf = cpool.tile([128, H], F32, tag="wv1cf")
            nc.sync.dma_start(Wv1_cf[64:128, :], Wv1[DIN + 64:DIN + 128, :])
            Wv1_cos_t = cpool.tile([128, H], F32R, tag="wv1cos")
            nc.vector.tensor_copy(Wv1_cos_t[64:128, :], Wv1_cf[64:128, :])
            Wv1_sin = Wv1_sin_t[64:128, :]
            Wv1_cos = Wv1_cos_t[64:128, :]
            Wq2_t = load_cast(Wq2[:], [H, NH * A], F32R, "wq2")
            Wv2_t = load_cast(Wv2[:], [H, 2 * H], F32R, "wv2")
            Wk_t = load_cast(Wk[:], [H, NH * A], F32, "wk")
            Wv_bf = load_cast(Wv[:], [H, NH * H], BF16, "wv")
            # Wo1 as [128, (c2, f) 2048] bf16
            Wo1_f32 = cpool.tile([128, 4 * 512], F32, tag="wo1f")
            for c2 in range(4):
                nc.sync.dma_start(Wo1_f32[:, c2 * 512:(c2 + 1) * 512],
                                  Wo1[c2 * 128:(c2 + 1) * 128, :])
            Wo1_bf = cpool.tile([128, 4 * 512], BF16, tag="wo1")
            nc.vector.tensor_copy(Wo1_bf[:], Wo1_f32[:])
            Wo2_f32 = cpool.tile([128, 4 * DOUT], F32, tag="wo2f")
            for c2 in range(4):
                nc.sync.dma_start(Wo2_f32[:, c2 * DOUT:(c2 + 1) * DOUT],
                                  Wo2[c2 * 128:(c2 + 1) * 128, :])
            Wo2_bf = cpool.tile([128, 4 * DOUT], BF16, tag="wo2")
            nc.vector.tensor_copy(Wo2_bf[:], Wo2_f32[:])

            bstem_t = load_bias(b_stem, H, "bstem")
            bq1_t = load_bias(bq1, H, "bq1")
            bq2_t = load_bias(bq2, NH * A, "bq2")
            bv1_t = load_bias(bv1, H, "bv1")
            bv2_t = load_bias(bv2, 2 * H, "bv2")
            bk_t = load_bias(bk, NH * A, "bk")
            bv_t = load_bias(bv, NH * H, "bvt")
            bo1_t = load_bias(bo1, NH * H, "bo1")  # [512,1] -> use 4 chunks
            bo2_t = load_bias(bo2, DOUT, "bo2")

            # bo1' = bo1 + Wo1.T @ bv  (fold attention bias)
            bo1p = cpool.tile([128, 4], F32, tag="bo1p")  # col f2 = chunk
            bo1p_ps = psS.tile([128, 128], F32, tag="S", name="bo1p_s")[:, 0:4]
            for f2 in range(4):
                for c2 in range(4):
                    nc.tensor.matmul(
                        bo1p_ps[:, f2:f2 + 1],
                        Wo1_f32[:, c2 * 512 + f2 * 128:c2 * 512 + (f2 + 1) * 128],
                        bv_t[:, c2:c2 + 1],
                        start=(c2 == 0), stop=(c2 == 3))
            nc.vector.tensor_tensor(bo1p[:], bo1p_ps[:], bo1_t[:], OP.add)

            # ============ latent tables (per batch/core) ============
            cT = cpool.tile([DLAT, L], F32, tag="cT")
            nc.sync.dma_start(cT[:], cd[:].rearrange("l d -> d l"))
            cstem_ps = psA.tile([128, 512], F32, tag="A", name="cstem_s")[:, 0:L]
            nc.tensor.matmul(cstem_ps[:], Wstem_t[:], cT[:], start=True, stop=True)
            cstemT = cpool.tile([128, L], F32, tag="cstemT")
            nc.scalar.activation(cstemT[:], cstem_ps[:], AF.Identity, bias=bstem_t[:, 0:1])

            k_hi, k_lo, c_hi, c_lo, s_hi, s_lo = [], [], [], [], [], []
            for lc in range(2):
                # k_lat chunk
                kl_ps = psA.tile([128, 512], F32, tag="A", name="kl_s")[:, 0:NH * A]
                nc.tensor.matmul(kl_ps[:], cstemT[:, lc * 128:(lc + 1) * 128],
                                 Wk_t[:], start=True, stop=True)
                kl = cpool.tile([128, NH * A], F32, tag=f"kl{lc}")
                nc.vector.tensor_copy(kl[:], kl_ps[:])
                khi = cpool.tile([128, NH * A], BF16, tag=f"khi{lc}")
                nc.vector.tensor_copy(khi[:], kl[:])
                klo = cpool.tile([128, NH * A], BF16, tag=f"klo{lc}")
                nc.vector.tensor_tensor(klo[:], kl[:], khi[:], OP.subtract)
                k_hi.append(khi); k_lo.append(klo)
                # c_stem natural chunk (transpose)
                cn_ps = psA.tile([128, 512], F32, tag="A", name="cn_s")[:, 0:128]
                nc.tensor.transpose(cn_ps[:], cstemT[:, lc * 128:(lc + 1) * 128], ident[:])
                cn = cpool.tile([128, 128], F32, tag=f"cn{lc}")
                nc.vector.tensor_copy(cn[:], cn_ps[:])
                chi = cpool.tile([128, 128], BF16, tag=f"chi{lc}")
                nc.vector.tensor_copy(chi[:], cn[:])
                clo = cpool.tile([128, 128], BF16, tag=f"clo{lc}")
                nc.vector.tensor_tensor(clo[:], cn[:], chi[:], OP.subtract)
                c_hi.append(chi); c_lo.append(clo)
                # small table [128, 3] = (p0, p1, 1/g^2)
                sm = cpool.tile([128, 3], F32, tag=f"sm{lc}")
                nc.sync.dma_start(sm[:, 0:2], pd[lc * 128:(lc + 1) * 128, :])
                gt = cpool.tile([128, 1], F32, tag=f"gt{lc}")
                nc.sync.dma_start(gt[:], gd[lc * 128:(lc + 1) * 128, :])
                g2 = cpool.tile([128, 1], F32, tag=f"g2{lc}")
                nc.vector.tensor_tensor(g2[:], gt[:], gt[:], OP.mult)
                nc.vector.reciprocal(sm[:, 2:3], g2[:])
                shi = cpool.tile([128, 3], BF16, tag=f"shi{lc}")
                nc.vector.tensor_copy(shi[:], sm[:])
                slo = cpool.tile([128, 3], BF16, tag=f"slo{lc}")
                nc.vector.tensor_tensor(slo[:], sm[:], shi[:], OP.subtract)
                s_hi.append(shi); s_lo.append(slo)

            # p broadcast rows [128, 256] per coord, via ones-matmul
            prow = cpool.tile([1, 2 * L], F32, tag="prow")
            nc.sync.dma_start(prow[:, 0:L], pd[:, 0:1].rearrange("l o -> o l"))
            nc.sync.dma_start(prow[:, L:2 * L], pd[:, 1:2].rearrange("l o -> o l"))
            ones_f1 = cpool.tile([1, 128], F32, tag="ones_f1")
            nc.gpsimd.memset(ones_f1[:], 1.0)
            pB_ps = psA.tile([128, 512], F32, tag="A")
            nc.tensor.matmul(pB_ps[:, 0:2*L], ones_f1[:], prow[:], start=True, stop=True)
            pB = cpool.tile([128, 2 * L], F32, tag="pB")
            nc.vector.tensor_copy(pB[:], pB_ps[:, 0:2*L])

            # ============ main loop over pixel chunks ============
            for ci in range(nchunk):
                n0 = ci * CHUNK
                # ---- A: zx + top4 ----
                x2T = wpool.tile([2, 128], F32, tag="x2T", bufs=3)
                nc.sync.dma_start(x2T[:], xd[n0:n0 + 128, :].rearrange("n c -> c n"))
                x0 = wpool.tile([128, 2], F32, tag="x0", bufs=3)
                nc.sync.dma_start(x0[:], xd[n0:n0 + 128, :])
                d0 = wpool.tile([128, 256], F32, tag="d0", bufs=2)
                nc.gpsimd.tensor_scalar(d0[:], pB[:, 0:L], x0[:, 0:1], None, OP.subtract)
                d1 = wpool.tile([128, 256], F32, tag="d1", bufs=2)
                nc.gpsimd.tensor_scalar(d1[:], pB[:, L:2 * L], x0[:, 1:2], None, OP.subtract)
                sq0 = wpool.tile([128, 256], F32, tag="sq0", bufs=2)
                nc.gpsimd.tensor_tensor(sq0[:], d0[:], d0[:], OP.mult)
                sq1 = wpool.tile([128, 256], F32, tag="sq1", bufs=2)
                nc.gpsimd.tensor_tensor(sq1[:], d1[:], d1[:], OP.mult)
                nzx = wpool.tile([128, 256], F32, tag="nzx", bufs=2)
                nc.vector.scalar_tensor_tensor(nzx[:], sq0[:], -1.0, sq1[:],
                                               OP.mult, OP.subtract)
                m8 = wpool.tile([128, 8], F32, tag="m8", bufs=3)
                nc.vector.max(m8[:], nzx[:])
                i8 = wpool.tile([128, 8], U32, tag="i8", bufs=3)
                nc.vector.max_index(i8[:], m8[:], nzx[:])
                idxf = wpool.tile([128, 8], F32, tag="idxf", bufs=3)
                nc.vector.tensor_copy(idxf[:], i8[:])

                # ---- B: one-hots -> transposed -> gathers ----
                ohT = [wpool.tile([128, 4 * 128], BF16, tag=f"ohT{lc}", name=f"ohT{lc}") for lc in range(2)]
                ohbig = psS.tile([128, 1024], BF16, tag="S", name="ohbig")
                for s in range(K):
                    oh = wpool.tile([128, 256], BF16, tag="oh", bufs=3)
                    nc.gpsimd.tensor_scalar(oh[:], iota_f[:], idxf[:, s:s + 1],
                                             None, OP.is_equal)
                    for lc in range(2):
                        nc.tensor.transpose(
                            ohbig[:, (lc * 4 + s) * 128:(lc * 4 + s + 1) * 128],
                            oh[:, lc * 128:(lc + 1) * 128], ident_bf[:])
                for lc in range(2):
                    nc.vector.tensor_copy(ohT[lc][:],
                                          ohbig[:, lc * 512:(lc + 1) * 512])

                ck_ps = psA.tile([128, 512], F32, tag="A")
                kk_ps = psA.tile([128, 512], F32, tag="A")
                smlg_ps = psS.tile([36, 512], F32, tag="S", name="smlg_s")
                sm_ps = smlg_ps[0:3, :]
                lg_own = psS.tile([4, 512], F32, tag="S", name="lg_own")
                for lc in range(2):
                    nc.tensor.matmul(ck_ps[:], c_hi[lc][:], ohT[lc][:],
                                     start=(lc == 0), stop=False)
                    nc.tensor.matmul(ck_ps[:], c_lo[lc][:], ohT[lc][:],
                                     start=False, stop=(lc == 1))
                for lc in range(2):
                    nc.tensor.matmul(kk_ps[:], k_hi[lc][:], ohT[lc][:],
                                     start=(lc == 0), stop=False)
                    nc.tensor.matmul(kk_ps[:], k_lo[lc][:], ohT[lc][:],
                                     start=False, stop=(lc == 1))
                for lc in range(2):
                    nc.tensor.matmul(sm_ps[:], s_hi[lc][:], ohT[lc][:],
                                     start=(lc == 0), stop=False)
                    nc.tensor.matmul(sm_ps[:], s_lo[lc][:], ohT[lc][:],
                                     start=False, stop=(lc == 1))
                c_kT = wpool.tile([128, 512], F32, tag="c_kT", bufs=2)
                nc.scalar.copy(c_kT[:], ck_ps[:])
                k_kT = wpool.tile([128, 512], F32, tag="k_kT", bufs=2)
                nc.scalar.activation(k_kT[:], kk_ps[:], AF.Identity, bias=bk_t[:, 0:1])
                smT = wpool.tile([3, 512], F32, tag="smT", bufs=3)
                nc.vector.tensor_copy(smT[:], sm_ps[:])

                # ---- C: sin features ----
                relp = wpool.tile([2, 512], F32, tag="relp", bufs=3)  # p_sel - x
                nc.vector.tensor_tensor(
                    relp[:].rearrange("c (s n) -> c s n", s=4), smT[0:2, :]
                    .rearrange("c (s n) -> c s n", s=4),
                    x2T[:].rearrange("c (s n) -> c s n", s=1).to_broadcast([2, 4, 128]),
                    OP.subtract)
                # cc = pi*(x - p + 1) = -pi*relp + pi ; f_cc = cc/(2pi) wrapped
                tcc = wpool.tile([2, 512], F32, tag="tcc", bufs=3)
                nc.vector.tensor_scalar(tcc[:], relp[:], -0.5, 0.5, OP.mult, OP.add)
                icc = wpool.tile([2, 512], I32, tag="icc", bufs=3)
                nc.gpsimd.tensor_copy(icc[:], tcc[:])
                fcc32 = wpool.tile([2, 512], F32, tag="fcc32", bufs=3)
                nc.gpsimd.tensor_copy(fcc32[:], icc[:])
                fcc = wpool.tile([2, 512], F32, tag="fcc", bufs=3)
                nc.vector.tensor_tensor(fcc[:], tcc[:], fcc32[:], OP.subtract)
                sincc = wpool.tile([2, 512], F32R, tag="sincc", bufs=3)
                nc.scalar.activation(sincc[:], fcc[:], AF.Sin, scale=float(2 * PI))
                cc = wpool.tile([2, 512], F32, tag="cc", bufs=3)
                nc.vector.tensor_scalar(cc[:], relp[:], -PI, PI, OP.mult, OP.add)

                e_ps = psA.tile([128, 512], F32, tag="A")
                nc.tensor.matmul(e_ps[0:64, :], Wqsin_t[:], cc[:], start=True, stop=True)
                nc.tensor.matmul(e_ps[64:128, :], Wvsin_t[:], cc[:], start=True, stop=True, tile_position=(0, 64))
                te = wpool.tile([128, 512], F32, tag="te", bufs=2)
                nc.scalar.activation(te[:], e_ps[:], AF.Copy, scale=float(1.0 / (2 * PI)))
                ie = wpool.tile([128, 512], I32, tag="ie", bufs=2)
                nc.gpsimd.tensor_copy(ie[:], te[:])
                fe32 = wpool.tile([128, 512], F32, tag="fe32", bufs=2)
                nc.gpsimd.tensor_copy(fe32[:], ie[:])
                fe = wpool.tile([128, 512], F32, tag="fe", bufs=2)
                nc.vector.tensor_tensor(fe[:], te[:], fe32[:], OP.subtract)
                fabs = wpool.tile([128, 512], F32, tag="fabs", bufs=2)
                nc.scalar.activation(fabs[:], fe[:], AF.Abs)
                S = wpool.tile([128, 512], F32R, tag="S", bufs=2)   # sin(e): q rows 0:64, v 64:128
                nc.scalar.activation(S[:], fe[:], AF.Sin, scale=float(2 * PI))
                Ct = wpool.tile([128, 512], F32R, tag="Ct", bufs=2)  # cos(e)
                nc.scalar.activation(Ct[:], fabs[:], AF.Sin, scale=float(-2 * PI),
                                     bias=halfpi[:, 0:1])

                # ---- D: MLPs ----
                h1q_ps = psA.tile([128, 512], F32, tag="A")
                nc.tensor.matmul(h1q_ps[:], Wq1_sin[:], S[0:64, :], start=True, stop=False)
                nc.tensor.matmul(h1q_ps[:], Wq1_cos[:], Ct[0:64, :], start=False, stop=False)
                nc.tensor.matmul(h1q_ps[:], Wq1_cc[:], sincc[:], start=False, stop=True)
                h1q = wpool.tile([128, 512], F32R, tag="h1q", bufs=2)
                nc.scalar.activation(h1q[:], h1q_ps[:], AF.Gelu, bias=bq1_t[:, 0:1])
                q_ps = psA.tile([128, 512], F32, tag="A")
                nc.tensor.matmul(q_ps[:], Wq2_t[:], h1q[:], start=True, stop=True)

                h1v_ps = psA.tile([128, 512], F32, tag="A")
                nc.tensor.matmul(h1v_ps[:], Wv1_sin, S[64:128, :], start=True, stop=False)
                nc.tensor.matmul(h1v_ps[:], Wv1_cos, Ct[64:128, :], start=False, stop=False)
                nc.tensor.matmul(h1v_ps[:], Wv1_cc[:], sincc[:], start=False, stop=True)
                h1v = wpool.tile([128, 512], F32R, tag="h1v", bufs=2)
                nc.scalar.activation(h1v[:], h1v_ps[:], AF.Gelu, bias=bv1_t[:, 0:1])
                vg_ps = psA.tile([128, 512], F32, tag="A")
                nc.tensor.matmul(vg_ps[:], Wv2_t[:, 0:H], h1v[:], start=True, stop=True)
                vb_ps = psA.tile([128, 512], F32, tag="A")
                nc.tensor.matmul(vb_ps[:], Wv2_t[:, H:2 * H], h1v[:], start=True, stop=True)

                qk = wpool.tile([128, 512], F32R, tag="qk", bufs=2)
                nc.vector.scalar_tensor_tensor(qk[:], q_ps[:], bq2_t[:, 0:1], k_kT[:],
                                               OP.add, OP.mult)
                utmp = wpool.tile([128, 512], F32, tag="utmp", bufs=2)
                nc.vector.scalar_tensor_tensor(utmp[:], vg_ps[:], bv2_t[:, 0:1],
                                               c_kT[:], OP.add, OP.mult)
                u = wpool.tile([128, 512], F32R, tag="u", bufs=2)
                nc.vector.scalar_tensor_tensor(u[:], vb_ps[:], bv2_t[:, 1:2],
                                               utmp[:], OP.add, OP.add)

                # ---- E: logits + softmax (pixel-major) ----
                lg_ps = lg_own
                nc.tensor.matmul(lg_ps[:], blockones[:], qk[:], start=True, stop=True)
                lg_sb = wpool.tile([4, 512], F32, tag="lg_sb", bufs=3)
                nc.vector.tensor_copy(lg_sb[:], lg_ps[:])
                misc_ps = psS.tile([128, 512], F32, tag="S", name="misc_s")
                lgpm_ps = misc_ps[:, 0:16]
                smpm_ps = misc_ps[:, 16:28]
                for s in range(K):
                    nc.tensor.transpose(lgpm_ps[:, s * 4:(s + 1) * 4],
                                        lg_sb[:, s * 128:(s + 1) * 128], ident[0:4, 0:4])
                    nc.tensor.transpose(smpm_ps[:, s * 3:(s + 1) * 3],
                                        smT[:, s * 128:(s + 1) * 128], ident[0:3, 0:3])
                pen = wpool.tile([128, 4], F32, tag="pen", bufs=3)  # -zx*invg2
                nc.vector.tensor_tensor(
                    pen[:],
                    smpm_ps[:].rearrange("p (s c) -> p s c", c=3)[:, :, 2:3]
                    .rearrange("p s o -> p (s o)"),
                    m8[:, 0:4], OP.mult)
                lgpm = wpool.tile([128, 16], F32, tag="lgpm", bufs=3)
                nc.vector.scalar_tensor_tensor(
                    lgpm[:].rearrange("p (s h) -> p s h", s=4),
                    lgpm_ps[:].rearrange("p (s h) -> p s h", s=4), 0.0,
                    pen[:].to_broadcast([128, 4, 4]), OP.add, OP.add)
                mx = wpool.tile([128, 4], F32, tag="mx", bufs=3)
                nc.vector.tensor_reduce(
                    mx[:], lgpm[:].rearrange("p (s h) -> p h s", s=4),
                    mybir.AxisListType.X, OP.max)
                esub = wpool.tile([128, 16], F32, tag="esub", bufs=3)
                nc.vector.tensor_tensor(
                    esub[:].rearrange("p (s h) -> p s h", s=4),
                    lgpm[:].rearrange("p (s h) -> p s h", s=4),
                    mx[:].rearrange("p (h o) -> p o h", o=1).to_broadcast([128, 4, 4]),
                    OP.subtract)
                epm = wpool.tile([128, 16], F32, tag="epm", bufs=3)
                nc.scalar.activation(epm[:], esub[:], AF.Exp)
                zs = wpool.tile([128, 4], F32, tag="zs", bufs=3)
                nc.vector.tensor_reduce(
                    zs[:], epm[:].rearrange("p (s h) -> p h s", s=4),
                    mybir.AxisListType.X, OP.add)
                rz = wpool.tile([128, 4], F32, tag="rz", bufs=3)
                nc.vector.reciprocal(rz[:], zs[:])
                att_pm = wpool.tile([128, 16], F32, tag="att_pm", bufs=3)
                nc.vector.tensor_tensor(
                    att_pm[:].rearrange("p (h s) -> p s h", h=4),
                    epm[:].rearrange("p (s h) -> p s h", s=4),
                    rz[:].rearrange("p (h o) -> p o h", o=1).to_broadcast([128, 4, 4]),
                    OP.mult)
                att_ps = misc_ps[0:16, 64:192]
                nc.tensor.transpose(att_ps[:], att_pm[:], ident[:])
                att_sh = wpool.tile([16, 128], BF16, tag="att_sh", bufs=3)
                nc.vector.tensor_copy(att_sh[:], att_ps[:])
                att_dr = drpool.tile([16, 128], BF16, tag="att_dr")
                nc.sync.dma_start(att_dr[:], att_sh[:])
                att_flat = wpool.tile([1, 2048], BF16, tag="att_flat", bufs=2)
                nc.sync.dma_start(att_flat[:],
                                  att_dr[:].rearrange("r n -> (r n)")
                                  .rearrange("(o f) -> o f", o=1))

                # ---- F: apply attention + output MLP ----
                uw = wpool.tile([128, 2048], BF16, tag="uw")
                for h in range(NH):
                    attB_ps = psB.tile([128, 512], F32, tag="B")
                    nc.tensor.matmul(attB_ps[:], ones16_bf[0:1, :],
                                     att_flat[0:1, h * 512:(h + 1) * 512],
                                     start=True, stop=True)
                    if debug and ci == 0 and h == 0:
                        attB_sb = wpool.tile([128, 512], F32, tag="attB_sb")
                        nc.vector.tensor_copy(attB_sb[:], attB_ps[:])
                    nc.vector.tensor_tensor(
                        uw[:, h * 512:(h + 1) * 512], u[:], attB_ps[:], OP.mult)
                y_ps = psA.tile([128, 512], F32, tag="A")
                for h in range(NH):
                    for s in range(K):
                        nc.tensor.matmul(
                            y_ps[:, h * 128:(h + 1) * 128],
                            Wv_bf[:, h * 128:(h + 1) * 128],
                            uw[:, h * 512 + s * 128:h * 512 + (s + 1) * 128],
                            start=(s == 0), stop=(s == 3))
                y_bf = wpool.tile([128, 512], BF16, tag="y_bf", bufs=2)
                nc.scalar.copy(y_bf[:], y_ps[:])
                y1_ps = psA.tile([128, 512], F32, tag="A")
                for f2 in range(4):
                    for h in range(4):
                        nc.tensor.matmul(
                            y1_ps[:, f2 * 128:(f2 + 1) * 128],
                            Wo1_bf[:, h * 512 + f2 * 128:h * 512 + (f2 + 1) * 128],
                            y_bf[:, h * 128:(h + 1) * 128],
                            start=(h == 0), stop=(h == 3))
                y1 = wpool.tile([128, 512], BF16, tag="y1", bufs=2)
                for f2 in range(4):
                    nc.scalar.activation(y1[:, f2 * 128:(f2 + 1) * 128],
                                         y1_ps[:, f2 * 128:(f2 + 1) * 128],
                                         AF.Gelu, bias=bo1p[:, f2:f2 + 1])
                o_ps = misc_ps[0:3, 192:320]
                for c2 in range(4):
                    nc.tensor.matmul(o_ps[:], Wo2_bf[:, c2 * 3:(c2 + 1) * 3],
                                     y1[:, c2 * 128:(c2 + 1) * 128],
                                     start=(c2 == 0), stop=(c2 == 3))
                o_sb = wpool.tile([3, 128], F32, tag="o_sb", bufs=3)
                nc.scalar.activation(o_sb[:], o_ps[:], AF.Identity, bias=bo2_t[:, 0:1])
                nc.sync.dma_start(outd[n0:n0 + 128, :].rearrange("n c -> c n"), o_sb[:])
                if debug and ci == 0:
                    for nm, t in [("nzx", nzx), ("m8", m8), ("idxf", idxf),
                                  ("c_kT", c_kT), ("k_kT", k_kT), ("smT", smT),
                                  ("relp", relp), ("fe", fe), ("S", S), ("Ct", Ct),
                                  ("sincc", sincc), ("h1q", h1q), ("qk", qk), ("u", u),
                                  ("lgpm", lgpm), ("att_pm", att_pm), ("uw", uw),
                                  ("y_bf", y_bf), ("y1", y1), ("ohT0", ohT[0]),
                                  ("att_flat", att_flat), ("attB0", attB_sb), ("att_sh", att_sh)]:
                    # cast to f32 staging then DMA (uniform dtype)
                        st = wpool.tile([128, 2048], F32, tag="dbgst",
                                        name="dbgst_" + nm)[0:t.shape[0], 0:t.shape[1]]
                        nc.vector.tensor_copy(st[:], t[:])
                        nc.sync.dma_start(dbg[nm][:], st[:])

    nc.compile()
    return nc


def kernel(**inputs):
    import jax
    try:
        jax.config.update('jax_platforms', 'axon,cpu')
    except Exception:
        pass
    from concourse.bass_utils import run_bass_kernel_spmd

    nchunk = NPC // CHUNK
    if nchunk not in _cache:
        _cache[nchunk] = _build(nchunk)
    nc = _cache[nchunk]

    x = np.asarray(inputs["x"], np.float32)
    wkeys = ["W_stem", "b_stem", "Wq_sin", "Wq1", "bq1", "Wq2", "bq2",
             "Wv_sin", "Wv1", "bv1", "Wv2", "bv2", "Wk", "bk", "Wv", "bv",
             "Wo1", "bo1", "Wo2", "bo2"]
    in_maps = []
    for core in range(NCORE):
        b = core // (NCORE // B)
        sh = (core % (NCORE // B))
        m = {k: np.ascontiguousarray(np.asarray(inputs[k], np.float32)) for k in wkeys}
        m["x"] = np.ascontiguousarray(x[b, sh * NPC:(sh + 1) * NPC])
        m["p"] = np.ascontiguousarray(np.asarray(inputs["p"], np.float32)[b])
        m["c"] = np.ascontiguousarray(np.asarray(inputs["c"], np.float32)[b])
        m["g"] = np.ascontiguousarray(np.asarray(inputs["g"], np.float32)[b])
        in_maps.append(m)

    res = run_bass_kernel_spmd(nc, in_maps, core_ids=list(range(NCORE)))
    out = np.zeros((B, N, DOUT), np.float32)
    for core in range(NCORE):
        b = core // (NCORE // B)
        sh = core % (NCORE // B)
        out[b, sh * NPC:(sh + 1) * NPC] = res.results[core]["out"]
    return out



# revision 44
# speedup vs baseline: 1.5229x; 1.5229x over previous
"""Trainium2 Bass kernel for nn_EquivariantNeuralField.

Per-pixel top-4-nearest-latent cross-attention neural field.
Sharding: 8 cores; core i handles batch i//4, pixel rows (i%4)*4096..+4096.

v2: phase-split pipeline (A=trig table, B=gelu table) over 4-chunk groups
to kill activation-table thrashing; f32r single-pass gathers; latent-major
one-hot build (no big transposes); exp via tanh; DMA-broadcast attention.
"""
import numpy as np

B, N, L, K = 2, 16384, 256, 4
DIN, DOUT, DLAT, H, A, NH = 2, 3, 64, 128, 32, 4
NCORE = 8
NPC = N * B // NCORE          # pixels per core = 4096
CHUNK = 128
GS = 8                        # chunks per phase group
PI = float(np.pi)

_cache = {}


def _build(nchunk):
    import concourse.bacc as bacc
    import concourse.mybir as mybir
    from concourse.tile import TileContext

    F32 = mybir.dt.float32
    F32R = mybir.dt.float32r
    BF16 = mybir.dt.bfloat16
    I32 = mybir.dt.int32
    U32 = mybir.dt.uint32
    AF = mybir.ActivationFunctionType
    OP = mybir.AluOpType

    nc = bacc.Bacc()

    # ---------------- DRAM tensors ----------------
    # Tables and weight-derived constants are precomputed host-side in kernel().
    xd = nc.dram_tensor("x", [NPC, DIN], F32, kind="ExternalInput")
    pBd = nc.dram_tensor("pB", [128, 2 * L], F32, kind="ExternalInput")
    ktabd = nc.dram_tensor("k_tab", [L, NH * A], F32R, kind="ExternalInput")
    ctabd = nc.dram_tensor("c_tab", [L, H], F32R, kind="ExternalInput")
    smtabd = nc.dram_tensor("sm_tab", [L, 3], F32R, kind="ExternalInput")
    Wsin_d = nc.dram_tensor("Wsin", [DIN, H], F32R, kind="ExternalInput")
    biase_d = nc.dram_tensor("bias_e", [H, 1], F32, kind="ExternalInput")
    Wq1 = nc.dram_tensor("Wq1", [H + DIN, H], F32, kind="ExternalInput")
    bq1 = nc.dram_tensor("bq1", [H], F32, kind="ExternalInput")
    Wq2 = nc.dram_tensor("Wq2", [H, NH * A], F32R, kind="ExternalInput")
    bq2 = nc.dram_tensor("bq2", [NH * A], F32, kind="ExternalInput")
    Wv1 = nc.dram_tensor("Wv1", [H + DIN, H], F32, kind="ExternalInput")
    bv1 = nc.dram_tensor("bv1", [H], F32, kind="ExternalInput")
    Wv2 = nc.dram_tensor("Wv2", [H, 2 * H], F32R, kind="ExternalInput")
    bv2 = nc.dram_tensor("bv2", [2 * H], F32, kind="ExternalInput")
    Wv = nc.dram_tensor("Wv", [H, NH * H], F32, kind="ExternalInput")
    Wo1 = nc.dram_tensor("Wo1", [NH * H, NH * H], F32, kind="ExternalInput")
    bo1p_d = nc.dram_tensor("bo1p", [128, 4], F32, kind="ExternalInput")
    Wo2 = nc.dram_tensor("Wo2", [NH * H, DOUT], F32, kind="ExternalInput")
    bo2 = nc.dram_tensor("bo2", [DOUT], F32, kind="ExternalInput")
    outd = nc.dram_tensor("out", [NPC, DOUT], F32, kind="ExternalOutput")

    with TileContext(nc) as tc:
        with tc.tile_pool(name="const", bufs=1) as cpool, \
             tc.tile_pool(name="work", bufs=2) as wpool, \
             tc.tile_pool(name="psA", bufs=5, space="PSUM") as psA, \
             tc.tile_pool(name="psS", bufs=3, space="PSUM") as psS, \
             tc.tile_pool(name="drp", bufs=4, space="DRAM") as drpool:

            # ============ one-time constants ============
            idn_i = cpool.tile([128, 128], I32)
            nc.gpsimd.iota(idn_i[:], [[1, 128]], base=0, channel_multiplier=-1)
            idn_f0 = cpool.tile([128, 128], F32)
            nc.vector.tensor_copy(idn_f0[:], idn_i[:])
            ident = cpool.tile([128, 128], F32)
            nc.vector.tensor_scalar(ident[:], idn_f0[:], 0.0, None, OP.is_equal)
            ident_bf = cpool.tile([128, 128], BF16)
            nc.vector.tensor_copy(ident_bf[:], ident[:])
            # per-partition iota columns (f32): values p and p+128
            iop_i = cpool.tile([128, 1], I32)
            nc.gpsimd.iota(iop_i[:], [[1, 1]], base=0, channel_multiplier=1)
            iota0 = cpool.tile([128, 1], F32)
            nc.vector.tensor_copy(iota0[:], iop_i[:])
            iota1 = cpool.tile([128, 1], F32)
            nc.vector.tensor_scalar(iota1[:], iota0[:], 128.0, None, OP.add)
            # blockones [128, NH] f32r : bo[c, h] = (c//A == h)
            blockones_f = cpool.tile([128, NH], F32)
            nc.gpsimd.memset(blockones_f[:], 0.0)
            for h in range(NH):
                nc.gpsimd.memset(blockones_f[h * A:(h + 1) * A, h:h + 1], 1.0)
            blockones = cpool.tile([128, NH], F32R)
            nc.vector.tensor_copy(blockones[:], blockones_f[:])
            # ACT-ordering token: sin ops of group g+1 wait on gelu ops of g
            tok = cpool.tile([128, 1], F32)
            nc.gpsimd.memset(tok[:], 0.0)

            # ============ weights (host-precomputed, straight DMA loads) ===
            def load_cast(dram_ap, shape, dt, tag):
                if dt in (F32, F32R):
                    t0 = cpool.tile(shape, dt, tag=tag + "_d", name=tag)
                    nc.sync.dma_start(t0[:], dram_ap)
                    return t0
                t0 = wpool.tile([128, 512], F32, tag="stage", name="stage_" + tag)
                nc.sync.dma_start(t0[0:shape[0], 0:shape[1]], dram_ap)
                t1 = cpool.tile(shape, dt, tag=tag)
                nc.vector.tensor_copy(t1[:], t0[0:shape[0], 0:shape[1]])
                return t1

            def load_bias(dram, n, tag):
                if n <= 128:
                    t = cpool.tile([n, 1], F32, tag=tag)
                    nc.sync.dma_start(t[:], dram[:].rearrange("(n o) -> n o", o=1))
                    return t
                k = n // 128
                t = cpool.tile([128, k], F32, tag=tag)
                nc.sync.dma_start(t[:], dram[:].rearrange("(j p) -> p j", p=128))
                return t

            Wsin_t = load_cast(Wsin_d[:], [DIN, H], F32R, "wsin")
            bias_e = cpool.tile([H, 1], F32, tag="bias_e")
            nc.sync.dma_start(bias_e[:], biase_d[:])
            Wq1_cc = load_cast(Wq1[0:DIN, :], [DIN, H], BF16, "wq1cc")
            Wq1_sin = load_cast(Wq1[DIN:DIN + 64, :], [64, H], BF16, "wq1sin")
            Wq1_cos = load_cast(Wq1[DIN + 64:DIN + 128, :], [64, H], BF16, "wq1cos")
            Wv1_cc = load_cast(Wv1[0:DIN, :], [DIN, H], BF16, "wv1cc")
            Wv1_sf = cpool.tile([128, H], F32, tag="wv1sf")
            nc.sync.dma_start(Wv1_sf[64:128, :], Wv1[DIN:DIN + 64, :])
            Wv1_sin_t = cpool.tile([128, H], BF16, tag="wv1sin")
            nc.vector.tensor_copy(Wv1_sin_t[64:128, :], Wv1_sf[64:128, :])
            Wv1_cf = cpool.tile([128, H], F32, tag="wv1cf")
            nc.sync.dma_start(Wv1_cf[64:128, :], Wv1[DIN + 64:DIN + 128, :])
            Wv1_cos_t = cpool.tile([128, H], BF16, tag="wv1cos")
            nc.vector.tensor_copy(Wv1_cos_t[64:128, :], Wv1_cf[64:128, :])
            Wv1_sin = Wv1_sin_t[64:128, :]
            Wv1_cos = Wv1_cos_t[64:128, :]
            Wq2_t = load_cast(Wq2[:], [H, NH * A], F32R, "wq2")
            Wv2_t = load_cast(Wv2[:], [H, 2 * H], F32R, "wv2")
            Wv_bf = load_cast(Wv[:], [H, NH * H], BF16, "wv")
            # Wo1 as [128, (c2, f) 2048] bf16 (staged through rotating buffer)
            Wo1_bf = cpool.tile([128, 4 * 512], BF16, tag="wo1")
            for c2 in range(4):
                st = wpool.tile([128, 512], F32, tag="stage", name=f"wo1st{c2}")
                nc.sync.dma_start(st[:], Wo1[c2 * 128:(c2 + 1) * 128, :])
                nc.vector.tensor_copy(Wo1_bf[:, c2 * 512:(c2 + 1) * 512], st[:])
            Wo2_f32 = cpool.tile([128, 4 * DOUT], F32, tag="wo2f")
            for c2 in range(4):
                nc.sync.dma_start(Wo2_f32[:, c2 * DOUT:(c2 + 1) * DOUT],
                                  Wo2[c2 * 128:(c2 + 1) * 128, :])
            Wo2_bf = cpool.tile([128, 4 * DOUT], BF16, tag="wo2")
            nc.vector.tensor_copy(Wo2_bf[:], Wo2_f32[:])

            bq1_t = load_bias(bq1, H, "bq1")
            bq2_t = load_bias(bq2, NH * A, "bq2")
            bv1_t = load_bias(bv1, H, "bv1")
            bv2_t = load_bias(bv2, 2 * H, "bv2")
            bo2_t = load_bias(bo2, DOUT, "bo2")
            bo1p = cpool.tile([128, 4], F32, tag="bo1p")
            nc.sync.dma_start(bo1p[:], bo1p_d[:])

            # ============ latent tables (host-precomputed) ============
            k_tab, c_tab, s_tab = [], [], []
            for lc in range(2):
                kl = cpool.tile([128, NH * A], F32R, tag=f"kl{lc}")
                nc.sync.dma_start(kl[:], ktabd[lc * 128:(lc + 1) * 128, :])
                k_tab.append(kl)
                cn = cpool.tile([128, 128], F32R, tag=f"cn{lc}")
                nc.sync.dma_start(cn[:], ctabd[lc * 128:(lc + 1) * 128, :])
                c_tab.append(cn)
                smr = cpool.tile([128, 3], F32R, tag=f"smr{lc}")
                nc.sync.dma_start(smr[:], smtabd[lc * 128:(lc + 1) * 128, :])
                s_tab.append(smr)
            pB = cpool.tile([128, 2 * L], F32, tag="pB")
            nc.sync.dma_start(pB[:], pBd[:])

            # ============ phase A: distances, top-4, gathers, sin features ====
            def phase_a(ci, j):
                n0 = ci * CHUNK
                x0 = wpool.tile([128, 2], F32, tag=f"x0_{j}", bufs=1)
                nc.sync.dma_start(x0[:], xd[n0:n0 + 128, :])
                x2T = wpool.tile([2, 128], F32, tag=f"x2T_{j}", bufs=1)
                nc.sync.dma_start(x2T[:], xd[n0:n0 + 128, :].rearrange("n c -> c n"))
                d0 = wpool.tile([128, 256], F32, tag="d0")
                nc.gpsimd.tensor_scalar(d0[:], pB[:, 0:L], x0[:, 0:1], None, OP.subtract)
                d1 = wpool.tile([128, 256], F32, tag="d1")
                nc.gpsimd.tensor_scalar(d1[:], pB[:, L:2 * L], x0[:, 1:2], None, OP.subtract)
                sq0 = wpool.tile([128, 256], F32, tag="sq0")
                nc.gpsimd.tensor_tensor(sq0[:], d0[:], d0[:], OP.mult)
                sq1 = wpool.tile([128, 256], F32, tag="sq1")
                nc.gpsimd.tensor_tensor(sq1[:], d1[:], d1[:], OP.mult)
                nzx = wpool.tile([128, 256], F32, tag="nzx")
                nc.vector.scalar_tensor_tensor(nzx[:], sq0[:], -1.0, sq1[:],
                                               OP.mult, OP.subtract)
                m8 = wpool.tile([128, 8], F32, tag=f"m8_{j}", bufs=1)
                nc.vector.max(m8[:], nzx[:])
                i8 = wpool.tile([128, 8], U32, tag="i8", bufs=3)
                nc.vector.max_index(i8[:], m8[:], nzx[:])
                idxb = wpool.tile([128, 4], BF16, tag="idxb", bufs=3)
                nc.vector.tensor_copy(idxb[:], i8[:, 0:4])

                # --- one-hot, latent-major: ohT[l, s*128+p] = (idx[p,s] == l)
                # idx -> DRAM (s-major) -> broadcast-read to all 128 partitions
                idx_dr = drpool.tile([4, 128], BF16, tag="idx_dr")
                nc.sync.dma_start(idx_dr[:].rearrange("s p -> p s"), idxb[:])
                idxB = wpool.tile([128, 512], BF16, tag="idxB", bufs=2)
                nc.sync.dma_start(
                    idxB[:],
                    idx_dr[:].rearrange("r n -> (r n)")
                    .rearrange("(o f) -> o f", o=1).to_broadcast([128, 512]))
                ohT = [wpool.tile([128, 512], F32R, tag=f"ohT{lc}",
                                  name=f"ohT{lc}") for lc in range(2)]
                nc.gpsimd.tensor_scalar(ohT[0][:], idxB[:], iota0[:], None, OP.is_equal)
                nc.gpsimd.tensor_scalar(ohT[1][:], idxB[:], iota1[:], None, OP.is_equal)

                # --- gathers (single-pass f32r) ---
                ck_ps = psA.tile([128, 512], F32, tag="A")
                kk_ps = psA.tile([128, 512], F32, tag="A")
                smlg_ps = psS.tile([36, 512], F32, tag="S", name="smlg_s")
                sm_ps = smlg_ps[0:3, :]
                for lc in range(2):
                    nc.tensor.matmul(ck_ps[:], c_tab[lc][:], ohT[lc][:],
                                     start=(lc == 0), stop=(lc == 1))
                for lc in range(2):
                    nc.tensor.matmul(kk_ps[:], k_tab[lc][:], ohT[lc][:],
                                     start=(lc == 0), stop=(lc == 1))
                for lc in range(2):
                    nc.tensor.matmul(sm_ps, s_tab[lc][:], ohT[lc][:],
                                     start=(lc == 0), stop=(lc == 1))
                c_kT = wpool.tile([128, 512], BF16, tag=f"c_kT_{j}", bufs=1)
                nc.vector.tensor_copy(c_kT[:], ck_ps[:])
                k_kT = wpool.tile([128, 512], F32, tag=f"k_kT_{j}", bufs=1)
                nc.scalar.copy(k_kT[:], kk_ps[:])
                smT = wpool.tile([3, 512], F32, tag="smT", bufs=2)
                nc.scalar.copy(smT[:], sm_ps[:])
                # invg2 pixel-major [128, 12] for the softmax penalty
                smpm_ps = psS.tile([128, 16], F32, tag="S", name="smpm_s")[:, 0:12]
                for s in range(K):
                    nc.tensor.transpose(smpm_ps[:, s * 3:(s + 1) * 3],
                                        smT[:, s * 128:(s + 1) * 128], ident[0:3, 0:3])
                smpm = wpool.tile([128, 12], F32, tag=f"smpm_{j}", bufs=1)
                nc.vector.tensor_copy(smpm[:], smpm_ps[:])

                # --- sin features ---
                relp = wpool.tile([2, 512], F32R, tag="relp", bufs=2)
                nc.vector.tensor_tensor(
                    relp[:].rearrange("c (s n) -> c s n", s=4), sm_ps[0:2, :]
                    .rearrange("c (s n) -> c s n", s=4),
                    x2T[:].rearrange("c (s n) -> c s n", s=1).to_broadcast([2, 4, 128]),
                    OP.subtract)
                # cc = pi*(x - p + 1) = -pi*relp + pi ; f_cc = cc/(2pi) wrapped
                tcc = wpool.tile([2, 512], F32, tag="tcc", bufs=2)
                nc.vector.tensor_scalar(tcc[:], relp[:], -0.5, 0.5, OP.mult, OP.add)
                icc = wpool.tile([2, 512], I32, tag="icc", bufs=2)
                nc.gpsimd.tensor_copy(icc[:], tcc[:])
                fcc32 = wpool.tile([2, 512], F32, tag="fcc32", bufs=2)
                nc.gpsimd.tensor_copy(fcc32[:], icc[:])
                fcc = wpool.tile([2, 512], F32, tag="fcc", bufs=2)
                nc.vector.tensor_tensor(fcc[:], tcc[:], fcc32[:], OP.subtract)
                sincc = wpool.tile([2, 512], BF16, tag=f"sincc_{j}", bufs=1)
                nc.scalar.activation(sincc[:], fcc[:], AF.Sin, scale=float(2 * PI),
                                     bias=tok[0:2, 0:1])

                # te = e/(2pi) computed directly from relp via pre-folded weights
                # (Wsin pre-scaled by -0.5 on host; constant term added as ACT bias)
                e_ps = psA.tile([128, 512], F32, tag="A")
                nc.tensor.matmul(e_ps[:], Wsin_t[:], relp[:], start=True, stop=True)
                te = wpool.tile([128, 512], F32, tag="te", bufs=2)
                nc.scalar.activation(te[:], e_ps[:], AF.Identity, bias=bias_e[:, 0:1])
                ie = wpool.tile([128, 512], I32, tag="ie", bufs=2)
                nc.gpsimd.tensor_copy(ie[:], te[:])
                fe32 = wpool.tile([128, 512], F32, tag="fe32", bufs=2)
                nc.gpsimd.tensor_copy(fe32[:], ie[:])
                # fboth = [fe | 0.25-|fe|]; one Sin gives [sin(e) | cos(e)]
                fboth = wpool.tile([128, 1024], F32, tag="fboth", bufs=2)
                nc.vector.tensor_tensor(fboth[:, 0:512], te[:], fe32[:], OP.subtract)
                fab = wpool.tile([128, 512], F32, tag="fab", bufs=2)
                nc.vector.scalar_tensor_tensor(fab[:], fboth[:, 0:512], -1.0,
                                               fboth[:, 0:512], OP.mult, OP.max)
                nc.gpsimd.tensor_scalar(fboth[:, 512:1024], fab[:], -1.0, 0.25,
                                        OP.mult, OP.add)
                SCt = wpool.tile([128, 1024], BF16, tag=f"SCt_{j}", bufs=1)
                nc.scalar.activation(SCt[:], fboth[:], AF.Sin, scale=float(2 * PI),
                                     bias=tok[:, 0:1])
                return dict(SCt=SCt, sincc=sincc, c_kT=c_kT, k_kT=k_kT,
                            smpm=smpm, m8=m8)

            # ============ phase B: MLPs, attention, output ============
            def phase_b(ci, j, a):
                n0 = ci * CHUNK
                SCt, sincc = a["SCt"], a["sincc"]
                c_kT, k_kT, smpm, m8 = a["c_kT"], a["k_kT"], a["smpm"], a["m8"]

                h1q_ps = psA.tile([128, 512], F32, tag="A")
                nc.tensor.matmul(h1q_ps[:], Wq1_sin[:], SCt[0:64, 0:512], start=True, stop=False)
                nc.tensor.matmul(h1q_ps[:], Wq1_cos[:], SCt[0:64, 512:1024], start=False, stop=False)
                nc.tensor.matmul(h1q_ps[:], Wq1_cc[:], sincc[:], start=False, stop=True)
                h1q = wpool.tile([128, 512], F32R, tag="h1q", bufs=2)
                nc.scalar.activation(h1q[:], h1q_ps[:], AF.Gelu, bias=bq1_t[:, 0:1])
                q_ps = psA.tile([128, 512], F32, tag="A")
                nc.tensor.matmul(q_ps[:], Wq2_t[:], h1q[:], start=True, stop=True)

                qk = wpool.tile([128, 512], F32R, tag="qk", bufs=2)
                nc.vector.scalar_tensor_tensor(qk[:], q_ps[:], bq2_t[:, 0:1], k_kT[:],
                                               OP.add, OP.mult)

                # ---- logits + softmax (pixel-major), exp via tanh ----
                lg_ps = psS.tile([4, 512], F32, tag="S", name="lg_s")
                nc.tensor.matmul(lg_ps[:], blockones[:], qk[:], start=True, stop=True)
                lg_sb = wpool.tile([4, 512], F32, tag="lg_sb", bufs=2)
                nc.scalar.copy(lg_sb[:], lg_ps[:])
                misc_ps = psS.tile([128, 512], F32, tag="S", name="misc_s")
                lgpm_ps = misc_ps[:, 0:16]
                for s in range(K):
                    nc.tensor.transpose(lgpm_ps[:, s * 4:(s + 1) * 4],
                                        lg_sb[:, s * 128:(s + 1) * 128], ident[0:4, 0:4])
                pen = wpool.tile([128, 4], F32, tag="pen", bufs=3)  # -zx*invg2
                nc.vector.tensor_tensor(
                    pen[:],
                    smpm[:].rearrange("p (s c) -> p s c", c=3)[:, :, 2:3]
                    .rearrange("p s o -> p (s o)"),
                    m8[:, 0:4], OP.mult)
                lgpm = wpool.tile([128, 16], F32, tag="lgpm", bufs=3)
                nc.vector.scalar_tensor_tensor(
                    lgpm[:].rearrange("p (s h) -> p s h", s=4),
                    lgpm_ps[:].rearrange("p (s h) -> p s h", s=4), 0.0,
                    pen[:].to_broadcast([128, 4, 4]), OP.add, OP.add)
                mx = wpool.tile([128, 4], F32, tag="mx", bufs=3)
                nc.vector.tensor_reduce(
                    mx[:], lgpm[:].rearrange("p (s h) -> p h s", s=4),
                    mybir.AxisListType.X, OP.max)
                esub = wpool.tile([128, 16], F32, tag="esub", bufs=3)
                nc.vector.tensor_tensor(
                    esub[:].rearrange("p (s h) -> p s h", s=4),
                    lgpm[:].rearrange("p (s h) -> p s h", s=4),
                    mx[:].rearrange("p (h o) -> p o h", o=1).to_broadcast([128, 4, 4]),
                    OP.subtract)
                # exp(x) = (1+t)/(1-t), t = tanh(x/2)  (keeps ACT in gelu set)
                th = wpool.tile([128, 16], F32, tag="th", bufs=3)
                nc.scalar.activation(th[:], esub[:], AF.Tanh, scale=0.5)
                num = wpool.tile([128, 16], F32, tag="num", bufs=3)
                nc.vector.tensor_scalar(num[:], th[:], 1.0, None, OP.add)
                den = wpool.tile([128, 16], F32, tag="den", bufs=3)
                nc.vector.tensor_scalar(den[:], th[:], -1.0, 1.0, OP.mult, OP.add)
                rcp = wpool.tile([128, 16], F32, tag="rcp", bufs=3)
                nc.vector.reciprocal(rcp[:], den[:])
                epm = wpool.tile([128, 16], F32, tag="epm", bufs=3)
                nc.vector.tensor_tensor(epm[:], num[:], rcp[:], OP.mult)
                zs = wpool.tile([128, 4], F32, tag="zs", bufs=3)
                nc.vector.tensor_reduce(
                    zs[:], epm[:].rearrange("p (s h) -> p h s", s=4),
                    mybir.AxisListType.X, OP.add)
                rz = wpool.tile([128, 4], F32, tag="rz", bufs=3)
                nc.vector.reciprocal(rz[:], zs[:])
                att_pm = wpool.tile([128, 16], F32, tag="att_pm", bufs=4)
                nc.vector.tensor_tensor(
                    att_pm[:].rearrange("p (h s) -> p s h", h=4),
                    epm[:].rearrange("p (s h) -> p s h", s=4),
                    rz[:].rearrange("p (h o) -> p o h", o=1).to_broadcast([128, 4, 4]),
                    OP.mult)
                att_ps = misc_ps[0:16, 64:192]
                nc.tensor.transpose(att_ps, att_pm[:], ident[:])
                att_sh = wpool.tile([16, 128], BF16, tag="att_sh", bufs=4)
                nc.vector.tensor_copy(att_sh[:], att_ps)
                att_dr = drpool.tile([16, 128], BF16, tag="att_dr")
                nc.sync.dma_start(att_dr[:], att_sh[:])
                # broadcast att rows to all 128 partitions: [128, (h,s,p) 2048]
                attB = wpool.tile([128, 2048], BF16, tag="attB", bufs=2)
                nc.sync.dma_start(
                    attB[:],
                    att_dr[:].rearrange("r n -> (r n)")
                    .rearrange("(o f) -> o f", o=1).to_broadcast([128, 2048]))

                # ---- v-side MLP (independent of the att round trip) ----
                h1v_ps = psA.tile([128, 512], F32, tag="A")
                nc.tensor.matmul(h1v_ps[:], Wv1_sin, SCt[64:128, 0:512], start=True, stop=False)
                nc.tensor.matmul(h1v_ps[:], Wv1_cos, SCt[64:128, 512:1024], start=False, stop=False)
                nc.tensor.matmul(h1v_ps[:], Wv1_cc[:], sincc[:], start=False, stop=True)
                h1v = wpool.tile([128, 512], F32R, tag="h1v", bufs=2)
                nc.scalar.activation(h1v[:], h1v_ps[:], AF.Gelu, bias=bv1_t[:, 0:1])
                vg_ps = psA.tile([128, 512], F32, tag="A")
                nc.tensor.matmul(vg_ps[:], Wv2_t[:, 0:H], h1v[:], start=True, stop=True)
                vb_ps = psA.tile([128, 512], F32, tag="A")
                nc.tensor.matmul(vb_ps[:], Wv2_t[:, H:2 * H], h1v[:], start=True, stop=True)
                utmp = wpool.tile([128, 512], F32, tag="utmp", bufs=2)
                nc.vector.scalar_tensor_tensor(utmp[:], vg_ps[:], bv2_t[:, 0:1],
                                               c_kT[:], OP.add, OP.mult)
                u_bf = wpool.tile([128, 512], BF16, tag="u_bf", bufs=3)
                nc.vector.scalar_tensor_tensor(u_bf[:], vb_ps[:], bv2_t[:, 1:2],
                                               utmp[:], OP.add, OP.add)

                # ---- apply attention + output MLP ----
                uw = wpool.tile([128, 2048], BF16, tag="uw", bufs=2)
                for h in range(NH):
                    nc.gpsimd.tensor_tensor(uw[:, h * 512:(h + 1) * 512], u_bf[:],
                                            attB[:, h * 512:(h + 1) * 512], OP.mult)
                y_ps = psA.tile([128, 512], F32, tag="A")
                for h in range(NH):
                    for s in range(K):
                        nc.tensor.matmul(
                            y_ps[:, h * 128:(h + 1) * 128],
                            Wv_bf[:, h * 128:(h + 1) * 128],
                            uw[:, h * 512 + s * 128:h * 512 + (s + 1) * 128],
                            start=(s == 0), stop=(s == 3))
                y_bf = wpool.tile([128, 512], BF16, tag="y_bf", bufs=3)
                nc.scalar.copy(y_bf[:], y_ps[:])
                y1_ps = psA.tile([128, 512], F32, tag="A")
                for f2 in range(4):
                    for h in range(4):
                        nc.tensor.matmul(
                            y1_ps[:, f2 * 128:(f2 + 1) * 128],
                            Wo1_bf[:, h * 512 + f2 * 128:h * 512 + (f2 + 1) * 128],
                            y_bf[:, h * 128:(h + 1) * 128],
                            start=(h == 0), stop=(h == 3))
                y1 = wpool.tile([128, 512], BF16, tag="y1", bufs=3)
                for f2 in range(4):
                    nc.scalar.activation(y1[:, f2 * 128:(f2 + 1) * 128],
                                         y1_ps[:, f2 * 128:(f2 + 1) * 128],
                                         AF.Gelu, bias=bo1p[:, f2:f2 + 1])
                if j == GS - 1:
                    # refresh the ACT-ordering token after this group's gelus
                    nc.scalar.activation(tok[:], y1[:, 0:1], AF.Copy, scale=0.0)
                o_ps = misc_ps[0:3, 192:320]
                for c2 in range(4):
                    nc.tensor.matmul(o_ps, Wo2_bf[:, c2 * 3:(c2 + 1) * 3],
                                     y1[:, c2 * 128:(c2 + 1) * 128],
                                     start=(c2 == 0), stop=(c2 == 3))
                o_sb = wpool.tile([3, 128], F32, tag="o_sb", bufs=3)
                nc.scalar.activation(o_sb[:], o_ps, AF.Identity, bias=bo2_t[:, 0:1])
                nc.sync.dma_start(outd[n0:n0 + 128, :].rearrange("n c -> c n"), o_sb[:])

            # ============ main loop: groups of GS chunks, A then B ============
            for g in range(nchunk // GS):
                acc = []
                for j in range(GS):
                    acc.append(phase_a(g * GS + j, j))
                for j in range(GS):
                    phase_b(g * GS + j, j, acc[j])

    nc.compile()
    return nc


def make_in_maps(inputs):
    x = np.asarray(inputs["x"], np.float32)
    f = {k: np.asarray(v, np.float32) for k, v in inputs.items()}

    # ---- host-side precompute of weight/latent-derived constants ----
    wcom = {k: np.ascontiguousarray(f[k]) for k in
            ["Wq1", "bq1", "Wq2", "bq2", "Wv1", "bv1", "Wv2", "bv2",
             "Wv", "Wo1", "Wo2", "bo2"]}
    wcom["Wsin"] = np.ascontiguousarray(
        -0.5 * np.concatenate([f["Wq_sin"], f["Wv_sin"]], axis=1))
    bias_e = np.concatenate([0.5 * f["Wq_sin"].sum(0), 0.5 * f["Wv_sin"].sum(0)])
    wcom["bias_e"] = np.ascontiguousarray(bias_e.reshape(H, 1))
    bo1p = f["bo1"] + f["Wo1"].T @ f["bv"]
    wcom["bo1p"] = np.ascontiguousarray(bo1p.reshape(4, 128).T)

    in_maps = []
    for core in range(NCORE):
        b = core // (NCORE // B)
        sh = (core % (NCORE // B))
        m = dict(wcom)
        m["x"] = np.ascontiguousarray(x[b, sh * NPC:(sh + 1) * NPC])
        p, c, g = f["p"][b], f["c"][b], f["g"][b]
        cstem = c @ f["W_stem"] + f["b_stem"]          # [L, H]
        m["c_tab"] = np.ascontiguousarray(cstem)
        m["k_tab"] = np.ascontiguousarray(cstem @ f["Wk"] + f["bk"])
        sm = np.concatenate([p, 1.0 / (g * g)], axis=1)  # [L, 3]
        m["sm_tab"] = np.ascontiguousarray(sm)
        pB = np.concatenate([p[:, 0], p[:, 1]])          # [2L]
        m["pB"] = np.ascontiguousarray(np.broadcast_to(pB, (128, 2 * L)))
        in_maps.append(m)
    return in_maps


def kernel(**inputs):
    import jax
    try:
        jax.config.update('jax_platforms', 'axon,cpu')
    except Exception:
        pass
    from concourse.bass_utils import run_bass_kernel_spmd

    nchunk = NPC // CHUNK
    if nchunk not in _cache:
        _cache[nchunk] = _build(nchunk)
    nc = _cache[nchunk]

    in_maps = make_in_maps(inputs)
    res = run_bass_kernel_spmd(nc, in_maps, core_ids=list(range(NCORE)))
    out = np.zeros((B, N, DOUT), np.float32)
    for core in range(NCORE):
        b = core // (NCORE // B)
        sh = core % (NCORE // B)
        out[b, sh * NPC:(sh + 1) * NPC] = res.results[core]["out"]
    return out


# revision 53
# speedup vs baseline: 2.1440x; 1.4078x over previous
"""Trainium2 Bass kernel for nn_EquivariantNeuralField.

Per-pixel top-4-nearest-latent cross-attention neural field.
Sharding: 8 cores; core i handles batch i//4, pixel rows (i%4)*4096..+4096.

v2: phase-split pipeline (A=trig table, B=gelu table) over 4-chunk groups
to kill activation-table thrashing; f32r single-pass gathers; latent-major
one-hot build (no big transposes); exp via tanh; DMA-broadcast attention.
"""
import numpy as np

B, N, L, K = 2, 16384, 256, 4
DIN, DOUT, DLAT, H, A, NH = 2, 3, 64, 128, 32, 4
NCORE = 8
NPC = N * B // NCORE          # pixels per core = 4096
CHUNK = 128
GS = 8                        # chunks per phase group
PI = float(np.pi)

_cache = {}


def _build(nchunk):
    import concourse.bacc as bacc
    import concourse.mybir as mybir
    from concourse.tile import TileContext

    F32 = mybir.dt.float32
    F32R = mybir.dt.float32r
    BF16 = mybir.dt.bfloat16
    I32 = mybir.dt.int32
    U32 = mybir.dt.uint32
    AF = mybir.ActivationFunctionType
    OP = mybir.AluOpType

    nc = bacc.Bacc()

    # ---------------- DRAM tensors ----------------
    # Tables and weight-derived constants are precomputed host-side in kernel().
    xd = nc.dram_tensor("x", [NPC, DIN], F32, kind="ExternalInput")
    pBd = nc.dram_tensor("pB", [128, 2 * L], F32, kind="ExternalInput")
    ktabd = nc.dram_tensor("k_tab", [L, NH * A], F32R, kind="ExternalInput")
    ctabd = nc.dram_tensor("c_tab", [L, H], F32R, kind="ExternalInput")
    smtabd = nc.dram_tensor("sm_tab", [L, 3], F32R, kind="ExternalInput")
    Wsin_d = nc.dram_tensor("Wsin", [DIN, H], F32R, kind="ExternalInput")
    biase_d = nc.dram_tensor("bias_e", [H, 1], F32, kind="ExternalInput")
    Wq1 = nc.dram_tensor("Wq1", [H + DIN, H], F32, kind="ExternalInput")
    bq1 = nc.dram_tensor("bq1", [H], F32, kind="ExternalInput")
    Wq2 = nc.dram_tensor("Wq2", [H, NH * A], F32R, kind="ExternalInput")
    bq2 = nc.dram_tensor("bq2", [NH * A], F32, kind="ExternalInput")
    Wv1 = nc.dram_tensor("Wv1", [H + DIN, H], F32, kind="ExternalInput")
    bv1 = nc.dram_tensor("bv1", [H], F32, kind="ExternalInput")
    Wv2 = nc.dram_tensor("Wv2", [H, 2 * H], F32R, kind="ExternalInput")
    bv2 = nc.dram_tensor("bv2", [2 * H], F32, kind="ExternalInput")
    Wv = nc.dram_tensor("Wv", [H, NH * H], F32, kind="ExternalInput")
    Wo1 = nc.dram_tensor("Wo1", [NH * H, NH * H], F32, kind="ExternalInput")
    bo1p_d = nc.dram_tensor("bo1p", [128, 4], F32, kind="ExternalInput")
    Wo2 = nc.dram_tensor("Wo2", [NH * H, DOUT], F32, kind="ExternalInput")
    bo2 = nc.dram_tensor("bo2", [DOUT], F32, kind="ExternalInput")
    outd = nc.dram_tensor("out", [NPC, DOUT], F32, kind="ExternalOutput")

    with TileContext(nc) as tc:
        with tc.tile_pool(name="const", bufs=1) as cpool, \
             tc.tile_pool(name="work", bufs=2) as wpool, \
             tc.tile_pool(name="psA", bufs=5, space="PSUM") as psA, \
             tc.tile_pool(name="psS", bufs=3, space="PSUM") as psS, \
             tc.tile_pool(name="drp", bufs=4, space="DRAM") as drpool:

            # ============ one-time constants ============
            idn_i = cpool.tile([128, 128], I32)
            nc.gpsimd.iota(idn_i[:], [[1, 128]], base=0, channel_multiplier=-1)
            idn_f0 = cpool.tile([128, 128], F32)
            nc.vector.tensor_copy(idn_f0[:], idn_i[:])
            ident = cpool.tile([128, 128], F32)
            nc.vector.tensor_scalar(ident[:], idn_f0[:], 0.0, None, OP.is_equal)
            ident_bf = cpool.tile([128, 128], BF16)
            nc.vector.tensor_copy(ident_bf[:], ident[:])
            # per-partition iota columns (f32): values p and p+128
            iop_i = cpool.tile([128, 1], I32)
            nc.gpsimd.iota(iop_i[:], [[1, 1]], base=0, channel_multiplier=1)
            iota0 = cpool.tile([128, 1], F32)
            nc.vector.tensor_copy(iota0[:], iop_i[:])
            iota1 = cpool.tile([128, 1], F32)
            nc.vector.tensor_scalar(iota1[:], iota0[:], 128.0, None, OP.add)
            # blockones [128, NH] f32r : bo[c, h] = (c//A == h)
            blockones_f = cpool.tile([128, NH], F32)
            nc.gpsimd.memset(blockones_f[:], 0.0)
            for h in range(NH):
                nc.gpsimd.memset(blockones_f[h * A:(h + 1) * A, h:h + 1], 1.0)
            blockones = cpool.tile([128, NH], F32R)
            nc.vector.tensor_copy(blockones[:], blockones_f[:])
            # ACT-ordering token: sin ops of group g+1 wait on gelu ops of g
            tok = cpool.tile([128, 1], F32)
            nc.gpsimd.memset(tok[:], 0.0)

            # ============ weights (host-precomputed, straight DMA loads) ===
            def load_cast(dram_ap, shape, dt, tag):
                if dt in (F32, F32R):
                    t0 = cpool.tile(shape, dt, tag=tag + "_d", name=tag)
                    nc.sync.dma_start(t0[:], dram_ap)
                    return t0
                t0 = wpool.tile([128, 512], F32, tag="stage", name="stage_" + tag)
                nc.sync.dma_start(t0[0:shape[0], 0:shape[1]], dram_ap)
                t1 = cpool.tile(shape, dt, tag=tag)
                nc.vector.tensor_copy(t1[:], t0[0:shape[0], 0:shape[1]])
                return t1

            def load_bias(dram, n, tag):
                if n <= 128:
                    t = cpool.tile([n, 1], F32, tag=tag)
                    nc.sync.dma_start(t[:], dram[:].rearrange("(n o) -> n o", o=1))
                    return t
                k = n // 128
                t = cpool.tile([128, k], F32, tag=tag)
                nc.sync.dma_start(t[:], dram[:].rearrange("(j p) -> p j", p=128))
                return t

            Wsin_t = load_cast(Wsin_d[:], [DIN, H], F32R, "wsin")
            bias_e = cpool.tile([H, 1], F32, tag="bias_e")
            nc.sync.dma_start(bias_e[:], biase_d[:])
            Wq1_cc = load_cast(Wq1[0:DIN, :], [DIN, H], BF16, "wq1cc")
            Wq1_sin = load_cast(Wq1[DIN:DIN + 64, :], [64, H], BF16, "wq1sin")
            Wq1_cos = load_cast(Wq1[DIN + 64:DIN + 128, :], [64, H], BF16, "wq1cos")
            Wv1_cc = load_cast(Wv1[0:DIN, :], [DIN, H], BF16, "wv1cc")
            Wv1_sf = cpool.tile([128, H], F32, tag="wv1sf")
            nc.sync.dma_start(Wv1_sf[64:128, :], Wv1[DIN:DIN + 64, :])
            Wv1_sin_t = cpool.tile([128, H], BF16, tag="wv1sin")
            nc.vector.tensor_copy(Wv1_sin_t[64:128, :], Wv1_sf[64:128, :])
            Wv1_cf = cpool.tile([128, H], F32, tag="wv1cf")
            nc.sync.dma_start(Wv1_cf[64:128, :], Wv1[DIN + 64:DIN + 128, :])
            Wv1_cos_t = cpool.tile([128, H], BF16, tag="wv1cos")
            nc.vector.tensor_copy(Wv1_cos_t[64:128, :], Wv1_cf[64:128, :])
            Wv1_sin = Wv1_sin_t[64:128, :]
            Wv1_cos = Wv1_cos_t[64:128, :]
            Wq2_t = load_cast(Wq2[:], [H, NH * A], F32R, "wq2")
            Wv2_t = load_cast(Wv2[:], [H, 2 * H], F32R, "wv2")
            Wv_bf = load_cast(Wv[:], [H, NH * H], BF16, "wv")
            # Wo1 as [128, (c2, f) 2048] bf16 (staged through rotating buffer)
            Wo1_bf = cpool.tile([128, 4 * 512], BF16, tag="wo1")
            for c2 in range(4):
                st = wpool.tile([128, 512], F32, tag="stage", name=f"wo1st{c2}")
                nc.sync.dma_start(st[:], Wo1[c2 * 128:(c2 + 1) * 128, :])
                nc.vector.tensor_copy(Wo1_bf[:, c2 * 512:(c2 + 1) * 512], st[:])
            Wo2_f32 = cpool.tile([128, 4 * DOUT], F32, tag="wo2f")
            for c2 in range(4):
                nc.sync.dma_start(Wo2_f32[:, c2 * DOUT:(c2 + 1) * DOUT],
                                  Wo2[c2 * 128:(c2 + 1) * 128, :])
            Wo2_bf = cpool.tile([128, 4 * DOUT], BF16, tag="wo2")
            nc.vector.tensor_copy(Wo2_bf[:], Wo2_f32[:])

            bq1_t = load_bias(bq1, H, "bq1")
            bq2_t = load_bias(bq2, NH * A, "bq2")
            bv1_t = load_bias(bv1, H, "bv1")
            bv2_t = load_bias(bv2, 2 * H, "bv2")
            bo2_t = load_bias(bo2, DOUT, "bo2")
            bo1p = cpool.tile([128, 4], F32, tag="bo1p")
            nc.sync.dma_start(bo1p[:], bo1p_d[:])

            # ============ latent tables (host-precomputed) ============
            k_tab, c_tab, s_tab = [], [], []
            for lc in range(2):
                kl = cpool.tile([128, NH * A], F32R, tag=f"kl{lc}")
                nc.sync.dma_start(kl[:], ktabd[lc * 128:(lc + 1) * 128, :])
                k_tab.append(kl)
                cn = cpool.tile([128, 128], F32R, tag=f"cn{lc}")
                nc.sync.dma_start(cn[:], ctabd[lc * 128:(lc + 1) * 128, :])
                c_tab.append(cn)
                smr = cpool.tile([128, 3], F32R, tag=f"smr{lc}")
                nc.sync.dma_start(smr[:], smtabd[lc * 128:(lc + 1) * 128, :])
                s_tab.append(smr)
            pB = cpool.tile([128, 2 * L], F32, tag="pB")
            nc.sync.dma_start(pB[:], pBd[:])

            # ============ phase A: distances, top-4, gathers, sin features ====
            def phase_a(ci, j):
                n0 = ci * CHUNK
                x0 = wpool.tile([128, 2], F32, tag=f"x0_{j}", bufs=1)
                nc.sync.dma_start(x0[:], xd[n0:n0 + 128, :])
                x2T = wpool.tile([2, 128], F32, tag=f"x2T_{j}", bufs=1)
                nc.sync.dma_start(x2T[:], xd[n0:n0 + 128, :].rearrange("n c -> c n"))
                d0 = wpool.tile([128, 256], F32, tag="d0")
                nc.gpsimd.tensor_scalar(d0[:], pB[:, 0:L], x0[:, 0:1], None, OP.subtract)
                d1 = wpool.tile([128, 256], F32, tag="d1")
                nc.gpsimd.tensor_scalar(d1[:], pB[:, L:2 * L], x0[:, 1:2], None, OP.subtract)
                sq0 = wpool.tile([128, 256], F32, tag="sq0")
                nc.gpsimd.tensor_tensor(sq0[:], d0[:], d0[:], OP.mult)
                sq1 = wpool.tile([128, 256], F32, tag="sq1")
                nc.gpsimd.tensor_tensor(sq1[:], d1[:], d1[:], OP.mult)
                nzx = wpool.tile([128, 256], F32, tag="nzx")
                nc.vector.scalar_tensor_tensor(nzx[:], sq0[:], -1.0, sq1[:],
                                               OP.mult, OP.subtract)
                m8 = wpool.tile([128, 8], F32, tag=f"m8_{j}", bufs=1)
                nc.vector.max(m8[:], nzx[:])
                i8 = wpool.tile([128, 8], U32, tag="i8", bufs=3)
                nc.vector.max_index(i8[:], m8[:], nzx[:])
                idxb = wpool.tile([128, 4], BF16, tag="idxb", bufs=3)
                nc.vector.tensor_copy(idxb[:], i8[:, 0:4])

                # --- one-hot, latent-major: ohT[l, s*128+p] = (idx[p,s] == l)
                # idx -> DRAM (s-major) -> broadcast-read to all 128 partitions
                idx_dr = drpool.tile([4, 128], BF16, tag="idx_dr")
                nc.sync.dma_start(idx_dr[:].rearrange("s p -> p s"), idxb[:])
                idxB = wpool.tile([128, 512], BF16, tag="idxB", bufs=2)
                nc.sync.dma_start(
                    idxB[:],
                    idx_dr[:].rearrange("r n -> (r n)")
                    .rearrange("(o f) -> o f", o=1).to_broadcast([128, 512]))
                ohT = [wpool.tile([128, 512], F32R, tag=f"ohT{lc}",
                                  name=f"ohT{lc}") for lc in range(2)]
                nc.gpsimd.tensor_scalar(ohT[0][:], idxB[:], iota0[:], None, OP.is_equal)
                nc.gpsimd.tensor_scalar(ohT[1][:], idxB[:], iota1[:], None, OP.is_equal)

                # --- gathers (single-pass f32r) ---
                ck_ps = psA.tile([128, 512], F32, tag="A")
                kk_ps = psA.tile([128, 512], F32, tag="A")
                smlg_ps = psS.tile([36, 512], F32, tag="S", name="smlg_s")
                sm_ps = smlg_ps[0:3, :]
                for lc in range(2):
                    nc.tensor.matmul(ck_ps[:], c_tab[lc][:], ohT[lc][:],
                                     start=(lc == 0), stop=(lc == 1))
                for lc in range(2):
                    nc.tensor.matmul(kk_ps[:], k_tab[lc][:], ohT[lc][:],
                                     start=(lc == 0), stop=(lc == 1))
                for lc in range(2):
                    nc.tensor.matmul(sm_ps, s_tab[lc][:], ohT[lc][:],
                                     start=(lc == 0), stop=(lc == 1))
                c_kT = wpool.tile([128, 512], BF16, tag=f"c_kT_{j}", bufs=1)
                nc.vector.tensor_copy(c_kT[:], ck_ps[:])
                k_kT = wpool.tile([128, 512], F32, tag=f"k_kT_{j}", bufs=1)
                nc.scalar.copy(k_kT[:], kk_ps[:])
                smT = wpool.tile([3, 512], F32, tag="smT", bufs=2)
                nc.scalar.copy(smT[:], sm_ps[:])
                # invg2 pixel-major [128, 12] for the softmax penalty
                smpm_ps = psS.tile([128, 16], F32, tag="S", name="smpm_s")[:, 0:12]
                for s in range(K):
                    nc.tensor.transpose(smpm_ps[:, s * 3:(s + 1) * 3],
                                        smT[:, s * 128:(s + 1) * 128], ident[0:3, 0:3])
                smpm = wpool.tile([128, 12], F32, tag=f"smpm_{j}", bufs=1)
                nc.vector.tensor_copy(smpm[:], smpm_ps[:])

                # --- sin features ---
                relp = wpool.tile([2, 512], F32R, tag="relp", bufs=2)
                nc.vector.tensor_tensor(
                    relp[:].rearrange("c (s n) -> c s n", s=4), sm_ps[0:2, :]
                    .rearrange("c (s n) -> c s n", s=4),
                    x2T[:].rearrange("c (s n) -> c s n", s=1).to_broadcast([2, 4, 128]),
                    OP.subtract)
                # cc = pi*(x - p + 1) = -pi*relp + pi ; f_cc = cc/(2pi) wrapped
                tcc = wpool.tile([2, 512], F32, tag="tcc", bufs=2)
                nc.vector.tensor_scalar(tcc[:], relp[:], -0.5, 0.5, OP.mult, OP.add)
                icc = wpool.tile([2, 512], I32, tag="icc", bufs=2)
                nc.gpsimd.tensor_copy(icc[:], tcc[:])
                fcc32 = wpool.tile([2, 512], F32, tag="fcc32", bufs=2)
                nc.gpsimd.tensor_copy(fcc32[:], icc[:])
                fcc = wpool.tile([2, 512], F32, tag="fcc", bufs=2)
                nc.vector.tensor_tensor(fcc[:], tcc[:], fcc32[:], OP.subtract)
                sincc = wpool.tile([2, 512], BF16, tag=f"sincc_{j}", bufs=1)
                nc.scalar.activation(sincc[:], fcc[:], AF.Sin, scale=float(2 * PI),
                                     bias=tok[0:2, 0:1])

                # te = e/(2pi) computed directly from relp via pre-folded weights
                # (Wsin pre-scaled by -0.5 on host; constant term added as ACT bias)
                e_ps = psA.tile([128, 512], F32, tag="A")
                nc.tensor.matmul(e_ps[:], Wsin_t[:], relp[:], start=True, stop=True)
                te = wpool.tile([128, 512], F32, tag="te", bufs=2)
                nc.scalar.activation(te[:], e_ps[:], AF.Identity, bias=bias_e[:, 0:1])
                ie = wpool.tile([128, 512], I32, tag="ie", bufs=2)
                nc.gpsimd.tensor_copy(ie[:], te[:])
                fe32 = wpool.tile([128, 512], F32, tag="fe32", bufs=2)
                nc.gpsimd.tensor_copy(fe32[:], ie[:])
                # fboth = [fe | 0.25-|fe|]; one Sin gives [sin(e) | cos(e)]
                fboth = wpool.tile([128, 1024], F32, tag="fboth", bufs=2)
                nc.vector.tensor_tensor(fboth[:, 0:512], te[:], fe32[:], OP.subtract)
                fab = wpool.tile([128, 512], F32, tag="fab", bufs=2)
                nc.vector.scalar_tensor_tensor(fab[:], fboth[:, 0:512], -1.0,
                                               fboth[:, 0:512], OP.mult, OP.max)
                nc.gpsimd.tensor_scalar(fboth[:, 512:1024], fab[:], -1.0, 0.25,
                                        OP.mult, OP.add)
                SCt = wpool.tile([128, 1024], BF16, tag=f"SCt_{j}", bufs=1)
                nc.scalar.activation(SCt[:], fboth[:], AF.Sin, scale=float(2 * PI),
                                     bias=tok[:, 0:1])
                return dict(SCt=SCt, sincc=sincc, c_kT=c_kT, k_kT=k_kT,
                            smpm=smpm, m8=m8)

            # ============ phase B1: q-side MLP, softmax, att DMAs ============
            def phase_b1(ci, j, a):
                SCt, sincc = a["SCt"], a["sincc"]
                k_kT, smpm, m8 = a["k_kT"], a["smpm"], a["m8"]

                h1q_ps = psA.tile([128, 512], F32, tag="A")
                nc.tensor.matmul(h1q_ps[:], Wq1_sin[:], SCt[0:64, 0:512], start=True, stop=False)
                nc.tensor.matmul(h1q_ps[:], Wq1_cos[:], SCt[0:64, 512:1024], start=False, stop=False)
                nc.tensor.matmul(h1q_ps[:], Wq1_cc[:], sincc[:], start=False, stop=True)
                h1q = wpool.tile([128, 512], F32R, tag="h1q", bufs=2)
                nc.scalar.activation(h1q[:], h1q_ps[:], AF.Gelu, bias=bq1_t[:, 0:1])
                q_ps = psA.tile([128, 512], F32, tag="A")
                nc.tensor.matmul(q_ps[:], Wq2_t[:], h1q[:], start=True, stop=True)

                qk = wpool.tile([128, 512], F32R, tag="qk", bufs=2)
                nc.vector.scalar_tensor_tensor(qk[:], q_ps[:], bq2_t[:, 0:1], k_kT[:],
                                               OP.add, OP.mult)

                # ---- logits + softmax (pixel-major), exp via tanh ----
                lg_ps = psS.tile([4, 512], F32, tag="S", name="lg_s")
                nc.tensor.matmul(lg_ps[:], blockones[:], qk[:], start=True, stop=True)
                lg_sb = wpool.tile([4, 512], F32, tag="lg_sb", bufs=2)
                nc.vector.tensor_copy(lg_sb[:], lg_ps[:])
                misc_ps = psS.tile([128, 512], F32, tag="S", name="misc_s")
                lgpm_ps = misc_ps[:, 0:16]
                for s in range(K):
                    nc.tensor.transpose(lgpm_ps[:, s * 4:(s + 1) * 4],
                                        lg_sb[:, s * 128:(s + 1) * 128], ident[0:4, 0:4])
                pen = wpool.tile([128, 4], F32, tag="pen", bufs=3)  # -zx*invg2
                nc.vector.tensor_tensor(
                    pen[:],
                    smpm[:].rearrange("p (s c) -> p s c", c=3)[:, :, 2:3]
                    .rearrange("p s o -> p (s o)"),
                    m8[:, 0:4], OP.mult)
                lgpm = wpool.tile([128, 16], F32, tag="lgpm", bufs=3)
                nc.vector.scalar_tensor_tensor(
                    lgpm[:].rearrange("p (s h) -> p s h", s=4),
                    lgpm_ps[:].rearrange("p (s h) -> p s h", s=4), 0.0,
                    pen[:].to_broadcast([128, 4, 4]), OP.add, OP.add)
                # exp(x) = (1+t)/(1-t), t = tanh(x/2)  (keeps ACT in gelu set)
                # logits are bounded (~[-10, 1]); no max-subtraction needed
                th = wpool.tile([128, 16], F32, tag="th", bufs=3)
                nc.scalar.activation(th[:], lgpm[:], AF.Tanh, scale=0.5)
                num = wpool.tile([128, 16], F32, tag="num", bufs=3)
                nc.vector.tensor_scalar(num[:], th[:], 1.0, None, OP.add)
                den = wpool.tile([128, 16], F32, tag="den", bufs=3)
                nc.vector.tensor_scalar(den[:], th[:], -1.0, 1.0, OP.mult, OP.add)
                rcp = wpool.tile([128, 16], F32, tag="rcp", bufs=3)
                nc.vector.reciprocal(rcp[:], den[:])
                epm = wpool.tile([128, 16], F32, tag="epm", bufs=3)
                nc.vector.tensor_tensor(epm[:], num[:], rcp[:], OP.mult)
                zs = wpool.tile([128, 4], F32, tag="zs", bufs=3)
                nc.vector.tensor_reduce(
                    zs[:], epm[:].rearrange("p (s h) -> p h s", s=4),
                    mybir.AxisListType.X, OP.add)
                rz = wpool.tile([128, 4], F32, tag="rz", bufs=3)
                nc.vector.reciprocal(rz[:], zs[:])
                att_pm = wpool.tile([128, 16], F32, tag="att_pm", bufs=4)
                nc.vector.tensor_tensor(
                    att_pm[:].rearrange("p (h s) -> p s h", h=4),
                    epm[:].rearrange("p (s h) -> p s h", s=4),
                    rz[:].rearrange("p (h o) -> p o h", o=1).to_broadcast([128, 4, 4]),
                    OP.mult)
                att_ps = misc_ps[0:16, 64:192]
                nc.tensor.transpose(att_ps, att_pm[:], ident[:])
                att_sh = wpool.tile([16, 128], BF16, tag="att_sh", bufs=4)
                nc.vector.tensor_copy(att_sh[:], att_ps)
                att_dr = drpool.tile([16, 128], BF16, tag="att_dr")
                nc.sync.dma_start(att_dr[:], att_sh[:])
                # broadcast att rows to all 128 partitions: [128, (h,s,p) 2048]
                attB = wpool.tile([128, 2048], BF16, tag="attB", bufs=3)
                nc.sync.dma_start(
                    attB[:],
                    att_dr[:].rearrange("r n -> (r n)")
                    .rearrange("(o f) -> o f", o=1).to_broadcast([128, 2048]))

                return dict(attB=attB)

            # ============ phase B2: v-side MLP, attention apply, output ======
            def phase_b2(ci, j, a, b):
                n0 = ci * CHUNK
                SCt, sincc, c_kT = a["SCt"], a["sincc"], a["c_kT"]
                attB = b["attB"]
                h1v_ps = psA.tile([128, 512], F32, tag="A")
                nc.tensor.matmul(h1v_ps[:], Wv1_sin, SCt[64:128, 0:512], start=True, stop=False)
                nc.tensor.matmul(h1v_ps[:], Wv1_cos, SCt[64:128, 512:1024], start=False, stop=False)
                nc.tensor.matmul(h1v_ps[:], Wv1_cc[:], sincc[:], start=False, stop=True)
                h1v = wpool.tile([128, 512], F32R, tag="h1v", bufs=2)
                nc.scalar.activation(h1v[:], h1v_ps[:], AF.Gelu, bias=bv1_t[:, 0:1])
                vg_ps = psA.tile([128, 512], F32, tag="A")
                nc.tensor.matmul(vg_ps[:], Wv2_t[:, 0:H], h1v[:], start=True, stop=True)
                vb_ps = psA.tile([128, 512], F32, tag="A")
                nc.tensor.matmul(vb_ps[:], Wv2_t[:, H:2 * H], h1v[:], start=True, stop=True)
                utmp = wpool.tile([128, 512], F32, tag="utmp", bufs=2)
                nc.vector.scalar_tensor_tensor(utmp[:], vg_ps[:], bv2_t[:, 0:1],
                                               c_kT[:], OP.add, OP.mult)
                u_bf = wpool.tile([128, 512], BF16, tag="u_bf", bufs=3)
                nc.vector.scalar_tensor_tensor(u_bf[:], vb_ps[:], bv2_t[:, 1:2],
                                               utmp[:], OP.add, OP.add)

                # ---- apply attention + output MLP ----
                uw = wpool.tile([128, 2048], BF16, tag="uw", bufs=3)
                for h in range(NH):
                    nc.gpsimd.tensor_tensor(uw[:, h * 512:(h + 1) * 512], u_bf[:],
                                            attB[:, h * 512:(h + 1) * 512], OP.mult)
                y_ps = psA.tile([128, 512], F32, tag="A")
                for h in range(NH):
                    for s in range(K):
                        nc.tensor.matmul(
                            y_ps[:, h * 128:(h + 1) * 128],
                            Wv_bf[:, h * 128:(h + 1) * 128],
                            uw[:, h * 512 + s * 128:h * 512 + (s + 1) * 128],
                            start=(s == 0), stop=(s == 3))
                y_bf = wpool.tile([128, 512], BF16, tag="y_bf", bufs=3)
                nc.vector.tensor_copy(y_bf[:], y_ps[:])
                y1_ps = psA.tile([128, 512], F32, tag="A")
                for f2 in range(4):
                    for h in range(4):
                        nc.tensor.matmul(
                            y1_ps[:, f2 * 128:(f2 + 1) * 128],
                            Wo1_bf[:, h * 512 + f2 * 128:h * 512 + (f2 + 1) * 128],
                            y_bf[:, h * 128:(h + 1) * 128],
                            start=(h == 0), stop=(h == 3))
                y1 = wpool.tile([128, 512], BF16, tag="y1", bufs=3)
                for f2 in range(4):
                    nc.scalar.activation(y1[:, f2 * 128:(f2 + 1) * 128],
                                         y1_ps[:, f2 * 128:(f2 + 1) * 128],
                                         AF.Gelu, bias=bo1p[:, f2:f2 + 1])
                if j == GS - 1:
                    # refresh the ACT-ordering token after this group's gelus
                    nc.scalar.activation(tok[:], y1[:, 0:1], AF.Copy, scale=0.0)
                misc2_ps = psS.tile([128, 512], F32, tag="S", name="misc2_s")
                o_ps = misc2_ps[0:3, 0:128]
                for c2 in range(4):
                    nc.tensor.matmul(o_ps, Wo2_bf[:, c2 * 3:(c2 + 1) * 3],
                                     y1[:, c2 * 128:(c2 + 1) * 128],
                                     start=(c2 == 0), stop=(c2 == 3))
                o_sb = wpool.tile([3, 128], F32, tag="o_sb", bufs=3)
                nc.scalar.activation(o_sb[:], o_ps, AF.Identity, bias=bo2_t[:, 0:1])
                nc.sync.dma_start(outd[n0:n0 + 128, :].rearrange("n c -> c n"), o_sb[:])

            # ============ main loop: groups of GS chunks, A then B1/B2 =======
            # B1(j+1) is emitted before B2(j) so the next chunk's q-side MLP
            # fills the attention-broadcast DMA latency.
            for g in range(nchunk // GS):
                acc = []
                for j in range(GS):
                    acc.append(phase_a(g * GS + j, j))
                bts = [phase_b1(g * GS, 0, acc[0])]
                for j in range(GS):
                    if j + 1 < GS:
                        bts.append(phase_b1(g * GS + j + 1, j + 1, acc[j + 1]))
                    phase_b2(g * GS + j, j, acc[j], bts[j])

    nc.compile()
    return nc


def make_in_maps(inputs):
    x = np.asarray(inputs["x"], np.float32)
    f = {k: np.asarray(v, np.float32) for k, v in inputs.items()}

    # ---- host-side precompute of weight/latent-derived constants ----
    wcom = {k: np.ascontiguousarray(f[k]) for k in
            ["Wq1", "bq1", "Wq2", "bq2", "Wv1", "bv1", "Wv2", "bv2",
             "Wv", "Wo1", "Wo2", "bo2"]}
    wcom["Wsin"] = np.ascontiguousarray(
        -0.5 * np.concatenate([f["Wq_sin"], f["Wv_sin"]], axis=1))
    bias_e = np.concatenate([0.5 * f["Wq_sin"].sum(0), 0.5 * f["Wv_sin"].sum(0)])
    wcom["bias_e"] = np.ascontiguousarray(bias_e.reshape(H, 1))
    bo1p = f["bo1"] + f["Wo1"].T @ f["bv"]
    wcom["bo1p"] = np.ascontiguousarray(bo1p.reshape(4, 128).T)

    in_maps = []
    for core in range(NCORE):
        b = core // (NCORE // B)
        sh = (core % (NCORE // B))
        m = dict(wcom)
        m["x"] = np.ascontiguousarray(x[b, sh * NPC:(sh + 1) * NPC])
        p, c, g = f["p"][b], f["c"][b], f["g"][b]
        cstem = c @ f["W_stem"] + f["b_stem"]          # [L, H]
        m["c_tab"] = np.ascontiguousarray(cstem)
        m["k_tab"] = np.ascontiguousarray(cstem @ f["Wk"] + f["bk"])
        sm = np.concatenate([p, 1.0 / (g * g)], axis=1)  # [L, 3]
        m["sm_tab"] = np.ascontiguousarray(sm)
        pB = np.concatenate([p[:, 0], p[:, 1]])          # [2L]
        m["pB"] = np.ascontiguousarray(np.broadcast_to(pB, (128, 2 * L)))
        in_maps.append(m)
    return in_maps


def kernel(**inputs):
    import jax
    try:
        jax.config.update('jax_platforms', 'axon,cpu')
    except Exception:
        pass
    from concourse.bass_utils import run_bass_kernel_spmd

    nchunk = NPC // CHUNK
    if nchunk not in _cache:
        _cache[nchunk] = _build(nchunk)
    nc = _cache[nchunk]

    in_maps = make_in_maps(inputs)
    res = run_bass_kernel_spmd(nc, in_maps, core_ids=list(range(NCORE)))
    out = np.zeros((B, N, DOUT), np.float32)
    for core in range(NCORE):
        b = core // (NCORE // B)
        sh = core % (NCORE // B)
        out[b, sh * NPC:(sh + 1) * NPC] = res.results[core]["out"]
    return out


# revision 55
# speedup vs baseline: 2.1580x; 1.0065x over previous
"""Trainium2 Bass kernel for nn_EquivariantNeuralField.

Per-pixel top-4-nearest-latent cross-attention neural field.
Sharding: 8 cores; core i handles batch i//4, pixel rows (i%4)*4096..+4096.

v2: phase-split pipeline (A=trig table, B=gelu table) over 4-chunk groups
to kill activation-table thrashing; f32r single-pass gathers; latent-major
one-hot build (no big transposes); exp via tanh; DMA-broadcast attention.
"""
import numpy as np

B, N, L, K = 2, 16384, 256, 4
DIN, DOUT, DLAT, H, A, NH = 2, 3, 64, 128, 32, 4
NCORE = 8
NPC = N * B // NCORE          # pixels per core = 4096
CHUNK = 128
GS = 8                        # chunks per phase group
PI = float(np.pi)

_cache = {}


def _build(nchunk):
    import concourse.bacc as bacc
    import concourse.mybir as mybir
    from concourse.tile import TileContext

    F32 = mybir.dt.float32
    F32R = mybir.dt.float32r
    BF16 = mybir.dt.bfloat16
    I32 = mybir.dt.int32
    U32 = mybir.dt.uint32
    AF = mybir.ActivationFunctionType
    OP = mybir.AluOpType

    nc = bacc.Bacc()

    # ---------------- DRAM tensors ----------------
    # Tables and weight-derived constants are precomputed host-side in kernel().
    xd = nc.dram_tensor("x", [NPC, DIN], F32, kind="ExternalInput")
    pBd = nc.dram_tensor("pB", [128, 2 * L], F32, kind="ExternalInput")
    ktabd = nc.dram_tensor("k_tab", [L, NH * A], F32R, kind="ExternalInput")
    ctabd = nc.dram_tensor("c_tab", [L, H], F32R, kind="ExternalInput")
    smtabd = nc.dram_tensor("sm_tab", [L, 3], F32R, kind="ExternalInput")
    Wsin_d = nc.dram_tensor("Wsin", [DIN, H], F32R, kind="ExternalInput")
    biase_d = nc.dram_tensor("bias_e", [H, 1], F32, kind="ExternalInput")
    Wq1 = nc.dram_tensor("Wq1", [H + DIN, H], F32, kind="ExternalInput")
    bq1 = nc.dram_tensor("bq1", [H], F32, kind="ExternalInput")
    Wq2 = nc.dram_tensor("Wq2", [H, NH * A], F32R, kind="ExternalInput")
    bq2 = nc.dram_tensor("bq2", [NH * A], F32, kind="ExternalInput")
    Wv1 = nc.dram_tensor("Wv1", [H + DIN, H], F32, kind="ExternalInput")
    bv1 = nc.dram_tensor("bv1", [H], F32, kind="ExternalInput")
    Wv2 = nc.dram_tensor("Wv2", [H, 2 * H], F32R, kind="ExternalInput")
    bv2 = nc.dram_tensor("bv2", [2 * H], F32, kind="ExternalInput")
    Wv = nc.dram_tensor("Wv", [H, NH * H], F32, kind="ExternalInput")
    Wo1 = nc.dram_tensor("Wo1", [NH * H, NH * H], F32, kind="ExternalInput")
    bo1p_d = nc.dram_tensor("bo1p", [128, 4], F32, kind="ExternalInput")
    Wo2 = nc.dram_tensor("Wo2", [NH * H, DOUT], F32, kind="ExternalInput")
    bo2 = nc.dram_tensor("bo2", [DOUT], F32, kind="ExternalInput")
    outd = nc.dram_tensor("out", [NPC, DOUT], F32, kind="ExternalOutput")

    with TileContext(nc) as tc:
        with tc.tile_pool(name="const", bufs=1) as cpool, \
             tc.tile_pool(name="work", bufs=2) as wpool, \
             tc.tile_pool(name="psA", bufs=5, space="PSUM") as psA, \
             tc.tile_pool(name="psS", bufs=3, space="PSUM") as psS, \
             tc.tile_pool(name="drp", bufs=4, space="DRAM") as drpool:

            # ============ one-time constants ============
            idn_i = cpool.tile([128, 128], I32)
            nc.gpsimd.iota(idn_i[:], [[1, 128]], base=0, channel_multiplier=-1)
            idn_f0 = cpool.tile([128, 128], F32)
            nc.vector.tensor_copy(idn_f0[:], idn_i[:])
            ident = cpool.tile([128, 128], F32)
            nc.vector.tensor_scalar(ident[:], idn_f0[:], 0.0, None, OP.is_equal)
            ident_bf = cpool.tile([128, 128], BF16)
            nc.vector.tensor_copy(ident_bf[:], ident[:])
            # per-partition iota columns (f32): values p and p+128
            iop_i = cpool.tile([128, 1], I32)
            nc.gpsimd.iota(iop_i[:], [[1, 1]], base=0, channel_multiplier=1)
            iota0 = cpool.tile([128, 1], F32)
            nc.vector.tensor_copy(iota0[:], iop_i[:])
            iota1 = cpool.tile([128, 1], F32)
            nc.vector.tensor_scalar(iota1[:], iota0[:], 128.0, None, OP.add)
            # blockones [128, NH] f32r : bo[c, h] = (c//A == h)
            blockones_f = cpool.tile([128, NH], F32)
            nc.gpsimd.memset(blockones_f[:], 0.0)
            for h in range(NH):
                nc.gpsimd.memset(blockones_f[h * A:(h + 1) * A, h:h + 1], 1.0)
            blockones = cpool.tile([128, NH], F32R)
            nc.vector.tensor_copy(blockones[:], blockones_f[:])
            # ACT-ordering token: sin ops of group g+1 wait on gelu ops of g
            tok = cpool.tile([128, 1], F32)
            nc.gpsimd.memset(tok[:], 0.0)

            # ============ weights (host-precomputed, straight DMA loads) ===
            def load_cast(dram_ap, shape, dt, tag):
                if dt in (F32, F32R):
                    t0 = cpool.tile(shape, dt, tag=tag + "_d", name=tag)
                    nc.sync.dma_start(t0[:], dram_ap)
                    return t0
                t0 = wpool.tile([128, 512], F32, tag="stage", name="stage_" + tag)
                nc.sync.dma_start(t0[0:shape[0], 0:shape[1]], dram_ap)
                t1 = cpool.tile(shape, dt, tag=tag)
                nc.vector.tensor_copy(t1[:], t0[0:shape[0], 0:shape[1]])
                return t1

            def load_bias(dram, n, tag):
                if n <= 128:
                    t = cpool.tile([n, 1], F32, tag=tag)
                    nc.sync.dma_start(t[:], dram[:].rearrange("(n o) -> n o", o=1))
                    return t
                k = n // 128
                t = cpool.tile([128, k], F32, tag=tag)
                nc.sync.dma_start(t[:], dram[:].rearrange("(j p) -> p j", p=128))
                return t

            Wsin_t = load_cast(Wsin_d[:], [DIN, H], F32R, "wsin")
            bias_e = cpool.tile([H, 1], F32, tag="bias_e")
            nc.sync.dma_start(bias_e[:], biase_d[:])
            Wq1_cc = load_cast(Wq1[0:DIN, :], [DIN, H], BF16, "wq1cc")
            Wq1_sin = load_cast(Wq1[DIN:DIN + 64, :], [64, H], BF16, "wq1sin")
            Wq1_cos = load_cast(Wq1[DIN + 64:DIN + 128, :], [64, H], BF16, "wq1cos")
            Wv1_cc = load_cast(Wv1[0:DIN, :], [DIN, H], BF16, "wv1cc")
            Wv1_sf = cpool.tile([128, H], F32, tag="wv1sf")
            nc.sync.dma_start(Wv1_sf[64:128, :], Wv1[DIN:DIN + 64, :])
            Wv1_sin_t = cpool.tile([128, H], BF16, tag="wv1sin")
            nc.vector.tensor_copy(Wv1_sin_t[64:128, :], Wv1_sf[64:128, :])
            Wv1_cf = cpool.tile([128, H], F32, tag="wv1cf")
            nc.sync.dma_start(Wv1_cf[64:128, :], Wv1[DIN + 64:DIN + 128, :])
            Wv1_cos_t = cpool.tile([128, H], BF16, tag="wv1cos")
            nc.vector.tensor_copy(Wv1_cos_t[64:128, :], Wv1_cf[64:128, :])
            Wv1_sin = Wv1_sin_t[64:128, :]
            Wv1_cos = Wv1_cos_t[64:128, :]
            Wq2_t = load_cast(Wq2[:], [H, NH * A], F32R, "wq2")
            Wv2_t = load_cast(Wv2[:], [H, 2 * H], F32R, "wv2")
            Wv_bf = load_cast(Wv[:], [H, NH * H], BF16, "wv")
            # Wo1 as [128, (c2, f) 2048] bf16 (staged through rotating buffer)
            Wo1_bf = cpool.tile([128, 4 * 512], BF16, tag="wo1")
            for c2 in range(4):
                st = wpool.tile([128, 512], F32, tag="stage", name=f"wo1st{c2}")
                nc.sync.dma_start(st[:], Wo1[c2 * 128:(c2 + 1) * 128, :])
                nc.vector.tensor_copy(Wo1_bf[:, c2 * 512:(c2 + 1) * 512], st[:])
            Wo2_f32 = cpool.tile([128, 4 * DOUT], F32, tag="wo2f")
            for c2 in range(4):
                nc.sync.dma_start(Wo2_f32[:, c2 * DOUT:(c2 + 1) * DOUT],
                                  Wo2[c2 * 128:(c2 + 1) * 128, :])
            Wo2_bf = cpool.tile([128, 4 * DOUT], BF16, tag="wo2")
            nc.vector.tensor_copy(Wo2_bf[:], Wo2_f32[:])

            bq1_t = load_bias(bq1, H, "bq1")
            bq2_t = load_bias(bq2, NH * A, "bq2")
            bv1_t = load_bias(bv1, H, "bv1")
            bv2_t = load_bias(bv2, 2 * H, "bv2")
            bo2_t = load_bias(bo2, DOUT, "bo2")
            bo1p = cpool.tile([128, 4], F32, tag="bo1p")
            nc.sync.dma_start(bo1p[:], bo1p_d[:])

            # ============ latent tables (host-precomputed) ============
            k_tab, c_tab, s_tab = [], [], []
            for lc in range(2):
                kl = cpool.tile([128, NH * A], F32R, tag=f"kl{lc}")
                nc.sync.dma_start(kl[:], ktabd[lc * 128:(lc + 1) * 128, :])
                k_tab.append(kl)
                cn = cpool.tile([128, 128], F32R, tag=f"cn{lc}")
                nc.sync.dma_start(cn[:], ctabd[lc * 128:(lc + 1) * 128, :])
                c_tab.append(cn)
                smr = cpool.tile([128, 3], F32R, tag=f"smr{lc}")
                nc.sync.dma_start(smr[:], smtabd[lc * 128:(lc + 1) * 128, :])
                s_tab.append(smr)
            pB = cpool.tile([128, 2 * L], F32, tag="pB")
            nc.sync.dma_start(pB[:], pBd[:])

            # ===== phase A1: distances, top-4, idx round trip =====
            def phase_a1(ci, j):
                n0 = ci * CHUNK
                x0 = wpool.tile([128, 2], F32, tag=f"x0_{j}", bufs=1)
                nc.sync.dma_start(x0[:], xd[n0:n0 + 128, :])
                x2T = wpool.tile([2, 128], F32, tag=f"x2T_{j}", bufs=1)
                nc.sync.dma_start(x2T[:], xd[n0:n0 + 128, :].rearrange("n c -> c n"))
                d0 = wpool.tile([128, 256], F32, tag="d0")
                nc.gpsimd.tensor_scalar(d0[:], pB[:, 0:L], x0[:, 0:1], None, OP.subtract)
                d1 = wpool.tile([128, 256], F32, tag="d1")
                nc.gpsimd.tensor_scalar(d1[:], pB[:, L:2 * L], x0[:, 1:2], None, OP.subtract)
                sq0 = wpool.tile([128, 256], F32, tag="sq0")
                nc.gpsimd.tensor_tensor(sq0[:], d0[:], d0[:], OP.mult)
                sq1 = wpool.tile([128, 256], F32, tag="sq1")
                nc.gpsimd.tensor_tensor(sq1[:], d1[:], d1[:], OP.mult)
                nzx = wpool.tile([128, 256], F32, tag="nzx")
                nc.vector.scalar_tensor_tensor(nzx[:], sq0[:], -1.0, sq1[:],
                                               OP.mult, OP.subtract)
                m8 = wpool.tile([128, 8], F32, tag=f"m8_{j}", bufs=1)
                nc.vector.max(m8[:], nzx[:])
                i8 = wpool.tile([128, 8], U32, tag="i8", bufs=3)
                nc.vector.max_index(i8[:], m8[:], nzx[:])
                idxb = wpool.tile([128, 4], BF16, tag="idxb", bufs=3)
                nc.vector.tensor_copy(idxb[:], i8[:, 0:4])

                # --- one-hot, latent-major: ohT[l, s*128+p] = (idx[p,s] == l)
                # idx -> DRAM (s-major) -> broadcast-read to all 128 partitions
                idx_dr = drpool.tile([4, 128], BF16, tag="idx_dr")
                nc.sync.dma_start(idx_dr[:].rearrange("s p -> p s"), idxb[:])
                idxB = wpool.tile([128, 512], BF16, tag="idxB", bufs=3)
                nc.sync.dma_start(
                    idxB[:],
                    idx_dr[:].rearrange("r n -> (r n)")
                    .rearrange("(o f) -> o f", o=1).to_broadcast([128, 512]))
                return dict(idxB=idxB, m8=m8, x0=x0, x2T=x2T)

            # ===== phase A2: gathers + sin features =====
            def phase_a2(ci, j, a1):
                idxB, m8, x2T = a1["idxB"], a1["m8"], a1["x2T"]
                ohT = [wpool.tile([128, 512], F32R, tag=f"ohT{lc}",
                                  name=f"ohT{lc}") for lc in range(2)]
                nc.gpsimd.tensor_scalar(ohT[0][:], idxB[:], iota0[:], None, OP.is_equal)
                nc.gpsimd.tensor_scalar(ohT[1][:], idxB[:], iota1[:], None, OP.is_equal)

                # --- gathers (single-pass f32r) ---
                ck_ps = psA.tile([128, 512], F32, tag="A")
                kk_ps = psA.tile([128, 512], F32, tag="A")
                smlg_ps = psS.tile([36, 512], F32, tag="S", name="smlg_s")
                sm_ps = smlg_ps[0:3, :]
                for lc in range(2):
                    nc.tensor.matmul(ck_ps[:], c_tab[lc][:], ohT[lc][:],
                                     start=(lc == 0), stop=(lc == 1))
                for lc in range(2):
                    nc.tensor.matmul(kk_ps[:], k_tab[lc][:], ohT[lc][:],
                                     start=(lc == 0), stop=(lc == 1))
                for lc in range(2):
                    nc.tensor.matmul(sm_ps, s_tab[lc][:], ohT[lc][:],
                                     start=(lc == 0), stop=(lc == 1))
                c_kT = wpool.tile([128, 512], BF16, tag=f"c_kT_{j}", bufs=1)
                nc.vector.tensor_copy(c_kT[:], ck_ps[:])
                k_kT = wpool.tile([128, 512], F32, tag=f"k_kT_{j}", bufs=1)
                nc.scalar.copy(k_kT[:], kk_ps[:])
                smT = wpool.tile([3, 512], F32, tag="smT", bufs=2)
                nc.scalar.copy(smT[:], sm_ps[:])
                # invg2 pixel-major [128, 12] for the softmax penalty
                smpm_ps = psS.tile([128, 16], F32, tag="S", name="smpm_s")[:, 0:12]
                for s in range(K):
                    nc.tensor.transpose(smpm_ps[:, s * 3:(s + 1) * 3],
                                        smT[:, s * 128:(s + 1) * 128], ident[0:3, 0:3])
                smpm = wpool.tile([128, 12], F32, tag=f"smpm_{j}", bufs=1)
                nc.vector.tensor_copy(smpm[:], smpm_ps[:])

                # --- sin features ---
                relp = wpool.tile([2, 512], F32R, tag="relp", bufs=2)
                nc.vector.tensor_tensor(
                    relp[:].rearrange("c (s n) -> c s n", s=4), sm_ps[0:2, :]
                    .rearrange("c (s n) -> c s n", s=4),
                    x2T[:].rearrange("c (s n) -> c s n", s=1).to_broadcast([2, 4, 128]),
                    OP.subtract)
                # cc = pi*(x - p + 1) = -pi*relp + pi ; f_cc = cc/(2pi) wrapped
                tcc = wpool.tile([2, 512], F32, tag="tcc", bufs=2)
                nc.vector.tensor_scalar(tcc[:], relp[:], -0.5, 0.5, OP.mult, OP.add)
                icc = wpool.tile([2, 512], I32, tag="icc", bufs=2)
                nc.gpsimd.tensor_copy(icc[:], tcc[:])
                fcc32 = wpool.tile([2, 512], F32, tag="fcc32", bufs=2)
                nc.gpsimd.tensor_copy(fcc32[:], icc[:])
                fcc = wpool.tile([2, 512], F32, tag="fcc", bufs=2)
                nc.vector.tensor_tensor(fcc[:], tcc[:], fcc32[:], OP.subtract)
                sincc = wpool.tile([2, 512], BF16, tag=f"sincc_{j}", bufs=1)
                nc.scalar.activation(sincc[:], fcc[:], AF.Sin, scale=float(2 * PI),
                                     bias=tok[0:2, 0:1])

                # te = e/(2pi) computed directly from relp via pre-folded weights
                # (Wsin pre-scaled by -0.5 on host; constant term added as ACT bias)
                e_ps = psA.tile([128, 512], F32, tag="A")
                nc.tensor.matmul(e_ps[:], Wsin_t[:], relp[:], start=True, stop=True)
                te = wpool.tile([128, 512], F32, tag="te", bufs=2)
                nc.scalar.activation(te[:], e_ps[:], AF.Identity, bias=bias_e[:, 0:1])
                ie = wpool.tile([128, 512], I32, tag="ie", bufs=2)
                nc.gpsimd.tensor_copy(ie[:], te[:])
                fe32 = wpool.tile([128, 512], F32, tag="fe32", bufs=2)
                nc.gpsimd.tensor_copy(fe32[:], ie[:])
                # fboth = [fe | 0.25-|fe|]; one Sin gives [sin(e) | cos(e)]
                fboth = wpool.tile([128, 1024], F32, tag="fboth", bufs=2)
                nc.vector.tensor_tensor(fboth[:, 0:512], te[:], fe32[:], OP.subtract)
                fab = wpool.tile([128, 512], F32, tag="fab", bufs=2)
                nc.vector.scalar_tensor_tensor(fab[:], fboth[:, 0:512], -1.0,
                                               fboth[:, 0:512], OP.mult, OP.max)
                nc.gpsimd.tensor_scalar(fboth[:, 512:1024], fab[:], -1.0, 0.25,
                                        OP.mult, OP.add)
                SCt = wpool.tile([128, 1024], BF16, tag=f"SCt_{j}", bufs=1)
                nc.scalar.activation(SCt[:], fboth[:], AF.Sin, scale=float(2 * PI),
                                     bias=tok[:, 0:1])
                return dict(SCt=SCt, sincc=sincc, c_kT=c_kT, k_kT=k_kT,
                            smpm=smpm, m8=m8)

            # ============ phase B1: q-side MLP, softmax, att DMAs ============
            def phase_b1(ci, j, a):
                SCt, sincc = a["SCt"], a["sincc"]
                k_kT, smpm, m8 = a["k_kT"], a["smpm"], a["m8"]

                h1q_ps = psA.tile([128, 512], F32, tag="A")
                nc.tensor.matmul(h1q_ps[:], Wq1_sin[:], SCt[0:64, 0:512], start=True, stop=False)
                nc.tensor.matmul(h1q_ps[:], Wq1_cos[:], SCt[0:64, 512:1024], start=False, stop=False)
                nc.tensor.matmul(h1q_ps[:], Wq1_cc[:], sincc[:], start=False, stop=True)
                h1q = wpool.tile([128, 512], F32R, tag="h1q", bufs=2)
                nc.scalar.activation(h1q[:], h1q_ps[:], AF.Gelu, bias=bq1_t[:, 0:1])
                q_ps = psA.tile([128, 512], F32, tag="A")
                nc.tensor.matmul(q_ps[:], Wq2_t[:], h1q[:], start=True, stop=True)

                qk = wpool.tile([128, 512], F32R, tag="qk", bufs=2)
                nc.vector.scalar_tensor_tensor(qk[:], q_ps[:], bq2_t[:, 0:1], k_kT[:],
                                               OP.add, OP.mult)

                # ---- logits + softmax (pixel-major), exp via tanh ----
                lg_ps = psS.tile([4, 512], F32, tag="S", name="lg_s")
                nc.tensor.matmul(lg_ps[:], blockones[:], qk[:], start=True, stop=True)
                lg_sb = wpool.tile([4, 512], F32, tag="lg_sb", bufs=2)
                nc.vector.tensor_copy(lg_sb[:], lg_ps[:])
                misc_ps = psS.tile([128, 512], F32, tag="S", name="misc_s")
                lgpm_ps = misc_ps[:, 0:16]
                for s in range(K):
                    nc.tensor.transpose(lgpm_ps[:, s * 4:(s + 1) * 4],
                                        lg_sb[:, s * 128:(s + 1) * 128], ident[0:4, 0:4])
                pen = wpool.tile([128, 4], F32, tag="pen", bufs=3)  # -zx*invg2
                nc.vector.tensor_tensor(
                    pen[:],
                    smpm[:].rearrange("p (s c) -> p s c", c=3)[:, :, 2:3]
                    .rearrange("p s o -> p (s o)"),
                    m8[:, 0:4], OP.mult)
                lgpm = wpool.tile([128, 16], F32, tag="lgpm", bufs=3)
                nc.vector.scalar_tensor_tensor(
                    lgpm[:].rearrange("p (s h) -> p s h", s=4),
                    lgpm_ps[:].rearrange("p (s h) -> p s h", s=4), 0.0,
                    pen[:].to_broadcast([128, 4, 4]), OP.add, OP.add)
                # exp(x) = (1+t)/(1-t), t = tanh(x/2)  (keeps ACT in gelu set)
                # logits are bounded (~[-10, 1]); no max-subtraction needed
                th = wpool.tile([128, 16], F32, tag="th", bufs=3)
                nc.scalar.activation(th[:], lgpm[:], AF.Tanh, scale=0.5)
                num = wpool.tile([128, 16], F32, tag="num", bufs=3)
                nc.vector.tensor_scalar(num[:], th[:], 1.0, None, OP.add)
                den = wpool.tile([128, 16], F32, tag="den", bufs=3)
                nc.vector.tensor_scalar(den[:], th[:], -1.0, 1.0, OP.mult, OP.add)
                rcp = wpool.tile([128, 16], F32, tag="rcp", bufs=3)
                nc.vector.reciprocal(rcp[:], den[:])
                epm = wpool.tile([128, 16], F32, tag="epm", bufs=3)
                nc.vector.tensor_tensor(epm[:], num[:], rcp[:], OP.mult)
                zs = wpool.tile([128, 4], F32, tag="zs", bufs=3)
                nc.vector.tensor_reduce(
                    zs[:], epm[:].rearrange("p (s h) -> p h s", s=4),
                    mybir.AxisListType.X, OP.add)
                rz = wpool.tile([128, 4], F32, tag="rz", bufs=3)
                nc.vector.reciprocal(rz[:], zs[:])
                att_pm = wpool.tile([128, 16], F32, tag="att_pm", bufs=4)
                nc.vector.tensor_tensor(
                    att_pm[:].rearrange("p (h s) -> p s h", h=4),
                    epm[:].rearrange("p (s h) -> p s h", s=4),
                    rz[:].rearrange("p (h o) -> p o h", o=1).to_broadcast([128, 4, 4]),
                    OP.mult)
                att_ps = misc_ps[0:16, 64:192]
                nc.tensor.transpose(att_ps, att_pm[:], ident[:])
                att_sh = wpool.tile([16, 128], BF16, tag="att_sh", bufs=4)
                nc.vector.tensor_copy(att_sh[:], att_ps)
                att_dr = drpool.tile([16, 128], BF16, tag="att_dr")
                nc.sync.dma_start(att_dr[:], att_sh[:])
                # broadcast att rows to all 128 partitions: [128, (h,s,p) 2048]
                attB = wpool.tile([128, 2048], BF16, tag="attB", bufs=3)
                nc.sync.dma_start(
                    attB[:],
                    att_dr[:].rearrange("r n -> (r n)")
                    .rearrange("(o f) -> o f", o=1).to_broadcast([128, 2048]))

                return dict(attB=attB)

            # ============ phase B2: v-side MLP, attention apply, output ======
            def phase_b2(ci, j, a, b):
                n0 = ci * CHUNK
                SCt, sincc, c_kT = a["SCt"], a["sincc"], a["c_kT"]
                attB = b["attB"]
                h1v_ps = psA.tile([128, 512], F32, tag="A")
                nc.tensor.matmul(h1v_ps[:], Wv1_sin, SCt[64:128, 0:512], start=True, stop=False)
                nc.tensor.matmul(h1v_ps[:], Wv1_cos, SCt[64:128, 512:1024], start=False, stop=False)
                nc.tensor.matmul(h1v_ps[:], Wv1_cc[:], sincc[:], start=False, stop=True)
                h1v = wpool.tile([128, 512], F32R, tag="h1v", bufs=2)
                nc.scalar.activation(h1v[:], h1v_ps[:], AF.Gelu, bias=bv1_t[:, 0:1])
                vg_ps = psA.tile([128, 512], F32, tag="A")
                nc.tensor.matmul(vg_ps[:], Wv2_t[:, 0:H], h1v[:], start=True, stop=True)
                vb_ps = psA.tile([128, 512], F32, tag="A")
                nc.tensor.matmul(vb_ps[:], Wv2_t[:, H:2 * H], h1v[:], start=True, stop=True)
                utmp = wpool.tile([128, 512], F32, tag="utmp", bufs=2)
                nc.vector.scalar_tensor_tensor(utmp[:], vg_ps[:], bv2_t[:, 0:1],
                                               c_kT[:], OP.add, OP.mult)
                u_bf = wpool.tile([128, 512], BF16, tag="u_bf", bufs=3)
                nc.vector.scalar_tensor_tensor(u_bf[:], vb_ps[:], bv2_t[:, 1:2],
                                               utmp[:], OP.add, OP.add)

                # ---- apply attention + output MLP ----
                uw = wpool.tile([128, 2048], BF16, tag="uw", bufs=3)
                for h in range(NH):
                    nc.gpsimd.tensor_tensor(uw[:, h * 512:(h + 1) * 512], u_bf[:],
                                            attB[:, h * 512:(h + 1) * 512], OP.mult)
                y_ps = psA.tile([128, 512], F32, tag="A")
                for h in range(NH):
                    for s in range(K):
                        nc.tensor.matmul(
                            y_ps[:, h * 128:(h + 1) * 128],
                            Wv_bf[:, h * 128:(h + 1) * 128],
                            uw[:, h * 512 + s * 128:h * 512 + (s + 1) * 128],
                            start=(s == 0), stop=(s == 3))
                y_bf = wpool.tile([128, 512], BF16, tag="y_bf", bufs=3)
                nc.vector.tensor_copy(y_bf[:], y_ps[:])
                y1_ps = psA.tile([128, 512], F32, tag="A")
                for f2 in range(4):
                    for h in range(4):
                        nc.tensor.matmul(
                            y1_ps[:, f2 * 128:(f2 + 1) * 128],
                            Wo1_bf[:, h * 512 + f2 * 128:h * 512 + (f2 + 1) * 128],
                            y_bf[:, h * 128:(h + 1) * 128],
                            start=(h == 0), stop=(h == 3))
                y1 = wpool.tile([128, 512], BF16, tag="y1", bufs=3)
                for f2 in range(4):
                    nc.scalar.activation(y1[:, f2 * 128:(f2 + 1) * 128],
                                         y1_ps[:, f2 * 128:(f2 + 1) * 128],
                                         AF.Gelu, bias=bo1p[:, f2:f2 + 1])
                if j == GS - 1:
                    # refresh the ACT-ordering token after this group's gelus
                    nc.scalar.activation(tok[:], y1[:, 0:1], AF.Copy, scale=0.0)
                misc2_ps = psS.tile([128, 512], F32, tag="S", name="misc2_s")
                o_ps = misc2_ps[0:3, 0:128]
                for c2 in range(4):
                    nc.tensor.matmul(o_ps, Wo2_bf[:, c2 * 3:(c2 + 1) * 3],
                                     y1[:, c2 * 128:(c2 + 1) * 128],
                                     start=(c2 == 0), stop=(c2 == 3))
                o_sb = wpool.tile([3, 128], F32, tag="o_sb", bufs=3)
                nc.scalar.activation(o_sb[:], o_ps, AF.Identity, bias=bo2_t[:, 0:1])
                nc.sync.dma_start(outd[n0:n0 + 128, :].rearrange("n c -> c n"), o_sb[:])

            # ============ main loop: groups of GS chunks, A then B1/B2 =======
            # B1(j+1) is emitted before B2(j) so the next chunk's q-side MLP
            # fills the attention-broadcast DMA latency.
            for g in range(nchunk // GS):
                a1s = [phase_a1(g * GS, 0)]
                acc = []
                for j in range(GS):
                    if j + 1 < GS:
                        a1s.append(phase_a1(g * GS + j + 1, j + 1))
                    acc.append(phase_a2(g * GS + j, j, a1s[j]))
                bts = [phase_b1(g * GS, 0, acc[0]),
                       phase_b1(g * GS + 1, 1, acc[1])]
                for j in range(GS):
                    if j + 2 < GS:
                        bts.append(phase_b1(g * GS + j + 2, j + 2, acc[j + 2]))
                    phase_b2(g * GS + j, j, acc[j], bts[j])

    nc.compile()
    return nc


def make_in_maps(inputs):
    x = np.asarray(inputs["x"], np.float32)
    f = {k: np.asarray(v, np.float32) for k, v in inputs.items()}

    # ---- host-side precompute of weight/latent-derived constants ----
    wcom = {k: np.ascontiguousarray(f[k]) for k in
            ["Wq1", "bq1", "Wq2", "bq2", "Wv1", "bv1", "Wv2", "bv2",
             "Wv", "Wo1", "Wo2", "bo2"]}
    wcom["Wsin"] = np.ascontiguousarray(
        -0.5 * np.concatenate([f["Wq_sin"], f["Wv_sin"]], axis=1))
    bias_e = np.concatenate([0.5 * f["Wq_sin"].sum(0), 0.5 * f["Wv_sin"].sum(0)])
    wcom["bias_e"] = np.ascontiguousarray(bias_e.reshape(H, 1))
    bo1p = f["bo1"] + f["Wo1"].T @ f["bv"]
    wcom["bo1p"] = np.ascontiguousarray(bo1p.reshape(4, 128).T)

    in_maps = []
    for core in range(NCORE):
        b = core // (NCORE // B)
        sh = (core % (NCORE // B))
        m = dict(wcom)
        m["x"] = np.ascontiguousarray(x[b, sh * NPC:(sh + 1) * NPC])
        p, c, g = f["p"][b], f["c"][b], f["g"][b]
        cstem = c @ f["W_stem"] + f["b_stem"]          # [L, H]
        m["c_tab"] = np.ascontiguousarray(cstem)
        m["k_tab"] = np.ascontiguousarray(cstem @ f["Wk"] + f["bk"])
        sm = np.concatenate([p, 1.0 / (g * g)], axis=1)  # [L, 3]
        m["sm_tab"] = np.ascontiguousarray(sm)
        pB = np.concatenate([p[:, 0], p[:, 1]])          # [2L]
        m["pB"] = np.ascontiguousarray(np.broadcast_to(pB, (128, 2 * L)))
        in_maps.append(m)
    return in_maps


def kernel(**inputs):
    import jax
    try:
        jax.config.update('jax_platforms', 'axon,cpu')
    except Exception:
        pass
    from concourse.bass_utils import run_bass_kernel_spmd

    nchunk = NPC // CHUNK
    if nchunk not in _cache:
        _cache[nchunk] = _build(nchunk)
    nc = _cache[nchunk]

    in_maps = make_in_maps(inputs)
    res = run_bass_kernel_spmd(nc, in_maps, core_ids=list(range(NCORE)))
    out = np.zeros((B, N, DOUT), np.float32)
    for core in range(NCORE):
        b = core // (NCORE // B)
        sh = core % (NCORE // B)
        out[b, sh * NPC:(sh + 1) * NPC] = res.results[core]["out"]
    return out


# revision 59
# speedup vs baseline: 2.1974x; 1.0183x over previous
"""Trainium2 Bass kernel for nn_EquivariantNeuralField.

Per-pixel top-4-nearest-latent cross-attention neural field.
Sharding: 8 cores; core i handles batch i//4, pixel rows (i%4)*4096..+4096.

v2: phase-split pipeline (A=trig table, B=gelu table) over 4-chunk groups
to kill activation-table thrashing; f32r single-pass gathers; latent-major
one-hot build (no big transposes); exp via tanh; DMA-broadcast attention.
"""
import numpy as np

B, N, L, K = 2, 16384, 256, 4
DIN, DOUT, DLAT, H, A, NH = 2, 3, 64, 128, 32, 4
NCORE = 8
NPC = N * B // NCORE          # pixels per core = 4096
CHUNK = 128
GS = 8                        # chunks per phase group
PI = float(np.pi)

_cache = {}


def _build(nchunk):
    import concourse.bacc as bacc
    import concourse.mybir as mybir
    from concourse.tile import TileContext

    F32 = mybir.dt.float32
    F32R = mybir.dt.float32r
    BF16 = mybir.dt.bfloat16
    I32 = mybir.dt.int32
    U32 = mybir.dt.uint32
    AF = mybir.ActivationFunctionType
    OP = mybir.AluOpType

    nc = bacc.Bacc()

    # ---------------- DRAM tensors ----------------
    # Tables and weight-derived constants are precomputed host-side in kernel().
    xd = nc.dram_tensor("x", [NPC, DIN], F32, kind="ExternalInput")
    pBd = nc.dram_tensor("pB", [128, 2 * L], F32, kind="ExternalInput")
    ktabd = nc.dram_tensor("k_tab", [L, NH * A], F32R, kind="ExternalInput")
    ctabd = nc.dram_tensor("c_tab", [L, H], F32R, kind="ExternalInput")
    smtabd = nc.dram_tensor("sm_tab", [L, 3], F32R, kind="ExternalInput")
    Wsin_d = nc.dram_tensor("Wsin", [DIN, H], F32R, kind="ExternalInput")
    biase_d = nc.dram_tensor("bias_e", [H, 1], F32, kind="ExternalInput")
    Wq1 = nc.dram_tensor("Wq1", [H + DIN, H], F32, kind="ExternalInput")
    bq1 = nc.dram_tensor("bq1", [H], F32, kind="ExternalInput")
    Wq2 = nc.dram_tensor("Wq2", [H, NH * A], F32R, kind="ExternalInput")
    bq2 = nc.dram_tensor("bq2", [NH * A], F32, kind="ExternalInput")
    Wv1 = nc.dram_tensor("Wv1", [H + DIN, H], F32, kind="ExternalInput")
    bv1 = nc.dram_tensor("bv1", [H], F32, kind="ExternalInput")
    Wv2 = nc.dram_tensor("Wv2", [H, 2 * H], F32R, kind="ExternalInput")
    bv2 = nc.dram_tensor("bv2", [2 * H], F32, kind="ExternalInput")
    Wv = nc.dram_tensor("Wv", [H, NH * H], F32, kind="ExternalInput")
    Wo1 = nc.dram_tensor("Wo1", [NH * H, NH * H], F32, kind="ExternalInput")
    bo1p_d = nc.dram_tensor("bo1p", [128, 4], F32, kind="ExternalInput")
    Wo2 = nc.dram_tensor("Wo2", [NH * H, DOUT], F32, kind="ExternalInput")
    bo2 = nc.dram_tensor("bo2", [DOUT], F32, kind="ExternalInput")
    outd = nc.dram_tensor("out", [NPC, DOUT], F32, kind="ExternalOutput")

    with TileContext(nc) as tc:
        with tc.tile_pool(name="const", bufs=1) as cpool, \
             tc.tile_pool(name="work", bufs=2) as wpool, \
             tc.tile_pool(name="psA", bufs=5, space="PSUM") as psA, \
             tc.tile_pool(name="psS", bufs=3, space="PSUM") as psS, \
             tc.tile_pool(name="drp", bufs=4, space="DRAM") as drpool:

            # ============ one-time constants ============
            idn_i = cpool.tile([128, 128], I32)
            nc.gpsimd.iota(idn_i[:], [[1, 128]], base=0, channel_multiplier=-1)
            idn_f0 = cpool.tile([128, 128], F32)
            nc.vector.tensor_copy(idn_f0[:], idn_i[:])
            ident = cpool.tile([128, 128], F32)
            nc.vector.tensor_scalar(ident[:], idn_f0[:], 0.0, None, OP.is_equal)
            ident_bf = cpool.tile([128, 128], BF16)
            nc.vector.tensor_copy(ident_bf[:], ident[:])
            # per-partition iota columns (f32): values p and p+128
            iop_i = cpool.tile([128, 1], I32)
            nc.gpsimd.iota(iop_i[:], [[1, 1]], base=0, channel_multiplier=1)
            iota0 = cpool.tile([128, 1], F32)
            nc.vector.tensor_copy(iota0[:], iop_i[:])
            iota1 = cpool.tile([128, 1], F32)
            nc.vector.tensor_scalar(iota1[:], iota0[:], 128.0, None, OP.add)
            # blockones [128, NH] f32r : bo[c, h] = (c//A == h)
            blockones_f = cpool.tile([128, NH], F32)
            nc.gpsimd.memset(blockones_f[:], 0.0)
            for h in range(NH):
                nc.gpsimd.memset(blockones_f[h * A:(h + 1) * A, h:h + 1], 1.0)
            blockones = cpool.tile([128, NH], F32R)
            nc.vector.tensor_copy(blockones[:], blockones_f[:])
            # ACT-ordering token: sin ops of group g+1 wait on gelu ops of g
            tok = cpool.tile([128, 1], F32)
            nc.gpsimd.memset(tok[:], 0.0)

            # ============ weights (host-precomputed, straight DMA loads) ===
            def load_cast(dram_ap, shape, dt, tag):
                if dt in (F32, F32R):
                    t0 = cpool.tile(shape, dt, tag=tag + "_d", name=tag)
                    nc.sync.dma_start(t0[:], dram_ap)
                    return t0
                t0 = wpool.tile([128, 512], F32, tag="stage", name="stage_" + tag)
                nc.sync.dma_start(t0[0:shape[0], 0:shape[1]], dram_ap)
                t1 = cpool.tile(shape, dt, tag=tag)
                nc.vector.tensor_copy(t1[:], t0[0:shape[0], 0:shape[1]])
                return t1

            def load_bias(dram, n, tag):
                if n <= 128:
                    t = cpool.tile([n, 1], F32, tag=tag)
                    nc.sync.dma_start(t[:], dram[:].rearrange("(n o) -> n o", o=1))
                    return t
                k = n // 128
                t = cpool.tile([128, k], F32, tag=tag)
                nc.sync.dma_start(t[:], dram[:].rearrange("(j p) -> p j", p=128))
                return t

            Wsin_t = load_cast(Wsin_d[:], [DIN, H], F32R, "wsin")
            bias_e = cpool.tile([H, 1], F32, tag="bias_e")
            nc.sync.dma_start(bias_e[:], biase_d[:])
            Wq1_cc = load_cast(Wq1[0:DIN, :], [DIN, H], BF16, "wq1cc")
            Wq1_sin = load_cast(Wq1[DIN:DIN + 64, :], [64, H], BF16, "wq1sin")
            Wq1_cos = load_cast(Wq1[DIN + 64:DIN + 128, :], [64, H], BF16, "wq1cos")
            Wv1_cc = load_cast(Wv1[0:DIN, :], [DIN, H], BF16, "wv1cc")
            Wv1_sf = cpool.tile([128, H], F32, tag="wv1sf")
            nc.sync.dma_start(Wv1_sf[64:128, :], Wv1[DIN:DIN + 64, :])
            Wv1_sin_t = cpool.tile([128, H], BF16, tag="wv1sin")
            nc.vector.tensor_copy(Wv1_sin_t[64:128, :], Wv1_sf[64:128, :])
            Wv1_cf = cpool.tile([128, H], F32, tag="wv1cf")
            nc.sync.dma_start(Wv1_cf[64:128, :], Wv1[DIN + 64:DIN + 128, :])
            Wv1_cos_t = cpool.tile([128, H], BF16, tag="wv1cos")
            nc.vector.tensor_copy(Wv1_cos_t[64:128, :], Wv1_cf[64:128, :])
            Wv1_sin = Wv1_sin_t[64:128, :]
            Wv1_cos = Wv1_cos_t[64:128, :]
            Wq2_t = load_cast(Wq2[:], [H, NH * A], F32R, "wq2")
            Wv2_t = load_cast(Wv2[:], [H, 2 * H], F32R, "wv2")
            Wv_bf = load_cast(Wv[:], [H, NH * H], BF16, "wv")
            # Wo1 as [128, (c2, f) 2048] bf16 (staged through rotating buffer)
            Wo1_bf = cpool.tile([128, 4 * 512], BF16, tag="wo1")
            for c2 in range(4):
                st = wpool.tile([128, 512], F32, tag="stage", name=f"wo1st{c2}")
                nc.sync.dma_start(st[:], Wo1[c2 * 128:(c2 + 1) * 128, :])
                nc.vector.tensor_copy(Wo1_bf[:, c2 * 512:(c2 + 1) * 512], st[:])
            Wo2_f32 = cpool.tile([128, 4 * DOUT], F32, tag="wo2f")
            for c2 in range(4):
                nc.sync.dma_start(Wo2_f32[:, c2 * DOUT:(c2 + 1) * DOUT],
                                  Wo2[c2 * 128:(c2 + 1) * 128, :])
            Wo2_bf = cpool.tile([128, 4 * DOUT], BF16, tag="wo2")
            nc.vector.tensor_copy(Wo2_bf[:], Wo2_f32[:])

            bq1_t = load_bias(bq1, H, "bq1")
            bq2_t = load_bias(bq2, NH * A, "bq2")
            bv1_t = load_bias(bv1, H, "bv1")
            bv2_t = load_bias(bv2, 2 * H, "bv2")
            bo2_t = load_bias(bo2, DOUT, "bo2")
            bo1p = cpool.tile([128, 4], F32, tag="bo1p")
            nc.sync.dma_start(bo1p[:], bo1p_d[:])

            # ============ latent tables (host-precomputed) ============
            k_tab, c_tab, s_tab = [], [], []
            for lc in range(2):
                kl = cpool.tile([128, NH * A], F32R, tag=f"kl{lc}")
                nc.sync.dma_start(kl[:], ktabd[lc * 128:(lc + 1) * 128, :])
                k_tab.append(kl)
                cn = cpool.tile([128, 128], F32R, tag=f"cn{lc}")
                nc.sync.dma_start(cn[:], ctabd[lc * 128:(lc + 1) * 128, :])
                c_tab.append(cn)
                smr = cpool.tile([128, 3], F32R, tag=f"smr{lc}")
                nc.sync.dma_start(smr[:], smtabd[lc * 128:(lc + 1) * 128, :])
                s_tab.append(smr)
            pB = cpool.tile([128, 2 * L], F32, tag="pB")
            nc.sync.dma_start(pB[:], pBd[:])

            # ===== phase A1: distances, top-4, idx round trip =====
            def phase_a1(ci, j):
                n0 = ci * CHUNK
                x0 = wpool.tile([128, 2], F32, tag=f"x0_{j}", bufs=1)
                nc.sync.dma_start(x0[:], xd[n0:n0 + 128, :])
                x2T = wpool.tile([2, 128], F32, tag=f"x2T_{j}", bufs=1)
                nc.sync.dma_start(x2T[:], xd[n0:n0 + 128, :].rearrange("n c -> c n"))
                d0 = wpool.tile([128, 256], F32, tag="d0")
                nc.gpsimd.tensor_scalar(d0[:], pB[:, 0:L], x0[:, 0:1], None, OP.subtract)
                d1 = wpool.tile([128, 256], F32, tag="d1")
                nc.gpsimd.tensor_scalar(d1[:], pB[:, L:2 * L], x0[:, 1:2], None, OP.subtract)
                sq0 = wpool.tile([128, 256], F32, tag="sq0")
                nc.gpsimd.tensor_tensor(sq0[:], d0[:], d0[:], OP.mult)
                sq1 = wpool.tile([128, 256], F32, tag="sq1")
                nc.gpsimd.tensor_tensor(sq1[:], d1[:], d1[:], OP.mult)
                nzx = wpool.tile([128, 256], F32, tag="nzx")
                nc.vector.scalar_tensor_tensor(nzx[:], sq0[:], -1.0, sq1[:],
                                               OP.mult, OP.subtract)
                m8 = wpool.tile([128, 8], F32, tag=f"m8_{j}", bufs=1)
                nc.vector.max(m8[:], nzx[:])
                i8 = wpool.tile([128, 8], U32, tag="i8", bufs=3)
                nc.vector.max_index(i8[:], m8[:], nzx[:])
                idxb = wpool.tile([128, 4], BF16, tag="idxb", bufs=3)
                nc.vector.tensor_copy(idxb[:], i8[:, 0:4])

                # --- one-hot, latent-major: ohT[l, s*128+p] = (idx[p,s] == l)
                # idx -> DRAM (s-major) -> broadcast-read to all 128 partitions
                idx_dr = drpool.tile([4, 128], BF16, tag="idx_dr")
                nc.sync.dma_start(idx_dr[:].rearrange("s p -> p s"), idxb[:])
                idxB = wpool.tile([128, 512], BF16, tag="idxB", bufs=3)
                nc.sync.dma_start(
                    idxB[:],
                    idx_dr[:].rearrange("r n -> (r n)")
                    .rearrange("(o f) -> o f", o=1).to_broadcast([128, 512]))
                return dict(idxB=idxB, m8=m8, x0=x0, x2T=x2T)

            # ===== phase A2: gathers + sin features =====
            def phase_a2(ci, j, a1):
                idxB, m8, x2T = a1["idxB"], a1["m8"], a1["x2T"]
                ohT = [wpool.tile([128, 512], F32R, tag=f"ohT{lc}",
                                  name=f"ohT{lc}") for lc in range(2)]
                nc.gpsimd.tensor_scalar(ohT[0][:], idxB[:], iota0[:], None, OP.is_equal)
                nc.gpsimd.tensor_scalar(ohT[1][:], idxB[:], iota1[:], None, OP.is_equal)

                # --- gathers (single-pass f32r) ---
                ck_ps = psA.tile([128, 512], F32, tag="A")
                kk_ps = psA.tile([128, 512], F32, tag="A")
                smlg_ps = psS.tile([36, 512], F32, tag="S", name="smlg_s")
                sm_ps = smlg_ps[0:3, :]
                for lc in range(2):
                    nc.tensor.matmul(ck_ps[:], c_tab[lc][:], ohT[lc][:],
                                     start=(lc == 0), stop=(lc == 1))
                for lc in range(2):
                    nc.tensor.matmul(kk_ps[:], k_tab[lc][:], ohT[lc][:],
                                     start=(lc == 0), stop=(lc == 1))
                for lc in range(2):
                    nc.tensor.matmul(sm_ps, s_tab[lc][:], ohT[lc][:],
                                     start=(lc == 0), stop=(lc == 1))
                c_kT = wpool.tile([128, 512], BF16, tag=f"c_kT_{j}", bufs=1)
                nc.vector.tensor_copy(c_kT[:], ck_ps[:])
                k_kT = wpool.tile([128, 512], F32, tag=f"k_kT_{j}", bufs=1)
                nc.scalar.copy(k_kT[:], kk_ps[:])
                smT = wpool.tile([3, 512], F32, tag="smT", bufs=2)
                nc.scalar.copy(smT[:], sm_ps[:])
                # invg2 pixel-major [128, 12] for the softmax penalty
                smpm_ps = psS.tile([128, 16], F32, tag="S", name="smpm_s")[:, 0:12]
                for s in range(K):
                    nc.tensor.transpose(smpm_ps[:, s * 3:(s + 1) * 3],
                                        smT[:, s * 128:(s + 1) * 128], ident[0:3, 0:3])
                smpm = wpool.tile([128, 12], F32, tag=f"smpm_{j}", bufs=1)
                nc.vector.tensor_copy(smpm[:], smpm_ps[:])

                # --- sin features ---
                relp = wpool.tile([2, 512], F32R, tag="relp", bufs=2)
                nc.vector.tensor_tensor(
                    relp[:].rearrange("c (s n) -> c s n", s=4), sm_ps[0:2, :]
                    .rearrange("c (s n) -> c s n", s=4),
                    x2T[:].rearrange("c (s n) -> c s n", s=1).to_broadcast([2, 4, 128]),
                    OP.subtract)
                # cc = pi*(x - p + 1) = -pi*relp + pi ; f_cc = cc/(2pi) wrapped
                tcc = wpool.tile([2, 512], F32, tag="tcc", bufs=2)
                nc.vector.tensor_scalar(tcc[:], relp[:], -0.5, 0.5, OP.mult, OP.add)
                icc = wpool.tile([2, 512], I32, tag="icc", bufs=2)
                nc.gpsimd.tensor_copy(icc[:], tcc[:])
                fcc32 = wpool.tile([2, 512], F32, tag="fcc32", bufs=2)
                nc.gpsimd.tensor_copy(fcc32[:], icc[:])
                fcc = wpool.tile([2, 512], F32, tag="fcc", bufs=2)
                nc.vector.tensor_tensor(fcc[:], tcc[:], fcc32[:], OP.subtract)
                sincc = wpool.tile([2, 512], BF16, tag=f"sincc_{j}", bufs=1)
                nc.scalar.activation(sincc[:], fcc[:], AF.Sin, scale=float(2 * PI),
                                     bias=tok[0:2, 0:1])

                # te = e/(2pi) computed directly from relp via pre-folded weights
                # (Wsin pre-scaled by -0.5 on host; constant term added as ACT bias)
                e_ps = psA.tile([128, 512], F32, tag="A")
                nc.tensor.matmul(e_ps[:], Wsin_t[:], relp[:], start=True, stop=True)
                te = wpool.tile([128, 512], F32, tag="te", bufs=2)
                nc.scalar.activation(te[:], e_ps[:], AF.Identity, bias=bias_e[:, 0:1])
                ie = wpool.tile([128, 512], I32, tag="ie", bufs=2)
                nc.gpsimd.tensor_copy(ie[:], te[:])
                fe32 = wpool.tile([128, 512], F32, tag="fe32", bufs=2)
                nc.gpsimd.tensor_copy(fe32[:], ie[:])
                # fboth = [fe | 0.25-|fe|]; one Sin gives [sin(e) | cos(e)]
                fboth = wpool.tile([128, 1024], F32, tag="fboth", bufs=2)
                nc.vector.tensor_tensor(fboth[:, 0:512], te[:], fe32[:], OP.subtract)
                fab = wpool.tile([128, 512], F32, tag="fab", bufs=2)
                nc.vector.scalar_tensor_tensor(fab[:], fboth[:, 0:512], -1.0,
                                               fboth[:, 0:512], OP.mult, OP.max)
                nc.gpsimd.tensor_scalar(fboth[:, 512:1024], fab[:], -1.0, 0.25,
                                        OP.mult, OP.add)
                SCt = wpool.tile([128, 1024], BF16, tag=f"SCt_{j}", bufs=1)
                nc.scalar.activation(SCt[:], fboth[:], AF.Sin, scale=float(2 * PI),
                                     bias=tok[:, 0:1])
                return dict(SCt=SCt, sincc=sincc, c_kT=c_kT, k_kT=k_kT,
                            smpm=smpm, m8=m8)

            # ============ phase B1: q-side MLP, softmax, att DMAs ============
            def phase_b1(ci, j, a):
                SCt, sincc = a["SCt"], a["sincc"]
                k_kT, smpm, m8 = a["k_kT"], a["smpm"], a["m8"]

                h1q_ps = psA.tile([128, 512], F32, tag="A")
                nc.tensor.matmul(h1q_ps[:], Wq1_sin[:], SCt[0:64, 0:512], start=True, stop=False)
                nc.tensor.matmul(h1q_ps[:], Wq1_cos[:], SCt[0:64, 512:1024], start=False, stop=False)
                nc.tensor.matmul(h1q_ps[:], Wq1_cc[:], sincc[:], start=False, stop=True)
                h1q = wpool.tile([128, 512], F32R, tag="h1q", bufs=2)
                nc.scalar.activation(h1q[:], h1q_ps[:], AF.Gelu, bias=bq1_t[:, 0:1])
                q_ps = psA.tile([128, 512], F32, tag="A")
                nc.tensor.matmul(q_ps[:], Wq2_t[:], h1q[:], start=True, stop=True)

                qk = wpool.tile([128, 512], F32R, tag="qk", bufs=2)
                nc.vector.scalar_tensor_tensor(qk[:], q_ps[:], bq2_t[:, 0:1], k_kT[:],
                                               OP.add, OP.mult)

                # ---- logits + softmax (pixel-major), exp via tanh ----
                lg_ps = psS.tile([4, 512], F32, tag="S", name="lg_s")
                nc.tensor.matmul(lg_ps[:], blockones[:], qk[:], start=True, stop=True)
                lg_sb = wpool.tile([4, 512], F32, tag="lg_sb", bufs=2)
                nc.vector.tensor_copy(lg_sb[:], lg_ps[:])
                misc_ps = psS.tile([128, 512], F32, tag="S", name="misc_s")
                lgpm_ps = misc_ps[:, 0:16]
                for s in range(K):
                    nc.tensor.transpose(lgpm_ps[:, s * 4:(s + 1) * 4],
                                        lg_sb[:, s * 128:(s + 1) * 128], ident[0:4, 0:4])
                pen = wpool.tile([128, 4], F32, tag="pen", bufs=3)  # -zx*invg2
                nc.vector.tensor_tensor(
                    pen[:],
                    smpm[:].rearrange("p (s c) -> p s c", c=3)[:, :, 2:3]
                    .rearrange("p s o -> p (s o)"),
                    m8[:, 0:4], OP.mult)
                lgpm = wpool.tile([128, 16], F32, tag="lgpm", bufs=3)
                nc.vector.scalar_tensor_tensor(
                    lgpm[:].rearrange("p (s h) -> p s h", s=4),
                    lgpm_ps[:].rearrange("p (s h) -> p s h", s=4), 0.0,
                    pen[:].to_broadcast([128, 4, 4]), OP.add, OP.add)
                # exp(x) = (1+t)/(1-t), t = tanh(x/2)  (keeps ACT in gelu set)
                # logits are bounded (~[-10, 1]); no max-subtraction needed
                th = wpool.tile([128, 16], F32, tag="th", bufs=3)
                nc.scalar.activation(th[:], lgpm[:], AF.Tanh, scale=0.5)
                num = wpool.tile([128, 16], F32, tag="num", bufs=3)
                nc.vector.tensor_scalar(num[:], th[:], 1.0, None, OP.add)
                den = wpool.tile([128, 16], F32, tag="den", bufs=3)
                nc.vector.tensor_scalar(den[:], th[:], -1.0, 1.0, OP.mult, OP.add)
                rcp = wpool.tile([128, 16], F32, tag="rcp", bufs=3)
                nc.vector.reciprocal(rcp[:], den[:])
                epm = wpool.tile([128, 16], F32, tag="epm", bufs=3)
                nc.vector.tensor_tensor(epm[:], num[:], rcp[:], OP.mult)
                zs = wpool.tile([128, 4], F32, tag="zs", bufs=3)
                nc.vector.tensor_reduce(
                    zs[:], epm[:].rearrange("p (s h) -> p h s", s=4),
                    mybir.AxisListType.X, OP.add)
                rz = wpool.tile([128, 4], F32, tag="rz", bufs=3)
                nc.vector.reciprocal(rz[:], zs[:])
                att_pm = wpool.tile([128, 16], F32, tag="att_pm", bufs=4)
                nc.vector.tensor_tensor(
                    att_pm[:].rearrange("p (h s) -> p s h", h=4),
                    epm[:].rearrange("p (s h) -> p s h", s=4),
                    rz[:].rearrange("p (h o) -> p o h", o=1).to_broadcast([128, 4, 4]),
                    OP.mult)
                att_ps = misc_ps[0:16, 64:192]
                nc.tensor.transpose(att_ps, att_pm[:], ident[:])
                att_sh = wpool.tile([16, 128], BF16, tag="att_sh", bufs=4)
                nc.vector.tensor_copy(att_sh[:], att_ps)
                att_dr = drpool.tile([16, 128], BF16, tag="att_dr")
                nc.sync.dma_start(att_dr[:], att_sh[:])
                # broadcast att rows to all 128 partitions: [128, (h,s,p) 2048]
                attB = wpool.tile([128, 2048], BF16, tag="attB", bufs=3)
                nc.sync.dma_start(
                    attB[:],
                    att_dr[:].rearrange("r n -> (r n)")
                    .rearrange("(o f) -> o f", o=1).to_broadcast([128, 2048]))

                return dict(attB=attB)

            # ============ phase B2: v-side MLP, attention apply, output ======
            def phase_b2(ci, j, a, b):
                n0 = ci * CHUNK
                SCt, sincc, c_kT = a["SCt"], a["sincc"], a["c_kT"]
                attB = b["attB"]
                h1v_ps = psA.tile([128, 512], F32, tag="A")
                nc.tensor.matmul(h1v_ps[:], Wv1_sin, SCt[64:128, 0:512], start=True, stop=False)
                nc.tensor.matmul(h1v_ps[:], Wv1_cos, SCt[64:128, 512:1024], start=False, stop=False)
                nc.tensor.matmul(h1v_ps[:], Wv1_cc[:], sincc[:], start=False, stop=True)
                h1v = wpool.tile([128, 512], F32R, tag="h1v", bufs=2)
                nc.scalar.activation(h1v[:], h1v_ps[:], AF.Gelu, bias=bv1_t[:, 0:1])
                vg_ps = psA.tile([128, 512], F32, tag="A")
                nc.tensor.matmul(vg_ps[:], Wv2_t[:, 0:H], h1v[:], start=True, stop=True)
                vb_ps = psA.tile([128, 512], F32, tag="A")
                nc.tensor.matmul(vb_ps[:], Wv2_t[:, H:2 * H], h1v[:], start=True, stop=True)
                utmp = wpool.tile([128, 512], F32, tag="utmp", bufs=2)
                nc.vector.scalar_tensor_tensor(utmp[:], vg_ps[:], bv2_t[:, 0:1],
                                               c_kT[:], OP.add, OP.mult)
                u_bf = wpool.tile([128, 512], BF16, tag="u_bf", bufs=3)
                nc.vector.scalar_tensor_tensor(u_bf[:], vb_ps[:], bv2_t[:, 1:2],
                                               utmp[:], OP.add, OP.add)

                # ---- apply attention + output MLP ----
                uw = wpool.tile([128, 2048], BF16, tag="uw", bufs=3)
                for h in range(NH):
                    eng = nc.gpsimd if h < 2 else nc.vector
                    eng.tensor_tensor(uw[:, h * 512:(h + 1) * 512], u_bf[:],
                                      attB[:, h * 512:(h + 1) * 512], OP.mult)
                y_ps = psA.tile([128, 512], F32, tag="A")
                for h in range(NH):
                    for s in range(K):
                        nc.tensor.matmul(
                            y_ps[:, h * 128:(h + 1) * 128],
                            Wv_bf[:, h * 128:(h + 1) * 128],
                            uw[:, h * 512 + s * 128:h * 512 + (s + 1) * 128],
                            start=(s == 0), stop=(s == 3))
                y_bf = wpool.tile([128, 512], BF16, tag="y_bf", bufs=3)
                nc.vector.tensor_copy(y_bf[:], y_ps[:])
                y1_ps = psA.tile([128, 512], F32, tag="A")
                for f2 in range(4):
                    for h in range(4):
                        nc.tensor.matmul(
                            y1_ps[:, f2 * 128:(f2 + 1) * 128],
                            Wo1_bf[:, h * 512 + f2 * 128:h * 512 + (f2 + 1) * 128],
                            y_bf[:, h * 128:(h + 1) * 128],
                            start=(h == 0), stop=(h == 3))
                y1 = wpool.tile([128, 512], BF16, tag="y1", bufs=3)
                for f2 in range(4):
                    nc.scalar.activation(y1[:, f2 * 128:(f2 + 1) * 128],
                                         y1_ps[:, f2 * 128:(f2 + 1) * 128],
                                         AF.Gelu, bias=bo1p[:, f2:f2 + 1])
                if j == GS - 1:
                    # refresh the ACT-ordering token after this group's gelus
                    nc.scalar.activation(tok[:], y1[:, 0:1], AF.Copy, scale=0.0)
                misc2_ps = psS.tile([128, 512], F32, tag="S", name="misc2_s")
                o_ps = misc2_ps[0:3, 0:128]
                for c2 in range(4):
                    nc.tensor.matmul(o_ps, Wo2_bf[:, c2 * 3:(c2 + 1) * 3],
                                     y1[:, c2 * 128:(c2 + 1) * 128],
                                     start=(c2 == 0), stop=(c2 == 3))
                o_sb = wpool.tile([3, 128], F32, tag="o_sb", bufs=3)
                nc.scalar.activation(o_sb[:], o_ps, AF.Identity, bias=bo2_t[:, 0:1])
                nc.sync.dma_start(outd[n0:n0 + 128, :].rearrange("n c -> c n"), o_sb[:])

            # ============ main loop: groups of GS chunks, A then B1/B2 =======
            # B1(j+1) is emitted before B2(j) so the next chunk's q-side MLP
            # fills the attention-broadcast DMA latency.
            for g in range(nchunk // GS):
                a1s = [phase_a1(g * GS, 0)]
                acc = []
                for j in range(GS):
                    if j + 1 < GS:
                        a1s.append(phase_a1(g * GS + j + 1, j + 1))
                    acc.append(phase_a2(g * GS + j, j, a1s[j]))
                bts = [phase_b1(g * GS, 0, acc[0]),
                       phase_b1(g * GS + 1, 1, acc[1])]
                for j in range(GS):
                    if j + 2 < GS:
                        bts.append(phase_b1(g * GS + j + 2, j + 2, acc[j + 2]))
                    phase_b2(g * GS + j, j, acc[j], bts[j])

    nc.compile()
    return nc


def make_in_maps(inputs):
    x = np.asarray(inputs["x"], np.float32)
    f = {k: np.asarray(v, np.float32) for k, v in inputs.items()}

    # ---- host-side precompute of weight/latent-derived constants ----
    wcom = {k: np.ascontiguousarray(f[k]) for k in
            ["Wq1", "bq1", "Wq2", "bq2", "Wv1", "bv1", "Wv2", "bv2",
             "Wv", "Wo1", "Wo2", "bo2"]}
    wcom["Wsin"] = np.ascontiguousarray(
        -0.5 * np.concatenate([f["Wq_sin"], f["Wv_sin"]], axis=1))
    bias_e = np.concatenate([0.5 * f["Wq_sin"].sum(0), 0.5 * f["Wv_sin"].sum(0)])
    wcom["bias_e"] = np.ascontiguousarray(bias_e.reshape(H, 1))
    bo1p = f["bo1"] + f["Wo1"].T @ f["bv"]
    wcom["bo1p"] = np.ascontiguousarray(bo1p.reshape(4, 128).T)

    in_maps = []
    for core in range(NCORE):
        b = core // (NCORE // B)
        sh = (core % (NCORE // B))
        m = dict(wcom)
        m["x"] = np.ascontiguousarray(x[b, sh * NPC:(sh + 1) * NPC])
        p, c, g = f["p"][b], f["c"][b], f["g"][b]
        cstem = c @ f["W_stem"] + f["b_stem"]          # [L, H]
        m["c_tab"] = np.ascontiguousarray(cstem)
        m["k_tab"] = np.ascontiguousarray(cstem @ f["Wk"] + f["bk"])
        sm = np.concatenate([p, 1.0 / (g * g)], axis=1)  # [L, 3]
        m["sm_tab"] = np.ascontiguousarray(sm)
        pB = np.concatenate([p[:, 0], p[:, 1]])          # [2L]
        m["pB"] = np.ascontiguousarray(np.broadcast_to(pB, (128, 2 * L)))
        in_maps.append(m)
    return in_maps


def kernel(**inputs):
    import jax
    try:
        jax.config.update('jax_platforms', 'axon,cpu')
    except Exception:
        pass
    from concourse.bass_utils import run_bass_kernel_spmd

    nchunk = NPC // CHUNK
    if nchunk not in _cache:
        _cache[nchunk] = _build(nchunk)
    nc = _cache[nchunk]

    in_maps = make_in_maps(inputs)
    res = run_bass_kernel_spmd(nc, in_maps, core_ids=list(range(NCORE)))
    out = np.zeros((B, N, DOUT), np.float32)
    for core in range(NCORE):
        b = core // (NCORE // B)
        sh = core % (NCORE // B)
        out[b, sh * NPC:(sh + 1) * NPC] = res.results[core]["out"]
    return out


# revision 63
# speedup vs baseline: 2.2480x; 1.0230x over previous
"""Trainium2 Bass kernel for nn_EquivariantNeuralField.

Per-pixel top-4-nearest-latent cross-attention neural field.
Sharding: 8 cores; core i handles batch i//4, pixel rows (i%4)*4096..+4096.

v2: phase-split pipeline (A=trig table, B=gelu table) over 4-chunk groups
to kill activation-table thrashing; f32r single-pass gathers; latent-major
one-hot build (no big transposes); exp via tanh; DMA-broadcast attention.
"""
import numpy as np

B, N, L, K = 2, 16384, 256, 4
DIN, DOUT, DLAT, H, A, NH = 2, 3, 64, 128, 32, 4
NCORE = 8
NPC = N * B // NCORE          # pixels per core = 4096
CHUNK = 128
GS = 8                        # chunks per phase group
PI = float(np.pi)

_cache = {}


def _build(nchunk):
    import concourse.bacc as bacc
    import concourse.mybir as mybir
    from concourse.tile import TileContext

    F32 = mybir.dt.float32
    F32R = mybir.dt.float32r
    BF16 = mybir.dt.bfloat16
    I32 = mybir.dt.int32
    U32 = mybir.dt.uint32
    AF = mybir.ActivationFunctionType
    OP = mybir.AluOpType

    nc = bacc.Bacc()

    # ---------------- DRAM tensors ----------------
    # Tables and weight-derived constants are precomputed host-side in kernel().
    xd = nc.dram_tensor("x", [NPC, DIN], F32, kind="ExternalInput")
    pBd = nc.dram_tensor("pB", [128, 2 * L], F32, kind="ExternalInput")
    ktabd = nc.dram_tensor("k_tab", [L, NH * A], F32R, kind="ExternalInput")
    ctabd = nc.dram_tensor("c_tab", [L, H], F32R, kind="ExternalInput")
    smtabd = nc.dram_tensor("sm_tab", [L, 3], F32R, kind="ExternalInput")
    Wsin_d = nc.dram_tensor("Wsin", [DIN, H], F32R, kind="ExternalInput")
    biase_d = nc.dram_tensor("bias_e", [H, 1], F32, kind="ExternalInput")
    Wq1 = nc.dram_tensor("Wq1", [H + DIN, H], F32, kind="ExternalInput")
    bq1 = nc.dram_tensor("bq1", [H], F32, kind="ExternalInput")
    Wq2 = nc.dram_tensor("Wq2", [H, NH * A], F32R, kind="ExternalInput")
    bq2 = nc.dram_tensor("bq2", [NH * A], F32, kind="ExternalInput")
    Wv1 = nc.dram_tensor("Wv1", [H + DIN, H], F32, kind="ExternalInput")
    bv1 = nc.dram_tensor("bv1", [H], F32, kind="ExternalInput")
    Wv2 = nc.dram_tensor("Wv2", [H, 2 * H], F32R, kind="ExternalInput")
    bv2 = nc.dram_tensor("bv2", [2 * H], F32, kind="ExternalInput")
    Wv = nc.dram_tensor("Wv", [H, NH * H], F32, kind="ExternalInput")
    Wo1 = nc.dram_tensor("Wo1", [NH * H, NH * H], F32, kind="ExternalInput")
    bo1p_d = nc.dram_tensor("bo1p", [128, 4], F32, kind="ExternalInput")
    Wo2 = nc.dram_tensor("Wo2", [NH * H, DOUT], F32, kind="ExternalInput")
    bo2 = nc.dram_tensor("bo2", [DOUT], F32, kind="ExternalInput")
    outd = nc.dram_tensor("out", [NPC, DOUT], F32, kind="ExternalOutput")

    with TileContext(nc) as tc:
        with tc.tile_pool(name="const", bufs=1) as cpool, \
             tc.tile_pool(name="work", bufs=2) as wpool, \
             tc.tile_pool(name="psA", bufs=5, space="PSUM") as psA, \
             tc.tile_pool(name="psS", bufs=3, space="PSUM") as psS, \
             tc.tile_pool(name="drp", bufs=4, space="DRAM") as drpool:

            # ============ one-time constants ============
            idn_i = cpool.tile([128, 128], I32)
            nc.gpsimd.iota(idn_i[:], [[1, 128]], base=0, channel_multiplier=-1)
            idn_f0 = cpool.tile([128, 128], F32)
            nc.vector.tensor_copy(idn_f0[:], idn_i[:])
            ident = cpool.tile([128, 128], F32)
            nc.vector.tensor_scalar(ident[:], idn_f0[:], 0.0, None, OP.is_equal)
            ident_bf = cpool.tile([128, 128], BF16)
            nc.vector.tensor_copy(ident_bf[:], ident[:])
            # per-partition iota columns (f32): values p and p+128
            iop_i = cpool.tile([128, 1], I32)
            nc.gpsimd.iota(iop_i[:], [[1, 1]], base=0, channel_multiplier=1)
            iota0 = cpool.tile([128, 1], F32)
            nc.vector.tensor_copy(iota0[:], iop_i[:])
            iota1 = cpool.tile([128, 1], F32)
            nc.vector.tensor_scalar(iota1[:], iota0[:], 128.0, None, OP.add)
            # blockones [128, NH] f32r : bo[c, h] = (c//A == h)
            blockones_f = cpool.tile([128, NH], F32)
            nc.gpsimd.memset(blockones_f[:], 0.0)
            for h in range(NH):
                nc.gpsimd.memset(blockones_f[h * A:(h + 1) * A, h:h + 1], 1.0)
            blockones = cpool.tile([128, NH], F32R)
            nc.vector.tensor_copy(blockones[:], blockones_f[:])
            # ACT-ordering token: sin ops of group g+1 wait on gelu ops of g
            tok = cpool.tile([128, 1], F32)
            nc.gpsimd.memset(tok[:], 0.0)

            # ============ weights (host-precomputed, straight DMA loads) ===
            def load_cast(dram_ap, shape, dt, tag):
                if dt in (F32, F32R):
                    t0 = cpool.tile(shape, dt, tag=tag + "_d", name=tag)
                    nc.sync.dma_start(t0[:], dram_ap)
                    return t0
                t0 = wpool.tile([128, 512], F32, tag="stage", name="stage_" + tag)
                nc.sync.dma_start(t0[0:shape[0], 0:shape[1]], dram_ap)
                t1 = cpool.tile(shape, dt, tag=tag)
                nc.vector.tensor_copy(t1[:], t0[0:shape[0], 0:shape[1]])
                return t1

            def load_bias(dram, n, tag):
                if n <= 128:
                    t = cpool.tile([n, 1], F32, tag=tag)
                    nc.sync.dma_start(t[:], dram[:].rearrange("(n o) -> n o", o=1))
                    return t
                k = n // 128
                t = cpool.tile([128, k], F32, tag=tag)
                nc.sync.dma_start(t[:], dram[:].rearrange("(j p) -> p j", p=128))
                return t

            Wsin_t = load_cast(Wsin_d[:], [DIN, H], F32R, "wsin")
            bias_e = cpool.tile([H, 1], F32, tag="bias_e")
            nc.sync.dma_start(bias_e[:], biase_d[:])
            Wq1_cc = load_cast(Wq1[0:DIN, :], [DIN, H], BF16, "wq1cc")
            Wq1_sin = load_cast(Wq1[DIN:DIN + 64, :], [64, H], BF16, "wq1sin")
            Wq1_cos = load_cast(Wq1[DIN + 64:DIN + 128, :], [64, H], BF16, "wq1cos")
            Wv1_cc = load_cast(Wv1[0:DIN, :], [DIN, H], BF16, "wv1cc")
            Wv1_sf = cpool.tile([128, H], F32, tag="wv1sf")
            nc.sync.dma_start(Wv1_sf[64:128, :], Wv1[DIN:DIN + 64, :])
            Wv1_sin_t = cpool.tile([128, H], BF16, tag="wv1sin")
            nc.vector.tensor_copy(Wv1_sin_t[64:128, :], Wv1_sf[64:128, :])
            Wv1_cf = cpool.tile([128, H], F32, tag="wv1cf")
            nc.sync.dma_start(Wv1_cf[64:128, :], Wv1[DIN + 64:DIN + 128, :])
            Wv1_cos_t = cpool.tile([128, H], BF16, tag="wv1cos")
            nc.vector.tensor_copy(Wv1_cos_t[64:128, :], Wv1_cf[64:128, :])
            Wv1_sin = Wv1_sin_t[64:128, :]
            Wv1_cos = Wv1_cos_t[64:128, :]
            Wq2_t = load_cast(Wq2[:], [H, NH * A], F32R, "wq2")
            Wv2_t = load_cast(Wv2[:], [H, 2 * H], F32R, "wv2")
            Wv_bf = load_cast(Wv[:], [H, NH * H], BF16, "wv")
            # Wo1 as [128, (c2, f) 2048] bf16 (staged through rotating buffer)
            Wo1_bf = cpool.tile([128, 4 * 512], BF16, tag="wo1")
            for c2 in range(4):
                st = wpool.tile([128, 512], F32, tag="stage", name=f"wo1st{c2}")
                nc.sync.dma_start(st[:], Wo1[c2 * 128:(c2 + 1) * 128, :])
                nc.vector.tensor_copy(Wo1_bf[:, c2 * 512:(c2 + 1) * 512], st[:])
            Wo2_f32 = cpool.tile([128, 4 * DOUT], F32, tag="wo2f")
            for c2 in range(4):
                nc.sync.dma_start(Wo2_f32[:, c2 * DOUT:(c2 + 1) * DOUT],
                                  Wo2[c2 * 128:(c2 + 1) * 128, :])
            Wo2_bf = cpool.tile([128, 4 * DOUT], BF16, tag="wo2")
            nc.vector.tensor_copy(Wo2_bf[:], Wo2_f32[:])

            bq1_t = load_bias(bq1, H, "bq1")
            bq2_t = load_bias(bq2, NH * A, "bq2")
            bv1_t = load_bias(bv1, H, "bv1")
            bv2_t = load_bias(bv2, 2 * H, "bv2")
            bo2_t = load_bias(bo2, DOUT, "bo2")
            bo1p = cpool.tile([128, 4], F32, tag="bo1p")
            nc.sync.dma_start(bo1p[:], bo1p_d[:])

            # ============ latent tables (host-precomputed) ============
            k_tab, c_tab, s_tab = [], [], []
            for lc in range(2):
                kl = cpool.tile([128, NH * A], F32R, tag=f"kl{lc}")
                nc.sync.dma_start(kl[:], ktabd[lc * 128:(lc + 1) * 128, :])
                k_tab.append(kl)
                cn = cpool.tile([128, 128], F32R, tag=f"cn{lc}")
                nc.sync.dma_start(cn[:], ctabd[lc * 128:(lc + 1) * 128, :])
                c_tab.append(cn)
                smr = cpool.tile([128, 3], F32R, tag=f"smr{lc}")
                nc.sync.dma_start(smr[:], smtabd[lc * 128:(lc + 1) * 128, :])
                s_tab.append(smr)
            pB = cpool.tile([128, 2 * L], F32, tag="pB")
            nc.sync.dma_start(pB[:], pBd[:])

            # ===== phase A1: distances, top-4, idx round trip =====
            def phase_a1(ci, j):
                n0 = ci * CHUNK
                x0 = wpool.tile([128, 2], F32, tag=f"x0_{j}", bufs=1)
                nc.sync.dma_start(x0[:], xd[n0:n0 + 128, :])
                x2T = wpool.tile([2, 128], F32, tag=f"x2T_{j}", bufs=1)
                nc.sync.dma_start(x2T[:], xd[n0:n0 + 128, :].rearrange("n c -> c n"))
                d0 = wpool.tile([128, 256], F32, tag="d0")
                nc.gpsimd.tensor_scalar(d0[:], pB[:, 0:L], x0[:, 0:1], None, OP.subtract)
                d1 = wpool.tile([128, 256], F32, tag="d1")
                nc.gpsimd.tensor_scalar(d1[:], pB[:, L:2 * L], x0[:, 1:2], None, OP.subtract)
                sq0 = wpool.tile([128, 256], F32, tag="sq0")
                nc.gpsimd.tensor_tensor(sq0[:], d0[:], d0[:], OP.mult)
                sq1 = wpool.tile([128, 256], F32, tag="sq1")
                nc.gpsimd.tensor_tensor(sq1[:], d1[:], d1[:], OP.mult)
                nzx = wpool.tile([128, 256], F32, tag="nzx")
                nc.vector.scalar_tensor_tensor(nzx[:], sq0[:], -1.0, sq1[:],
                                               OP.mult, OP.subtract)
                m8 = wpool.tile([128, 8], F32, tag=f"m8_{j}", bufs=1)
                nc.vector.max(m8[:], nzx[:])
                i8 = wpool.tile([128, 8], U32, tag="i8", bufs=3)
                nc.vector.max_index(i8[:], m8[:], nzx[:])
                idxb = wpool.tile([128, 4], BF16, tag="idxb", bufs=3)
                nc.vector.tensor_copy(idxb[:], i8[:, 0:4])

                # --- one-hot, latent-major: ohT[l, s*128+p] = (idx[p,s] == l)
                # idx -> DRAM (s-major) -> broadcast-read to all 128 partitions
                idx_dr = drpool.tile([4, 128], BF16, tag="idx_dr")
                nc.sync.dma_start(idx_dr[:].rearrange("s p -> p s"), idxb[:])
                idxB = wpool.tile([128, 512], BF16, tag="idxB", bufs=3)
                nc.sync.dma_start(
                    idxB[:],
                    idx_dr[:].rearrange("r n -> (r n)")
                    .rearrange("(o f) -> o f", o=1).to_broadcast([128, 512]))
                return dict(idxB=idxB, m8=m8, x0=x0, x2T=x2T)

            # ===== phase A2: gathers + sin features =====
            def phase_a2(ci, j, a1):
                idxB, m8, x2T = a1["idxB"], a1["m8"], a1["x2T"]
                ohT = [wpool.tile([128, 512], F32R, tag=f"ohT{lc}",
                                  name=f"ohT{lc}") for lc in range(2)]
                nc.gpsimd.tensor_scalar(ohT[0][:], idxB[:], iota0[:], None, OP.is_equal)
                nc.gpsimd.tensor_scalar(ohT[1][:], idxB[:], iota1[:], None, OP.is_equal)

                # --- gathers (single-pass f32r) ---
                ck_ps = psA.tile([128, 512], F32, tag="A")
                kk_ps = psA.tile([128, 512], F32, tag="A")
                smlg_ps = psS.tile([36, 512], F32, tag="S", name="smlg_s")
                sm_ps = smlg_ps[0:3, :]
                for lc in range(2):
                    nc.tensor.matmul(ck_ps[:], c_tab[lc][:], ohT[lc][:],
                                     start=(lc == 0), stop=(lc == 1))
                for lc in range(2):
                    nc.tensor.matmul(kk_ps[:], k_tab[lc][:], ohT[lc][:],
                                     start=(lc == 0), stop=(lc == 1))
                for lc in range(2):
                    nc.tensor.matmul(sm_ps, s_tab[lc][:], ohT[lc][:],
                                     start=(lc == 0), stop=(lc == 1))
                c_kT = wpool.tile([128, 512], BF16, tag=f"c_kT_{j}", bufs=1)
                nc.vector.tensor_copy(c_kT[:], ck_ps[:])
                k_kT = wpool.tile([128, 512], F32, tag=f"k_kT_{j}", bufs=1)
                nc.scalar.copy(k_kT[:], kk_ps[:])
                smT = wpool.tile([3, 512], F32, tag="smT", bufs=2)
                nc.scalar.copy(smT[:], sm_ps[:])
                # invg2 pixel-major [128, 12] for the softmax penalty
                smpm_ps = psS.tile([128, 16], F32, tag="S", name="smpm_s")[:, 0:12]
                for s in range(K):
                    nc.tensor.transpose(smpm_ps[:, s * 3:(s + 1) * 3],
                                        smT[:, s * 128:(s + 1) * 128], ident[0:3, 0:3])
                smpm = wpool.tile([128, 12], F32, tag=f"smpm_{j}", bufs=1)
                nc.vector.tensor_copy(smpm[:], smpm_ps[:])

                # --- sin features ---
                relp = wpool.tile([2, 512], F32R, tag="relp", bufs=2)
                nc.vector.tensor_tensor(
                    relp[:].rearrange("c (s n) -> c s n", s=4), sm_ps[0:2, :]
                    .rearrange("c (s n) -> c s n", s=4),
                    x2T[:].rearrange("c (s n) -> c s n", s=1).to_broadcast([2, 4, 128]),
                    OP.subtract)
                # cc = pi*(x - p + 1) = -pi*relp + pi ; f_cc = cc/(2pi) wrapped
                tcc = wpool.tile([2, 512], F32, tag="tcc", bufs=2)
                nc.vector.tensor_scalar(tcc[:], relp[:], -0.5, 0.5, OP.mult, OP.add)
                icc = wpool.tile([2, 512], I32, tag="icc", bufs=2)
                nc.gpsimd.tensor_copy(icc[:], tcc[:])
                fcc32 = wpool.tile([2, 512], F32, tag="fcc32", bufs=2)
                nc.gpsimd.tensor_copy(fcc32[:], icc[:])
                fcc = wpool.tile([2, 512], F32, tag="fcc", bufs=2)
                nc.vector.tensor_tensor(fcc[:], tcc[:], fcc32[:], OP.subtract)
                sincc = wpool.tile([2, 512], BF16, tag=f"sincc_{j}", bufs=1)
                nc.scalar.activation(sincc[:], fcc[:], AF.Sin, scale=float(2 * PI),
                                     bias=tok[0:2, 0:1])

                # te = e/(2pi) computed directly from relp via pre-folded weights
                # (Wsin pre-scaled by -0.5 on host; constant term added as ACT bias)
                e_ps = psA.tile([128, 512], F32, tag="A")
                nc.tensor.matmul(e_ps[:], Wsin_t[:], relp[:], start=True, stop=True)
                te = wpool.tile([128, 512], F32, tag="te", bufs=2)
                nc.scalar.activation(te[:], e_ps[:], AF.Identity, bias=bias_e[:, 0:1])
                ie = wpool.tile([128, 512], I32, tag="ie", bufs=2)
                nc.gpsimd.tensor_copy(ie[:], te[:])
                fe32 = wpool.tile([128, 512], F32, tag="fe32", bufs=2)
                nc.gpsimd.tensor_copy(fe32[:], ie[:])
                # fboth = [fe | 0.25-|fe|]; one Sin gives [sin(e) | cos(e)]
                fboth = wpool.tile([128, 1024], F32, tag="fboth", bufs=2)
                nc.vector.tensor_tensor(fboth[:, 0:512], te[:], fe32[:], OP.subtract)
                fab = wpool.tile([128, 512], F32, tag="fab", bufs=2)
                nc.vector.scalar_tensor_tensor(fab[:], fboth[:, 0:512], -1.0,
                                               fboth[:, 0:512], OP.mult, OP.max)
                nc.gpsimd.tensor_scalar(fboth[:, 512:1024], fab[:], -1.0, 0.25,
                                        OP.mult, OP.add)
                SCt = wpool.tile([128, 1024], BF16, tag=f"SCt_{j}", bufs=1)
                nc.scalar.activation(SCt[:], fboth[:], AF.Sin, scale=float(2 * PI),
                                     bias=tok[:, 0:1])
                return dict(SCt=SCt, sincc=sincc, c_kT=c_kT, k_kT=k_kT,
                            smpm=smpm, m8=m8)

            # ============ phase B1: q-side MLP, softmax, att DMAs ============
            def phase_b1(ci, j, a):
                SCt, sincc = a["SCt"], a["sincc"]
                k_kT, smpm, m8 = a["k_kT"], a["smpm"], a["m8"]

                h1q_ps = psA.tile([128, 512], F32, tag="A")
                nc.tensor.matmul(h1q_ps[:], Wq1_sin[:], SCt[0:64, 0:512], start=True, stop=False)
                nc.tensor.matmul(h1q_ps[:], Wq1_cos[:], SCt[0:64, 512:1024], start=False, stop=False)
                nc.tensor.matmul(h1q_ps[:], Wq1_cc[:], sincc[:], start=False, stop=True)
                h1q = wpool.tile([128, 512], F32R, tag="h1q", bufs=2)
                nc.scalar.activation(h1q[:], h1q_ps[:], AF.Gelu, bias=bq1_t[:, 0:1])
                q_ps = psA.tile([128, 512], F32, tag="A")
                nc.tensor.matmul(q_ps[:], Wq2_t[:], h1q[:], start=True, stop=True)

                qk = wpool.tile([128, 512], F32R, tag="qk", bufs=2)
                nc.vector.scalar_tensor_tensor(qk[:], q_ps[:], bq2_t[:, 0:1], k_kT[:],
                                               OP.add, OP.mult)

                # ---- logits + softmax (pixel-major), exp via tanh ----
                lg_ps = psS.tile([4, 512], F32, tag="S", name="lg_s")
                nc.tensor.matmul(lg_ps[:], blockones[:], qk[:], start=True, stop=True)
                lg_sb = wpool.tile([4, 512], F32, tag="lg_sb", bufs=2)
                nc.vector.tensor_copy(lg_sb[:], lg_ps[:])
                misc_ps = psS.tile([128, 512], F32, tag="S", name="misc_s")
                lgpm_ps = misc_ps[:, 0:16]
                for s in range(K):
                    nc.tensor.transpose(lgpm_ps[:, s * 4:(s + 1) * 4],
                                        lg_sb[:, s * 128:(s + 1) * 128], ident[0:4, 0:4])
                pen = wpool.tile([128, 4], F32, tag="pen", bufs=3)  # -zx*invg2
                nc.vector.tensor_tensor(
                    pen[:],
                    smpm[:].rearrange("p (s c) -> p s c", c=3)[:, :, 2:3]
                    .rearrange("p s o -> p (s o)"),
                    m8[:, 0:4], OP.mult)
                lgpm = wpool.tile([128, 16], F32, tag="lgpm", bufs=3)
                nc.vector.scalar_tensor_tensor(
                    lgpm[:].rearrange("p (s h) -> p s h", s=4),
                    lgpm_ps[:].rearrange("p (s h) -> p s h", s=4), 0.0,
                    pen[:].to_broadcast([128, 4, 4]), OP.add, OP.add)
                # exp(x) = (1+t)/(1-t), t = tanh(x/2)  (keeps ACT in gelu set)
                # logits are bounded (~[-10, 1]); no max-subtraction needed
                th = wpool.tile([128, 16], F32, tag="th", bufs=3)
                nc.scalar.activation(th[:], lgpm[:], AF.Tanh, scale=0.5)
                num = wpool.tile([128, 16], F32, tag="num", bufs=3)
                nc.vector.tensor_scalar(num[:], th[:], 1.0, None, OP.add)
                den = wpool.tile([128, 16], F32, tag="den", bufs=3)
                nc.vector.tensor_scalar(den[:], th[:], -1.0, 1.0, OP.mult, OP.add)
                rcp = wpool.tile([128, 16], F32, tag="rcp", bufs=3)
                nc.vector.reciprocal(rcp[:], den[:])
                epm = wpool.tile([128, 16], F32, tag="epm", bufs=3)
                nc.vector.tensor_tensor(epm[:], num[:], rcp[:], OP.mult)
                zs = wpool.tile([128, 4], F32, tag="zs", bufs=3)
                nc.vector.tensor_reduce(
                    zs[:], epm[:].rearrange("p (s h) -> p h s", s=4),
                    mybir.AxisListType.X, OP.add)
                rz = wpool.tile([128, 4], F32, tag="rz", bufs=3)
                nc.vector.reciprocal(rz[:], zs[:])
                att_pm = wpool.tile([128, 16], F32, tag="att_pm", bufs=4)
                nc.vector.tensor_tensor(
                    att_pm[:].rearrange("p (h s) -> p s h", h=4),
                    epm[:].rearrange("p (s h) -> p s h", s=4),
                    rz[:].rearrange("p (h o) -> p o h", o=1).to_broadcast([128, 4, 4]),
                    OP.mult)
                att_ps = misc_ps[0:16, 64:192]
                nc.tensor.transpose(att_ps, att_pm[:], ident[:])
                att_sh = wpool.tile([16, 128], BF16, tag="att_sh", bufs=4)
                nc.vector.tensor_copy(att_sh[:], att_ps)
                att_dr = drpool.tile([16, 128], BF16, tag="att_dr")
                nc.sync.dma_start(att_dr[:], att_sh[:])
                # broadcast att rows to all 128 partitions: [128, (h,s,p) 2048]
                attB = wpool.tile([128, 2048], BF16, tag="attB", bufs=3)
                nc.sync.dma_start(
                    attB[:],
                    att_dr[:].rearrange("r n -> (r n)")
                    .rearrange("(o f) -> o f", o=1).to_broadcast([128, 2048]))

                return dict(attB=attB)

            # ============ phase B2: v-side MLP, attention apply, output ======
            def phase_b2(ci, j, a, b):
                n0 = ci * CHUNK
                SCt, sincc, c_kT = a["SCt"], a["sincc"], a["c_kT"]
                attB = b["attB"]
                h1v_ps = psA.tile([128, 512], F32, tag="A")
                nc.tensor.matmul(h1v_ps[:], Wv1_sin, SCt[64:128, 0:512], start=True, stop=False)
                nc.tensor.matmul(h1v_ps[:], Wv1_cos, SCt[64:128, 512:1024], start=False, stop=False)
                nc.tensor.matmul(h1v_ps[:], Wv1_cc[:], sincc[:], start=False, stop=True)
                h1v = wpool.tile([128, 512], F32R, tag="h1v", bufs=2)
                nc.scalar.activation(h1v[:], h1v_ps[:], AF.Gelu, bias=bv1_t[:, 0:1])
                vg_ps = psA.tile([128, 512], F32, tag="A")
                nc.tensor.matmul(vg_ps[:], Wv2_t[:, 0:H], h1v[:], start=True, stop=True)
                vb_ps = psA.tile([128, 512], F32, tag="A")
                nc.tensor.matmul(vb_ps[:], Wv2_t[:, H:2 * H], h1v[:], start=True, stop=True)
                utmp = wpool.tile([128, 512], F32, tag="utmp", bufs=2)
                nc.vector.scalar_tensor_tensor(utmp[:], vg_ps[:], bv2_t[:, 0:1],
                                               c_kT[:], OP.add, OP.mult)
                u_bf = wpool.tile([128, 512], BF16, tag="u_bf", bufs=3)
                nc.vector.scalar_tensor_tensor(u_bf[:], vb_ps[:], bv2_t[:, 1:2],
                                               utmp[:], OP.add, OP.add)

                # ---- apply attention + output MLP ----
                uw = wpool.tile([128, 2048], BF16, tag="uw", bufs=3)
                for h in range(NH):
                    eng = nc.gpsimd if h < 1 else nc.vector
                    eng.tensor_tensor(uw[:, h * 512:(h + 1) * 512], u_bf[:],
                                      attB[:, h * 512:(h + 1) * 512], OP.mult)
                y_ps = psA.tile([128, 512], F32, tag="A")
                for h in range(NH):
                    for s in range(K):
                        nc.tensor.matmul(
                            y_ps[:, h * 128:(h + 1) * 128],
                            Wv_bf[:, h * 128:(h + 1) * 128],
                            uw[:, h * 512 + s * 128:h * 512 + (s + 1) * 128],
                            start=(s == 0), stop=(s == 3))
                y_bf = wpool.tile([128, 512], BF16, tag="y_bf", bufs=3)
                nc.scalar.copy(y_bf[:], y_ps[:])
                y1_ps = psA.tile([128, 512], F32, tag="A")
                for f2 in range(4):
                    for h in range(4):
                        nc.tensor.matmul(
                            y1_ps[:, f2 * 128:(f2 + 1) * 128],
                            Wo1_bf[:, h * 512 + f2 * 128:h * 512 + (f2 + 1) * 128],
                            y_bf[:, h * 128:(h + 1) * 128],
                            start=(h == 0), stop=(h == 3))
                y1 = wpool.tile([128, 512], BF16, tag="y1", bufs=3)
                for f2 in range(4):
                    nc.scalar.activation(y1[:, f2 * 128:(f2 + 1) * 128],
                                         y1_ps[:, f2 * 128:(f2 + 1) * 128],
                                         AF.Gelu, bias=bo1p[:, f2:f2 + 1])
                if j == GS - 1:
                    # refresh the ACT-ordering token after this group's gelus
                    nc.scalar.activation(tok[:], y1[:, 0:1], AF.Copy, scale=0.0)
                misc2_ps = psS.tile([128, 512], F32, tag="S", name="misc2_s")
                o_ps = misc2_ps[0:3, 0:128]
                for c2 in range(4):
                    nc.tensor.matmul(o_ps, Wo2_bf[:, c2 * 3:(c2 + 1) * 3],
                                     y1[:, c2 * 128:(c2 + 1) * 128],
                                     start=(c2 == 0), stop=(c2 == 3))
                o_sb = wpool.tile([3, 128], F32, tag="o_sb", bufs=3)
                nc.scalar.activation(o_sb[:], o_ps, AF.Identity, bias=bo2_t[:, 0:1])
                nc.sync.dma_start(outd[n0:n0 + 128, :].rearrange("n c -> c n"), o_sb[:])

            # ============ main loop: groups of GS chunks, A then B1/B2 =======
            # B1(j+1) is emitted before B2(j) so the next chunk's q-side MLP
            # fills the attention-broadcast DMA latency.
            for g in range(nchunk // GS):
                a1s = [phase_a1(g * GS, 0)]
                acc = []
                for j in range(GS):
                    if j + 1 < GS:
                        a1s.append(phase_a1(g * GS + j + 1, j + 1))
                    acc.append(phase_a2(g * GS + j, j, a1s[j]))
                bts = [phase_b1(g * GS, 0, acc[0]),
                       phase_b1(g * GS + 1, 1, acc[1])]
                for j in range(GS):
                    if j + 2 < GS:
                        bts.append(phase_b1(g * GS + j + 2, j + 2, acc[j + 2]))
                    phase_b2(g * GS + j, j, acc[j], bts[j])

    nc.compile()
    return nc


def make_in_maps(inputs):
    x = np.asarray(inputs["x"], np.float32)
    f = {k: np.asarray(v, np.float32) for k, v in inputs.items()}

    # ---- host-side precompute of weight/latent-derived constants ----
    wcom = {k: np.ascontiguousarray(f[k]) for k in
            ["Wq1", "bq1", "Wq2", "bq2", "Wv1", "bv1", "Wv2", "bv2",
             "Wv", "Wo1", "Wo2", "bo2"]}
    wcom["Wsin"] = np.ascontiguousarray(
        -0.5 * np.concatenate([f["Wq_sin"], f["Wv_sin"]], axis=1))
    bias_e = np.concatenate([0.5 * f["Wq_sin"].sum(0), 0.5 * f["Wv_sin"].sum(0)])
    wcom["bias_e"] = np.ascontiguousarray(bias_e.reshape(H, 1))
    bo1p = f["bo1"] + f["Wo1"].T @ f["bv"]
    wcom["bo1p"] = np.ascontiguousarray(bo1p.reshape(4, 128).T)

    in_maps = []
    for core in range(NCORE):
        b = core // (NCORE // B)
        sh = (core % (NCORE // B))
        m = dict(wcom)
        m["x"] = np.ascontiguousarray(x[b, sh * NPC:(sh + 1) * NPC])
        p, c, g = f["p"][b], f["c"][b], f["g"][b]
        cstem = c @ f["W_stem"] + f["b_stem"]          # [L, H]
        m["c_tab"] = np.ascontiguousarray(cstem)
        m["k_tab"] = np.ascontiguousarray(cstem @ f["Wk"] + f["bk"])
        sm = np.concatenate([p, 1.0 / (g * g)], axis=1)  # [L, 3]
        m["sm_tab"] = np.ascontiguousarray(sm)
        pB = np.concatenate([p[:, 0], p[:, 1]])          # [2L]
        m["pB"] = np.ascontiguousarray(np.broadcast_to(pB, (128, 2 * L)))
        in_maps.append(m)
    return in_maps


def kernel(**inputs):
    import jax
    try:
        jax.config.update('jax_platforms', 'axon,cpu')
    except Exception:
        pass
    from concourse.bass_utils import run_bass_kernel_spmd

    nchunk = NPC // CHUNK
    if nchunk not in _cache:
        _cache[nchunk] = _build(nchunk)
    nc = _cache[nchunk]

    in_maps = make_in_maps(inputs)
    res = run_bass_kernel_spmd(nc, in_maps, core_ids=list(range(NCORE)))
    out = np.zeros((B, N, DOUT), np.float32)
    for core in range(NCORE):
        b = core // (NCORE // B)
        sh = core % (NCORE // B)
        out[b, sh * NPC:(sh + 1) * NPC] = res.results[core]["out"]
    return out


# revision 66
# speedup vs baseline: 2.2504x; 1.0011x over previous
"""Trainium2 Bass kernel for nn_EquivariantNeuralField.

Per-pixel top-4-nearest-latent cross-attention neural field.
Sharding: 8 cores; core i handles batch i//4, pixel rows (i%4)*4096..+4096.

v2: phase-split pipeline (A=trig table, B=gelu table) over 4-chunk groups
to kill activation-table thrashing; f32r single-pass gathers; latent-major
one-hot build (no big transposes); exp via tanh; DMA-broadcast attention.
"""
import numpy as np

B, N, L, K = 2, 16384, 256, 4
DIN, DOUT, DLAT, H, A, NH = 2, 3, 64, 128, 32, 4
NCORE = 8
NPC = N * B // NCORE          # pixels per core = 4096
CHUNK = 128
GS = 8                        # chunks per phase group
PI = float(np.pi)

_cache = {}


def _build(nchunk):
    import concourse.bacc as bacc
    import concourse.mybir as mybir
    from concourse.tile import TileContext

    F32 = mybir.dt.float32
    F32R = mybir.dt.float32r
    BF16 = mybir.dt.bfloat16
    I32 = mybir.dt.int32
    U32 = mybir.dt.uint32
    AF = mybir.ActivationFunctionType
    OP = mybir.AluOpType

    nc = bacc.Bacc()

    # ---------------- DRAM tensors ----------------
    # Tables and weight-derived constants are precomputed host-side in kernel().
    xd = nc.dram_tensor("x", [NPC, DIN], F32, kind="ExternalInput")
    pBd = nc.dram_tensor("pB", [128, 2 * L], F32, kind="ExternalInput")
    ktabd = nc.dram_tensor("k_tab", [L, NH * A], F32R, kind="ExternalInput")
    ctabd = nc.dram_tensor("c_tab", [L, H], F32R, kind="ExternalInput")
    smtabd = nc.dram_tensor("sm_tab", [L, 3], F32R, kind="ExternalInput")
    Wsin_d = nc.dram_tensor("Wsin", [DIN, H], F32R, kind="ExternalInput")
    biase_d = nc.dram_tensor("bias_e", [H, 1], F32, kind="ExternalInput")
    Wq1 = nc.dram_tensor("Wq1", [H + DIN, H], F32, kind="ExternalInput")
    bq1 = nc.dram_tensor("bq1", [H], F32, kind="ExternalInput")
    Wq2 = nc.dram_tensor("Wq2", [H, NH * A], F32R, kind="ExternalInput")
    bq2 = nc.dram_tensor("bq2", [NH * A], F32, kind="ExternalInput")
    Wv1 = nc.dram_tensor("Wv1", [H + DIN, H], F32, kind="ExternalInput")
    bv1 = nc.dram_tensor("bv1", [H], F32, kind="ExternalInput")
    Wv2 = nc.dram_tensor("Wv2", [H, 2 * H], F32R, kind="ExternalInput")
    bv2 = nc.dram_tensor("bv2", [2 * H], F32, kind="ExternalInput")
    Wv = nc.dram_tensor("Wv", [H, NH * H], F32, kind="ExternalInput")
    Wo1 = nc.dram_tensor("Wo1", [NH * H, NH * H], F32, kind="ExternalInput")
    bo1p_d = nc.dram_tensor("bo1p", [128, 4], F32, kind="ExternalInput")
    Wo2 = nc.dram_tensor("Wo2", [NH * H, DOUT], F32, kind="ExternalInput")
    bo2 = nc.dram_tensor("bo2", [DOUT], F32, kind="ExternalInput")
    outd = nc.dram_tensor("out", [NPC, DOUT], F32, kind="ExternalOutput")

    with TileContext(nc) as tc:
        with tc.tile_pool(name="const", bufs=1) as cpool, \
             tc.tile_pool(name="work", bufs=2) as wpool, \
             tc.tile_pool(name="psA", bufs=6, space="PSUM") as psA, \
             tc.tile_pool(name="psS", bufs=2, space="PSUM") as psS, \
             tc.tile_pool(name="drp", bufs=4, space="DRAM") as drpool:

            # ============ one-time constants ============
            idn_i = cpool.tile([128, 128], I32)
            nc.gpsimd.iota(idn_i[:], [[1, 128]], base=0, channel_multiplier=-1)
            idn_f0 = cpool.tile([128, 128], F32)
            nc.vector.tensor_copy(idn_f0[:], idn_i[:])
            ident = cpool.tile([128, 128], F32)
            nc.vector.tensor_scalar(ident[:], idn_f0[:], 0.0, None, OP.is_equal)
            ident_bf = cpool.tile([128, 128], BF16)
            nc.vector.tensor_copy(ident_bf[:], ident[:])
            # per-partition iota columns (f32): values p and p+128
            iop_i = cpool.tile([128, 1], I32)
            nc.gpsimd.iota(iop_i[:], [[1, 1]], base=0, channel_multiplier=1)
            iota0 = cpool.tile([128, 1], F32)
            nc.vector.tensor_copy(iota0[:], iop_i[:])
            iota1 = cpool.tile([128, 1], F32)
            nc.vector.tensor_scalar(iota1[:], iota0[:], 128.0, None, OP.add)
            # blockones [128, NH] f32r : bo[c, h] = (c//A == h)
            blockones_f = cpool.tile([128, NH], F32)
            nc.gpsimd.memset(blockones_f[:], 0.0)
            for h in range(NH):
                nc.gpsimd.memset(blockones_f[h * A:(h + 1) * A, h:h + 1], 1.0)
            blockones = cpool.tile([128, NH], F32R)
            nc.vector.tensor_copy(blockones[:], blockones_f[:])
            # ACT-ordering token: sin ops of group g+1 wait on gelu ops of g
            tok = cpool.tile([128, 1], F32)
            nc.gpsimd.memset(tok[:], 0.0)

            # ============ weights (host-precomputed, straight DMA loads) ===
            def load_cast(dram_ap, shape, dt, tag):
                if dt in (F32, F32R):
                    t0 = cpool.tile(shape, dt, tag=tag + "_d", name=tag)
                    nc.sync.dma_start(t0[:], dram_ap)
                    return t0
                t0 = wpool.tile([128, 512], F32, tag="stage", name="stage_" + tag)
                nc.sync.dma_start(t0[0:shape[0], 0:shape[1]], dram_ap)
                t1 = cpool.tile(shape, dt, tag=tag)
                nc.vector.tensor_copy(t1[:], t0[0:shape[0], 0:shape[1]])
                return t1

            def load_bias(dram, n, tag):
                if n <= 128:
                    t = cpool.tile([n, 1], F32, tag=tag)
                    nc.sync.dma_start(t[:], dram[:].rearrange("(n o) -> n o", o=1))
                    return t
                k = n // 128
                t = cpool.tile([128, k], F32, tag=tag)
                nc.sync.dma_start(t[:], dram[:].rearrange("(j p) -> p j", p=128))
                return t

            Wsin_t = load_cast(Wsin_d[:], [DIN, H], F32R, "wsin")
            bias_e = cpool.tile([H, 1], F32, tag="bias_e")
            nc.sync.dma_start(bias_e[:], biase_d[:])
            Wq1_cc = load_cast(Wq1[0:DIN, :], [DIN, H], BF16, "wq1cc")
            Wq1_sin = load_cast(Wq1[DIN:DIN + 64, :], [64, H], BF16, "wq1sin")
            Wq1_cos = load_cast(Wq1[DIN + 64:DIN + 128, :], [64, H], BF16, "wq1cos")
            Wv1_cc = load_cast(Wv1[0:DIN, :], [DIN, H], BF16, "wv1cc")
            Wv1_sf = cpool.tile([128, H], F32, tag="wv1sf")
            nc.sync.dma_start(Wv1_sf[64:128, :], Wv1[DIN:DIN + 64, :])
            Wv1_sin_t = cpool.tile([128, H], BF16, tag="wv1sin")
            nc.vector.tensor_copy(Wv1_sin_t[64:128, :], Wv1_sf[64:128, :])
            Wv1_cf = cpool.tile([128, H], F32, tag="wv1cf")
            nc.sync.dma_start(Wv1_cf[64:128, :], Wv1[DIN + 64:DIN + 128, :])
            Wv1_cos_t = cpool.tile([128, H], BF16, tag="wv1cos")
            nc.vector.tensor_copy(Wv1_cos_t[64:128, :], Wv1_cf[64:128, :])
            Wv1_sin = Wv1_sin_t[64:128, :]
            Wv1_cos = Wv1_cos_t[64:128, :]
            Wq2_t = load_cast(Wq2[:], [H, NH * A], F32R, "wq2")
            Wv2_t = load_cast(Wv2[:], [H, 2 * H], F32R, "wv2")
            Wv_bf = load_cast(Wv[:], [H, NH * H], BF16, "wv")
            # Wo1 as [128, (c2, f) 2048] bf16 (staged through rotating buffer)
            Wo1_bf = cpool.tile([128, 4 * 512], BF16, tag="wo1")
            for c2 in range(4):
                st = wpool.tile([128, 512], F32, tag="stage", name=f"wo1st{c2}")
                nc.sync.dma_start(st[:], Wo1[c2 * 128:(c2 + 1) * 128, :])
                nc.vector.tensor_copy(Wo1_bf[:, c2 * 512:(c2 + 1) * 512], st[:])
            Wo2_f32 = cpool.tile([128, 4 * DOUT], F32, tag="wo2f")
            for c2 in range(4):
                nc.sync.dma_start(Wo2_f32[:, c2 * DOUT:(c2 + 1) * DOUT],
                                  Wo2[c2 * 128:(c2 + 1) * 128, :])
            Wo2_bf = cpool.tile([128, 4 * DOUT], BF16, tag="wo2")
            nc.vector.tensor_copy(Wo2_bf[:], Wo2_f32[:])

            bq1_t = load_bias(bq1, H, "bq1")
            bq2_t = load_bias(bq2, NH * A, "bq2")
            bv1_t = load_bias(bv1, H, "bv1")
            bv2_t = load_bias(bv2, 2 * H, "bv2")
            bo2_t = load_bias(bo2, DOUT, "bo2")
            bo1p = cpool.tile([128, 4], F32, tag="bo1p")
            nc.sync.dma_start(bo1p[:], bo1p_d[:])

            # ============ latent tables (host-precomputed) ============
            k_tab, c_tab, s_tab = [], [], []
            for lc in range(2):
                kl = cpool.tile([128, NH * A], F32R, tag=f"kl{lc}")
                nc.sync.dma_start(kl[:], ktabd[lc * 128:(lc + 1) * 128, :])
                k_tab.append(kl)
                cn = cpool.tile([128, 128], F32R, tag=f"cn{lc}")
                nc.sync.dma_start(cn[:], ctabd[lc * 128:(lc + 1) * 128, :])
                c_tab.append(cn)
                smr = cpool.tile([128, 3], F32R, tag=f"smr{lc}")
                nc.sync.dma_start(smr[:], smtabd[lc * 128:(lc + 1) * 128, :])
                s_tab.append(smr)
            pB = cpool.tile([128, 2 * L], F32, tag="pB")
            nc.sync.dma_start(pB[:], pBd[:])

            # ===== phase A1: distances, top-4, idx round trip =====
            def phase_a1(ci, j):
                n0 = ci * CHUNK
                x0 = wpool.tile([128, 2], F32, tag=f"x0_{j}", bufs=1)
                nc.sync.dma_start(x0[:], xd[n0:n0 + 128, :])
                x2T = wpool.tile([2, 128], F32, tag=f"x2T_{j}", bufs=1)
                nc.sync.dma_start(x2T[:], xd[n0:n0 + 128, :].rearrange("n c -> c n"))
                d0 = wpool.tile([128, 256], F32, tag="d0")
                nc.gpsimd.tensor_scalar(d0[:], pB[:, 0:L], x0[:, 0:1], None, OP.subtract)
                d1 = wpool.tile([128, 256], F32, tag="d1")
                nc.gpsimd.tensor_scalar(d1[:], pB[:, L:2 * L], x0[:, 1:2], None, OP.subtract)
                sq0 = wpool.tile([128, 256], F32, tag="sq0")
                nc.gpsimd.tensor_tensor(sq0[:], d0[:], d0[:], OP.mult)
                sq1 = wpool.tile([128, 256], F32, tag="sq1")
                nc.gpsimd.tensor_tensor(sq1[:], d1[:], d1[:], OP.mult)
                nzx = wpool.tile([128, 256], F32, tag="nzx")
                nc.vector.scalar_tensor_tensor(nzx[:], sq0[:], -1.0, sq1[:],
                                               OP.mult, OP.subtract)
                m8 = wpool.tile([128, 8], F32, tag=f"m8_{j}", bufs=1)
                nc.vector.max(m8[:], nzx[:])
                i8 = wpool.tile([128, 8], U32, tag="i8", bufs=3)
                nc.vector.max_index(i8[:], m8[:], nzx[:])
                idxb = wpool.tile([128, 4], BF16, tag="idxb", bufs=3)
                nc.vector.tensor_copy(idxb[:], i8[:, 0:4])

                # --- one-hot, latent-major: ohT[l, s*128+p] = (idx[p,s] == l)
                # idx -> DRAM (s-major) -> broadcast-read to all 128 partitions
                idx_dr = drpool.tile([4, 128], BF16, tag="idx_dr")
                nc.sync.dma_start(idx_dr[:].rearrange("s p -> p s"), idxb[:])
                idxB = wpool.tile([128, 512], BF16, tag="idxB", bufs=3)
                nc.sync.dma_start(
                    idxB[:],
                    idx_dr[:].rearrange("r n -> (r n)")
                    .rearrange("(o f) -> o f", o=1).to_broadcast([128, 512]))
                return dict(idxB=idxB, m8=m8, x0=x0, x2T=x2T)

            # ===== phase A2: gathers + sin features =====
            def phase_a2(ci, j, a1):
                idxB, m8, x2T = a1["idxB"], a1["m8"], a1["x2T"]
                ohT = [wpool.tile([128, 512], F32R, tag=f"ohT{lc}",
                                  name=f"ohT{lc}") for lc in range(2)]
                nc.gpsimd.tensor_scalar(ohT[0][:], idxB[:], iota0[:], None, OP.is_equal)
                nc.gpsimd.tensor_scalar(ohT[1][:], idxB[:], iota1[:], None, OP.is_equal)

                # --- gathers (single-pass f32r) ---
                ck_ps = psA.tile([128, 512], F32, tag="A")
                kk_ps = psA.tile([128, 512], F32, tag="A")
                smlg_ps = psS.tile([36, 512], F32, tag="S", name="smlg_s")
                sm_ps = smlg_ps[0:3, :]
                for lc in range(2):
                    nc.tensor.matmul(ck_ps[:], c_tab[lc][:], ohT[lc][:],
                                     start=(lc == 0), stop=(lc == 1))
                for lc in range(2):
                    nc.tensor.matmul(kk_ps[:], k_tab[lc][:], ohT[lc][:],
                                     start=(lc == 0), stop=(lc == 1))
                for lc in range(2):
                    nc.tensor.matmul(sm_ps, s_tab[lc][:], ohT[lc][:],
                                     start=(lc == 0), stop=(lc == 1))
                c_kT = wpool.tile([128, 512], BF16, tag=f"c_kT_{j}", bufs=1)
                nc.vector.tensor_copy(c_kT[:], ck_ps[:])
                k_kT = wpool.tile([128, 512], F32, tag=f"k_kT_{j}", bufs=1)
                nc.scalar.copy(k_kT[:], kk_ps[:])
                smT = wpool.tile([3, 512], F32, tag="smT", bufs=2)
                nc.scalar.copy(smT[:], sm_ps[:])
                # invg2 pixel-major [128, 12] for the softmax penalty
                smpm_ps = psS.tile([128, 16], F32, tag="S", name="smpm_s")[:, 0:12]
                for s in range(K):
                    nc.tensor.transpose(smpm_ps[:, s * 3:(s + 1) * 3],
                                        smT[:, s * 128:(s + 1) * 128], ident[0:3, 0:3])
                smpm = wpool.tile([128, 12], F32, tag=f"smpm_{j}", bufs=1)
                nc.vector.tensor_copy(smpm[:], smpm_ps[:])

                # --- sin features ---
                relp = wpool.tile([2, 512], F32R, tag="relp", bufs=2)
                nc.vector.tensor_tensor(
                    relp[:].rearrange("c (s n) -> c s n", s=4), sm_ps[0:2, :]
                    .rearrange("c (s n) -> c s n", s=4),
                    x2T[:].rearrange("c (s n) -> c s n", s=1).to_broadcast([2, 4, 128]),
                    OP.subtract)
                # cc = pi*(x - p + 1) = -pi*relp + pi ; f_cc = cc/(2pi) wrapped
                tcc = wpool.tile([2, 512], F32, tag="tcc", bufs=2)
                nc.vector.tensor_scalar(tcc[:], relp[:], -0.5, 0.5, OP.mult, OP.add)
                icc = wpool.tile([2, 512], I32, tag="icc", bufs=2)
                nc.gpsimd.tensor_copy(icc[:], tcc[:])
                fcc32 = wpool.tile([2, 512], F32, tag="fcc32", bufs=2)
                nc.gpsimd.tensor_copy(fcc32[:], icc[:])
                fcc = wpool.tile([2, 512], F32, tag="fcc", bufs=2)
                nc.vector.tensor_tensor(fcc[:], tcc[:], fcc32[:], OP.subtract)
                sincc = wpool.tile([2, 512], BF16, tag=f"sincc_{j}", bufs=1)
                nc.scalar.activation(sincc[:], fcc[:], AF.Sin, scale=float(2 * PI),
                                     bias=tok[0:2, 0:1])

                # te = e/(2pi) computed directly from relp via pre-folded weights
                # (Wsin pre-scaled by -0.5 on host; constant term added as ACT bias)
                e_ps = psA.tile([128, 512], F32, tag="A")
                nc.tensor.matmul(e_ps[:], Wsin_t[:], relp[:], start=True, stop=True)
                te = wpool.tile([128, 512], F32, tag="te", bufs=2)
                nc.scalar.activation(te[:], e_ps[:], AF.Identity, bias=bias_e[:, 0:1])
                ie = wpool.tile([128, 512], I32, tag="ie", bufs=2)
                nc.gpsimd.tensor_copy(ie[:], te[:])
                fe32 = wpool.tile([128, 512], F32, tag="fe32", bufs=2)
                nc.gpsimd.tensor_copy(fe32[:], ie[:])
                # fboth = [fe | 0.25-|fe|]; one Sin gives [sin(e) | cos(e)]
                fboth = wpool.tile([128, 1024], F32, tag="fboth", bufs=2)
                nc.vector.tensor_tensor(fboth[:, 0:512], te[:], fe32[:], OP.subtract)
                fab = wpool.tile([128, 512], F32, tag="fab", bufs=2)
                nc.vector.scalar_tensor_tensor(fab[:], fboth[:, 0:512], -1.0,
                                               fboth[:, 0:512], OP.mult, OP.max)
                nc.gpsimd.tensor_scalar(fboth[:, 512:1024], fab[:], -1.0, 0.25,
                                        OP.mult, OP.add)
                SCt = wpool.tile([128, 1024], BF16, tag=f"SCt_{j}", bufs=1)
                nc.scalar.activation(SCt[:], fboth[:], AF.Sin, scale=float(2 * PI),
                                     bias=tok[:, 0:1])
                return dict(SCt=SCt, sincc=sincc, c_kT=c_kT, k_kT=k_kT,
                            smpm=smpm, m8=m8)

            # ============ phase B1: q-side MLP, softmax, att DMAs ============
            def phase_b1(ci, j, a):
                SCt, sincc = a["SCt"], a["sincc"]
                k_kT, smpm, m8 = a["k_kT"], a["smpm"], a["m8"]

                h1q_ps = psA.tile([128, 512], F32, tag="A")
                nc.tensor.matmul(h1q_ps[:], Wq1_sin[:], SCt[0:64, 0:512], start=True, stop=False)
                nc.tensor.matmul(h1q_ps[:], Wq1_cos[:], SCt[0:64, 512:1024], start=False, stop=False)
                nc.tensor.matmul(h1q_ps[:], Wq1_cc[:], sincc[:], start=False, stop=True)
                h1q = wpool.tile([128, 512], F32R, tag="h1q", bufs=3)
                nc.scalar.activation(h1q[:], h1q_ps[:], AF.Gelu, bias=bq1_t[:, 0:1])
                q_ps = psA.tile([128, 512], F32, tag="A")
                nc.tensor.matmul(q_ps[:], Wq2_t[:], h1q[:], start=True, stop=True)

                qk = wpool.tile([128, 512], F32R, tag="qk", bufs=2)
                nc.vector.scalar_tensor_tensor(qk[:], q_ps[:], bq2_t[:, 0:1], k_kT[:],
                                               OP.add, OP.mult)

                # ---- logits + softmax (pixel-major), exp via tanh ----
                lg_ps = psS.tile([4, 512], F32, tag="S", name="lg_s")
                nc.tensor.matmul(lg_ps[:], blockones[:], qk[:], start=True, stop=True)
                lg_sb = wpool.tile([4, 512], F32, tag="lg_sb", bufs=2)
                nc.vector.tensor_copy(lg_sb[:], lg_ps[:])
                misc_ps = psS.tile([128, 512], F32, tag="S", name="misc_s")
                lgpm_ps = misc_ps[:, 0:16]
                for s in range(K):
                    nc.tensor.transpose(lgpm_ps[:, s * 4:(s + 1) * 4],
                                        lg_sb[:, s * 128:(s + 1) * 128], ident[0:4, 0:4])
                pen = wpool.tile([128, 4], F32, tag="pen", bufs=3)  # -zx*invg2
                nc.vector.tensor_tensor(
                    pen[:],
                    smpm[:].rearrange("p (s c) -> p s c", c=3)[:, :, 2:3]
                    .rearrange("p s o -> p (s o)"),
                    m8[:, 0:4], OP.mult)
                lgpm = wpool.tile([128, 16], F32, tag="lgpm", bufs=3)
                nc.vector.scalar_tensor_tensor(
                    lgpm[:].rearrange("p (s h) -> p s h", s=4),
                    lgpm_ps[:].rearrange("p (s h) -> p s h", s=4), 0.0,
                    pen[:].to_broadcast([128, 4, 4]), OP.add, OP.add)
                # exp(x) = (1+t)/(1-t), t = tanh(x/2)  (keeps ACT in gelu set)
                # logits are bounded (~[-10, 1]); no max-subtraction needed
                th = wpool.tile([128, 16], F32, tag="th", bufs=3)
                nc.scalar.activation(th[:], lgpm[:], AF.Tanh, scale=0.5)
                num = wpool.tile([128, 16], F32, tag="num", bufs=3)
                nc.vector.tensor_scalar(num[:], th[:], 1.0, None, OP.add)
                den = wpool.tile([128, 16], F32, tag="den", bufs=3)
                nc.vector.tensor_scalar(den[:], th[:], -1.0, 1.0, OP.mult, OP.add)
                rcp = wpool.tile([128, 16], F32, tag="rcp", bufs=3)
                nc.vector.reciprocal(rcp[:], den[:])
                epm = wpool.tile([128, 16], F32, tag="epm", bufs=3)
                nc.vector.tensor_tensor(epm[:], num[:], rcp[:], OP.mult)
                zs = wpool.tile([128, 4], F32, tag="zs", bufs=3)
                nc.vector.tensor_reduce(
                    zs[:], epm[:].rearrange("p (s h) -> p h s", s=4),
                    mybir.AxisListType.X, OP.add)
                rz = wpool.tile([128, 4], F32, tag="rz", bufs=3)
                nc.vector.reciprocal(rz[:], zs[:])
                att_pm = wpool.tile([128, 16], F32, tag="att_pm", bufs=4)
                nc.vector.tensor_tensor(
                    att_pm[:].rearrange("p (h s) -> p s h", h=4),
                    epm[:].rearrange("p (s h) -> p s h", s=4),
                    rz[:].rearrange("p (h o) -> p o h", o=1).to_broadcast([128, 4, 4]),
                    OP.mult)
                att_ps = misc_ps[0:16, 64:192]
                nc.tensor.transpose(att_ps, att_pm[:], ident[:])
                att_sh = wpool.tile([16, 128], BF16, tag="att_sh", bufs=4)
                nc.vector.tensor_copy(att_sh[:], att_ps)
                att_dr = drpool.tile([16, 128], BF16, tag="att_dr")
                nc.sync.dma_start(att_dr[:], att_sh[:])
                # broadcast att rows to all 128 partitions: [128, (h,s,p) 2048]
                attB = wpool.tile([128, 2048], BF16, tag="attB", bufs=4)
                nc.sync.dma_start(
                    attB[:],
                    att_dr[:].rearrange("r n -> (r n)")
                    .rearrange("(o f) -> o f", o=1).to_broadcast([128, 2048]))

                return dict(attB=attB)

            # ============ phase B2: v-side MLP, attention apply, output ======
            def phase_b2(ci, j, a, b):
                n0 = ci * CHUNK
                SCt, sincc, c_kT = a["SCt"], a["sincc"], a["c_kT"]
                attB = b["attB"]
                h1v_ps = psA.tile([128, 512], F32, tag="A")
                nc.tensor.matmul(h1v_ps[:], Wv1_sin, SCt[64:128, 0:512], start=True, stop=False)
                nc.tensor.matmul(h1v_ps[:], Wv1_cos, SCt[64:128, 512:1024], start=False, stop=False)
                nc.tensor.matmul(h1v_ps[:], Wv1_cc[:], sincc[:], start=False, stop=True)
                h1v = wpool.tile([128, 512], F32R, tag="h1v", bufs=3)
                nc.scalar.activation(h1v[:], h1v_ps[:], AF.Gelu, bias=bv1_t[:, 0:1])
                vg_ps = psA.tile([128, 512], F32, tag="A")
                nc.tensor.matmul(vg_ps[:], Wv2_t[:, 0:H], h1v[:], start=True, stop=True)
                vb_ps = psA.tile([128, 512], F32, tag="A")
                nc.tensor.matmul(vb_ps[:], Wv2_t[:, H:2 * H], h1v[:], start=True, stop=True)
                utmp = wpool.tile([128, 512], F32, tag="utmp", bufs=2)
                nc.vector.scalar_tensor_tensor(utmp[:], vg_ps[:], bv2_t[:, 0:1],
                                               c_kT[:], OP.add, OP.mult)
                u_bf = wpool.tile([128, 512], BF16, tag="u_bf", bufs=3)
                nc.vector.scalar_tensor_tensor(u_bf[:], vb_ps[:], bv2_t[:, 1:2],
                                               utmp[:], OP.add, OP.add)

                # ---- apply attention + output MLP ----
                uw = wpool.tile([128, 2048], BF16, tag="uw", bufs=3)
                for h in range(NH):
                    eng = nc.gpsimd if h < 1 else nc.vector
                    eng.tensor_tensor(uw[:, h * 512:(h + 1) * 512], u_bf[:],
                                      attB[:, h * 512:(h + 1) * 512], OP.mult)
                y_ps = psA.tile([128, 512], F32, tag="A")
                for h in range(NH):
                    for s in range(K):
                        nc.tensor.matmul(
                            y_ps[:, h * 128:(h + 1) * 128],
                            Wv_bf[:, h * 128:(h + 1) * 128],
                            uw[:, h * 512 + s * 128:h * 512 + (s + 1) * 128],
                            start=(s == 0), stop=(s == 3))
                y_bf = wpool.tile([128, 512], BF16, tag="y_bf", bufs=3)
                nc.scalar.copy(y_bf[:], y_ps[:])
                y1_ps = psA.tile([128, 512], F32, tag="A")
                for f2 in range(4):
                    for h in range(4):
                        nc.tensor.matmul(
                            y1_ps[:, f2 * 128:(f2 + 1) * 128],
                            Wo1_bf[:, h * 512 + f2 * 128:h * 512 + (f2 + 1) * 128],
                            y_bf[:, h * 128:(h + 1) * 128],
                            start=(h == 0), stop=(h == 3))
                y1 = wpool.tile([128, 512], BF16, tag="y1", bufs=3)
                for f2 in range(4):
                    nc.scalar.activation(y1[:, f2 * 128:(f2 + 1) * 128],
                                         y1_ps[:, f2 * 128:(f2 + 1) * 128],
                                         AF.Gelu, bias=bo1p[:, f2:f2 + 1])
                if j == GS - 1:
                    # refresh the ACT-ordering token after this group's gelus
                    nc.scalar.activation(tok[:], y1[:, 0:1], AF.Copy, scale=0.0)
                misc2_ps = psS.tile([128, 512], F32, tag="S", name="misc2_s")
                o_ps = misc2_ps[0:3, 0:128]
                for c2 in range(4):
                    nc.tensor.matmul(o_ps, Wo2_bf[:, c2 * 3:(c2 + 1) * 3],
                                     y1[:, c2 * 128:(c2 + 1) * 128],
                                     start=(c2 == 0), stop=(c2 == 3))
                o_sb = wpool.tile([3, 128], F32, tag="o_sb", bufs=3)
                nc.scalar.activation(o_sb[:], o_ps, AF.Identity, bias=bo2_t[:, 0:1])
                nc.sync.dma_start(outd[n0:n0 + 128, :].rearrange("n c -> c n"), o_sb[:])

            # ============ main loop: groups of GS chunks, A then B1/B2 =======
            # B1(j+1) is emitted before B2(j) so the next chunk's q-side MLP
            # fills the attention-broadcast DMA latency.
            for g in range(nchunk // GS):
                a1s = [phase_a1(g * GS, 0)]
                acc = []
                for j in range(GS):
                    if j + 1 < GS:
                        a1s.append(phase_a1(g * GS + j + 1, j + 1))
                    acc.append(phase_a2(g * GS + j, j, a1s[j]))
                bts = [phase_b1(g * GS + i, i, acc[i]) for i in range(3)]
                for j in range(GS):
                    if j + 3 < GS:
                        bts.append(phase_b1(g * GS + j + 3, j + 3, acc[j + 3]))
                    phase_b2(g * GS + j, j, acc[j], bts[j])

    nc.compile()
    return nc


def make_in_maps(inputs):
    x = np.asarray(inputs["x"], np.float32)
    f = {k: np.asarray(v, np.float32) for k, v in inputs.items()}

    # ---- host-side precompute of weight/latent-derived constants ----
    wcom = {k: np.ascontiguousarray(f[k]) for k in
            ["Wq1", "bq1", "Wq2", "bq2", "Wv1", "bv1", "Wv2", "bv2",
             "Wv", "Wo1", "Wo2", "bo2"]}
    wcom["Wsin"] = np.ascontiguousarray(
        -0.5 * np.concatenate([f["Wq_sin"], f["Wv_sin"]], axis=1))
    bias_e = np.concatenate([0.5 * f["Wq_sin"].sum(0), 0.5 * f["Wv_sin"].sum(0)])
    wcom["bias_e"] = np.ascontiguousarray(bias_e.reshape(H, 1))
    bo1p = f["bo1"] + f["Wo1"].T @ f["bv"]
    wcom["bo1p"] = np.ascontiguousarray(bo1p.reshape(4, 128).T)

    in_maps = []
    for core in range(NCORE):
        b = core // (NCORE // B)
        sh = (core % (NCORE // B))
        m = dict(wcom)
        m["x"] = np.ascontiguousarray(x[b, sh * NPC:(sh + 1) * NPC])
        p, c, g = f["p"][b], f["c"][b], f["g"][b]
        cstem = c @ f["W_stem"] + f["b_stem"]          # [L, H]
        m["c_tab"] = np.ascontiguousarray(cstem)
        m["k_tab"] = np.ascontiguousarray(cstem @ f["Wk"] + f["bk"])
        sm = np.concatenate([p, 1.0 / (g * g)], axis=1)  # [L, 3]
        m["sm_tab"] = np.ascontiguousarray(sm)
        pB = np.concatenate([p[:, 0], p[:, 1]])          # [2L]
        m["pB"] = np.ascontiguousarray(np.broadcast_to(pB, (128, 2 * L)))
        in_maps.append(m)
    return in_maps


def kernel(**inputs):
    import jax
    try:
        jax.config.update('jax_platforms', 'axon,cpu')
    except Exception:
        pass
    from concourse.bass_utils import run_bass_kernel_spmd

    nchunk = NPC // CHUNK
    if nchunk not in _cache:
        _cache[nchunk] = _build(nchunk)
    nc = _cache[nchunk]

    in_maps = make_in_maps(inputs)
    res = run_bass_kernel_spmd(nc, in_maps, core_ids=list(range(NCORE)))
    out = np.zeros((B, N, DOUT), np.float32)
    for core in range(NCORE):
        b = core // (NCORE // B)
        sh = core % (NCORE // B)
        out[b, sh * NPC:(sh + 1) * NPC] = res.results[core]["out"]
    return out
